# revision 32
# baseline (speedup 1.0000x reference)
"""Causal self-attention (B=4, T=2048, C=1024, H=16) on 8 Trainium2 cores.

Sharding: core c -> batch b = c//2, head-group g = c%2 (8 heads each,
tensor-parallel). QKV + attention + c_proj computed per core on its head
slice; partial c_proj outputs of a (b) pair are summed with chunked
on-device ReduceScatters over the T dimension; host reassembles.

v2: x is pre-transposed/cast to bf16 on the host (layout only, like the
weight reshapes), AV uses v-stationary matmuls streaming 512 queries so
the PE array stays busy (HAM warm), softmax denominators ride as a ones
column of v, and the causal mask is applied in-place on PSUM with one
affine_select per diagonal block.

Self-contained: only imports concourse (installed library) + numpy.
"""

import ml_dtypes
import numpy as np

import concourse.mybir as mybir
import concourse.tile as tile
from concourse import bacc
from concourse.bass_utils import run_bass_kernel_spmd
from concourse.masks import make_identity

B, T, C = 4, 2048, 1024
H_TOTAL, D = 16, 64
N_CORES = 8
HL = H_TOTAL // 2  # local heads per core (8)
HC = HL * D  # local head cols (512)
NP = HL // 2  # head pairs (4)
P = 128
TT = T // P  # 16 t-chunks of 128
CK = C // P  # 8 contraction chunks for qkv
RS_CHUNKS = 4
F32 = mybir.dt.float32
BF16 = mybir.dt.bfloat16
MASK_VAL = -480.0  # -60 after the 1/8 attention scale; exp(-60) ~ 0
SCALE = 1.0 / 8.0  # 1/sqrt(D)

_CACHE = {}
_DEBUG_DUMP = False


def _build_nc():
    nc = bacc.Bacc("TRN2", target_bir_lowering=False, debug=False, num_devices=N_CORES)

    # x pre-transposed and cast on host: [ki, ck, t]
    xT_d = nc.dram_tensor("xT", [4, P, CK, 512], BF16, kind="ExternalInput")
    wq_d = nc.dram_tensor("wq", [P, NP, CK, P], BF16, kind="ExternalInput")
    wk_d = nc.dram_tensor("wk", [P, NP, CK, P], BF16, kind="ExternalInput")
    wv_d = nc.dram_tensor("wv", [P, CK, HC], BF16, kind="ExternalInput")
    bq_d = nc.dram_tensor("bq", [P, NP], F32, kind="ExternalInput")
    bk_d = nc.dram_tensor("bk", [P, NP], F32, kind="ExternalInput")
    bv_d = nc.dram_tensor("bv", [P, HC], BF16, kind="ExternalInput")
    wp_d = nc.dram_tensor("wp", [P, HC // P, C], BF16, kind="ExternalInput")
    bp_d = nc.dram_tensor("bp", [P, C], BF16, kind="ExternalInput")
    out_d = nc.dram_tensor("out", [T // 2, C], BF16, kind="ExternalOutput")

    with tile.TileContext(nc) as tc:
        with (
            tc.tile_pool(name="const", bufs=1) as constp,
            tc.tile_pool(name="big", bufs=1) as bigp,
            tc.tile_pool(name="rnorm", bufs=1) as rnp,
            tc.tile_pool(name="ostage", bufs=1) as ostp,
            tc.tile_pool(name="zout", bufs=1) as zoutp,
            tc.tile_pool(name="score_ps", bufs=1, space="PSUM") as score_ps,
            tc.tile_pool(name="avmm_ps", bufs=2, space="PSUM") as avmm_ps,
            tc.tile_pool(name="dram", bufs=1, space="DRAM") as dramp,
        ):
            # ---- constants ----
            # dmaskT @ ident seeds the diagonal PSUM block with the causal
            # mask on the PE itself (keeps DVE out of the score->exp chain):
            # dmaskT[p, c] = MASK_VAL where p < c, so (dmaskT^T I)[s, c] =
            # dmaskT[c, s] = MASK_VAL where c < s.
            dmaskT = constp.tile([P, P], BF16)
            nc.vector.memset(dmaskT[:], 0.0)
            nc.gpsimd.affine_select(
                out=dmaskT[:],
                in_=dmaskT[:],
                compare_op=mybir.AluOpType.is_ge,
                fill=MASK_VAL,
                base=0,
                pattern=[[-1, P]],
                channel_multiplier=1,
            )
            ident_bf = constp.tile([P, P], BF16)
            make_identity(nc, ident_bf)
            bq_sb = constp.tile([P, NP], F32)
            nc.sync.dma_start(bq_sb[:], bq_d[:])
            bk_sb = constp.tile([P, NP], F32)
            nc.sync.dma_start(bk_sb[:], bk_d[:])
            bv_sb, bv_free = tc.tile([P, HC], BF16, name="bv_sb")
            nc.sync.dma_start(bv_sb[:], bv_d[:])

            # ---- persistent activations ----
            qT = bigp.tile([P, NP, T], BF16)  # q^T [qcol, t]
            kT = bigp.tile([P, NP, T], BF16)  # k^T [kcol, t]
            v_ext = bigp.tile([P, TT, HL, D + 1], BF16)  # v with ones col
            nc.vector.memset(v_ext[:, :, :, D : D + 1], 1.0)
            yT = bigp.tile([P, NP, T], BF16)  # y^T [ci, t]
            # p/xT arena: 64K bf16 elems per partition (128 KB).
            #   u0 p-pair slots (8x2048 = 16K elems): j%3 -> [0,16K),[16K,32K),[32K,48K)
            #   xT (8x2048): [48K, 64K)
            #   u1 p-pair slots (16x2048 = 32K elems): j%2 -> [0,32K),[32K,64K)
            arena = bigp.tile([P, 64 * 1024], BF16)

            def p_view(j, u):
                if u == 0:
                    off = 16384 * (j % 3)
                    return arena[:, off : off + 16384].rearrange(
                        "p (i q) -> p i q", q=2048
                    )
                off = 32768 * (j % 2)
                return arena[:, off : off + 32768].rearrange(
                    "p (i q) -> p i q", q=2048
                )

            xT = arena[:, 49152:65536].rearrange("p (c t) -> p c t", t=T)

            # xT DMA in t-quarters (each contiguous in DRAM) so qkproj(0)
            # starts after ~3 us
            for q4 in range(4):
                nc.sync.dma_start(
                    xT[:, :, q4 * 512 : (q4 + 1) * 512],
                    xT_d[q4],
                )

            # ---- QKV projections ----
            def qkproj(j):
                for w_d, b_sb, dstT in ((wq_d, bq_sb, qT), (wk_d, bk_sb, kT)):
                    wj, wj_free = tc.tile([P, CK, P], BF16, name=f"w{j}")
                    nc.sync.dma_start(wj[:], w_d[:, j])
                    for u4 in range(T // 512):
                        ps = avmm_ps.tile([P, 1024], F32, tag="avmm", name="qk_ps")
                        for ck in range(CK):
                            nc.tensor.matmul(
                                ps[:, 0:512],
                                wj[:, ck, :],
                                xT[:, ck, u4 * 512 : (u4 + 1) * 512],
                                start=(ck == 0),
                                stop=(ck == CK - 1),
                            )
                        nc.vector.tensor_add(
                            out=dstT[:, j, u4 * 512 : (u4 + 1) * 512],
                            in0=ps[:, 0:512],
                            in1=b_sb[:, j : j + 1].to_broadcast((P, 512)),
                        )
                    wj_free()

            # wv lives in u0 p-slot 2 of the arena: all reads (vproj) are
            # scheduled before score_exp(2, 0) overwrites that slot.
            wv_sb = arena[:, 32768:36864].rearrange("p (c v) -> p c v", v=HC)
            nc.gpsimd.dma_start(wv_sb[:], wv_d[:])

            def vproj(tt_lo, tt_hi):
                for tt in range(tt_lo, tt_hi):
                    ps = avmm_ps.tile([P, 1024], F32, tag="avmm", name="v_ps")
                    for ck in range(CK):
                        nc.tensor.matmul(
                            ps[:, 0:512],
                            xT[:, ck, tt * P : (tt + 1) * P],
                            wv_sb[:, ck, :],
                            start=(ck == 0),
                            stop=(ck == CK - 1),
                        )
                    nc.vector.tensor_add(
                        out=v_ext[:, tt, :, 0:D],
                        in0=ps[:, 0:512].rearrange("p (h d) -> p h d", d=D),
                        in1=bv_sb[:].rearrange("p (h d) -> p h d", d=D),
                    )

            # ---- scores + exp ----
            def score_exp(j, u):
                n_i = 8 * (u + 1)
                pt = p_view(j, u)
                # one PSUM tile per head: double-buffered across units so the
                # next unit's matmuls run while this unit's exp drains. The
                # two heads' matmuls still pair up via row groups 0/64.
                ps2 = [
                    score_ps.tile([P, 1024], F32, tag="score", name=f"sc{hh}")
                    for hh in range(2)
                ]
                for i in range(n_i):
                    d0 = i * P - 1024 * u  # diag block col (within unit)
                    c0 = max(0, d0)
                    jj_diag = 2 * u + d0 // 512 if i >= 8 * u else -1
                    for hh in range(2):
                        hb = hh * D
                        for jj in range(2 * u, 2 * u + 2):
                            if jj < i // 4:
                                continue
                            c0j = (jj - 2 * u) * 512
                            if jj != jj_diag:
                                nc.tensor.matmul(
                                    ps2[hh][:, c0j : c0j + 512],
                                    kT[hb : hb + D, j, i * P : (i + 1) * P],
                                    qT[hb : hb + D, j, jj * 512 : (jj + 1) * 512],
                                    start=True,
                                    stop=True,
                                )
                                continue
                            # diag block: seed [d0, d0+128) with the causal
                            # mask, accumulate scores on top; cols left of the
                            # diag are above-diagonal (never exp'd/streamed),
                            # cols right of it get their own fresh matmul.
                            nc.tensor.matmul(
                                ps2[hh][:, d0 : d0 + P],
                                dmaskT[:],
                                ident_bf[:],
                                start=True,
                                stop=False,
                            )
                            nc.tensor.matmul(
                                ps2[hh][:, d0 : d0 + P],
                                kT[hb : hb + D, j, i * P : (i + 1) * P],
                                qT[hb : hb + D, j, d0 + 1024 * u : d0 + 1024 * u + P],
                                start=False,
                                stop=True,
                            )
                            hi = c0j + 512
                            if d0 + P < hi:
                                nc.tensor.matmul(
                                    ps2[hh][:, d0 + P : hi],
                                    kT[hb : hb + D, j, i * P : (i + 1) * P],
                                    qT[
                                        hb : hb + D,
                                        j,
                                        d0 + P + 1024 * u : hi + 1024 * u,
                                    ],
                                    start=True,
                                    stop=True,
                                )
                    for hh in range(2):
                        nc.scalar.activation(
                            out=pt[:, i, hh * 1024 + c0 : (hh + 1) * 1024],
                            in_=ps2[hh][:, c0:1024],
                            func=mybir.ActivationFunctionType.Exp,
                            scale=SCALE,
                        )

            # ---- attention-value product, v-stationary ----
            # out[0:64] = y^T (unnormalized), out[64] = softmax denominator
            # via the ones column of v_ext; p streams 512 queries per matmul.
            def av_unit(j, u, chunks=None):
                pt = p_view(j, u)
                for c in chunks if chunks is not None else (2 * u, 2 * u + 1):
                    i_max = 4 * (c + 1)
                    qo = (c - 2 * u) * 512
                    av = avmm_ps.tile([P, 1024], F32, tag="avmm", name=f"av{j}{c}")
                    for hh in range(2):
                        h = 2 * j + hh
                        for i in range(i_max):
                            # stream only cols at/after the causal boundary:
                            # p[:, i, :lo] above the diagonal is never written
                            lo = max(qo, i * P - 1024 * u)
                            nc.tensor.matmul(
                                av[0 : D + 1, hh * 512 + lo - qo : hh * 512 + 512],
                                v_ext[:, i, h, :],
                                pt[:, i, hh * 1024 + lo : hh * 1024 + qo + 512],
                                start=(i == 0),
                                stop=(i == i_max - 1),
                            )
                    # normalize: y * (1/denom); recips for both heads land on
                    # lanes 0-63 (rb cols 0-511 even head, 512-1023 odd head)
                    # Normalization: DVE reciprocal costs ~9 cyc/free-elem
                    # on ONE lane, so recip the denominators as a [128, 8]
                    # column (lane-parallel, ~0.15us) instead of a [1, 1024]
                    # row (6.5us). Rows<->columns via transposing DRAM APs;
                    # all legs ride the vector engine's own DMA queue.
                    rn = rnp.tile([P, 1024], BF16, tag="rn", name="rnorm")
                    nc.vector.tensor_copy(
                        out=rn[D : D + 1, :], in_=av[D : D + 1, :]
                    )
                    d1 = dramp.tile([1, 1024], BF16, tag="d1", name="d1", bufs=2)
                    nc.sync.dma_start(d1[:], rn[D : D + 1, :])
                    dcol = rnp.tile([P, 8], BF16, tag="dcol", name="dcol")
                    nc.sync.dma_start(
                        dcol[:], d1[0:1, :].rearrange("o (c p) -> (o p) c", p=P)
                    )
                    rcol = rnp.tile([P, 8], BF16, tag="rcol", name="rcol")
                    with nc.allow_low_precision(
                        reason="bf16 1/denom: ~0.4% rel on y, within tolerance"
                    ):
                        nc.vector.reciprocal(rcol[:], dcol[:])
                    d2 = dramp.tile([1, 1024], BF16, tag="d2", name="d2", bufs=2)
                    nc.sync.dma_start(
                        d2[0:1, :].rearrange("o (c p) -> (o p) c", p=P), rcol[:]
                    )
                    rb = rn[0:D, :]
                    nc.sync.dma_start(rb, d2[0:1, :].to_broadcast((D, 1024)))
                    nc.vector.tensor_mul(
                        out=yT[0:D, j, c * 512 : (c + 1) * 512],
                        in0=av[0:D, 0:512],
                        in1=rb[:, 0:512],
                    )
                    st = ostp.tile([D, 512], BF16, tag="ost", name="ostage")
                    nc.vector.tensor_mul(
                        out=st[:],
                        in0=av[0:D, 512:1024],
                        in1=rb[:, 512:1024],
                    )
                    # partition shift: odd head's y lives on lanes 0-63,
                    # belongs at yT rows 64-127
                    nc.sync.dma_start(
                        yT[D:P, j, c * 512 : (c + 1) * 512], st[:]
                    )
                    if _DEBUG_DUMP and (j, c) == (0, 0):
                        rn_o = nc.dram_tensor(
                            "rn_o", [P, 1024], BF16, kind="ExternalOutput"
                        )
                        nc.sync.dma_start(rn_o[:], rn[:])

            # ---- c_proj + ReduceScatter ----
            z_dram = dramp.tile([T, C], BF16)
            rs_out = dramp.tile([T // 2, C], BF16)
            rows = T // RS_CHUNKS  # 512
            half = rows // 2  # 256

            def proj_mm(tt, wp_sb, bp_sb):
                for n in range(C // 512):
                    ps = avmm_ps.tile([P, 1024], F32, tag="avmm", name="pj_ps")
                    for cc in range(HC // P):
                        nc.tensor.matmul(
                            ps[:, 0:512],
                            yT[:, cc, tt * P : (tt + 1) * P],
                            wp_sb[:, cc, n * 512 : (n + 1) * 512],
                            start=(cc == 0),
                            stop=(cc == HC // P - 1),
                        )
                    z_sb = zoutp.tile([P, 512], BF16, tag="z", name="z_sb")
                    nc.vector.tensor_add(
                        out=z_sb[:],
                        in0=ps[:, 0:512],
                        in1=bp_sb[:, n * 512 : (n + 1) * 512],
                    )
                    nc.sync.dma_start(
                        z_dram[tt * P : (tt + 1) * P, n * 512 : (n + 1) * 512],
                        z_sb[:],
                    )

            def rs_tail(rc):
                nc.gpsimd.collective_compute(
                    "ReduceScatter",
                    mybir.AluOpType.add,
                    replica_groups=[[0, 1], [2, 3], [4, 5], [6, 7]],
                    ins=[z_dram[rc * rows : (rc + 1) * rows, :].opt()],
                    outs=[rs_out[rc * half : (rc + 1) * half, :].opt()],
                )
                nc.sync.dma_start(
                    out_d[rc * half : (rc + 1) * half, :],
                    rs_out[rc * half : (rc + 1) * half, :],
                )

            def proj_rs(rc, wp_sb, bp_sb):
                tt_per_chunk = TT // RS_CHUNKS
                for tt in range(rc * tt_per_chunk, (rc + 1) * tt_per_chunk):
                    proj_mm(tt, wp_sb, bp_sb)
                rs_tail(rc)

            # ---- schedule ----
            qkproj(0)
            score_exp(0, 0)
            qkproj(1)
            vproj(0, 8)
            score_exp(1, 0)
            av_unit(0, 0)
            qkproj(2)
            vproj(8, 16)
            score_exp(2, 0)
            av_unit(1, 0)
            qkproj(3)
            score_exp(3, 0)
            av_unit(2, 0)
            av_unit(3, 0)

            bv_free()
            wp_sb, wp_free = tc.tile([P, HC // P, C], BF16, name="wp_sb")
            nc.gpsimd.dma_start(wp_sb[:], wp_d[:])
            bp_sb, bp_free = tc.tile([P, C], BF16, name="bp_sb")
            nc.gpsimd.dma_start(bp_sb[:], bp_d[:])

            # u1 phase is ACT(exp)-bound: spread proj(0)/proj(1) tt-slices
            # across the unit boundaries as PE filler.
            score_exp(0, 1)
            proj_mm(0, wp_sb, bp_sb)
            proj_mm(1, wp_sb, bp_sb)
            score_exp(1, 1)
            proj_mm(2, wp_sb, bp_sb)
            proj_mm(3, wp_sb, bp_sb)
            rs_tail(0)
            av_unit(0, 1)
            proj_mm(4, wp_sb, bp_sb)
            score_exp(2, 1)
            proj_mm(5, wp_sb, bp_sb)
            proj_mm(6, wp_sb, bp_sb)
            av_unit(1, 1)
            proj_mm(7, wp_sb, bp_sb)
            rs_tail(1)
            score_exp(3, 1)
            # chunk-major tail: finish every head pair's chunk 2 first so
            # proj/RS of rows 1024-1535 overlap the chunk-3 AV work
            av_unit(2, 1, chunks=(2,))
            av_unit(3, 1, chunks=(2,))
            for tt in range(8, 12):
                proj_mm(tt, wp_sb, bp_sb)
            rs_tail(2)
            av_unit(2, 1, chunks=(3,))
            av_unit(3, 1, chunks=(3,))
            for tt in range(12, 16):
                proj_mm(tt, wp_sb, bp_sb)
            rs_tail(3)
            bp_free()
            wp_free()

            if _DEBUG_DUMP:
                qT_o = nc.dram_tensor("qT_o", [P, NP, T], BF16, kind="ExternalOutput")
                kT_o = nc.dram_tensor("kT_o", [P, NP, T], BF16, kind="ExternalOutput")
                v_o = nc.dram_tensor(
                    "v_o", [P, TT, HL, D + 1], BF16, kind="ExternalOutput"
                )
                yT_o = nc.dram_tensor("yT_o", [P, NP, T], BF16, kind="ExternalOutput")
                ar_o = nc.dram_tensor(
                    "ar_o", [P, 64 * 1024], BF16, kind="ExternalOutput"
                )
                nc.sync.dma_start(qT_o[:], qT[:])
                nc.sync.dma_start(kT_o[:], kT[:])
                nc.sync.dma_start(v_o[:], v_ext[:])
                nc.sync.dma_start(yT_o[:], yT[:])
                nc.sync.dma_start(ar_o[:], arena[:])

    nc.compile()
    return nc


def _in_maps(inputs):
    x = np.asarray(inputs["x"], dtype=np.float32)
    w_attn = np.asarray(inputs["w_attn"], dtype=np.float32)
    b_attn = np.asarray(inputs["b_attn"], dtype=np.float32)
    w_proj = np.asarray(inputs["w_proj"], dtype=np.float32)
    b_proj = np.asarray(inputs["b_proj"], dtype=np.float32)

    maps = []
    for core in range(N_CORES):
        b, g = core // 2, core % 2
        s = g * HC
        # x[b] [T, C] -> xT [q4, ki, ck, t%512] with c = ck*128 + ki,
        # each t-quarter contiguous for clean DMA
        xT = (
            x[b]
            .reshape(4, 512, CK, P)
            .transpose(0, 3, 2, 1)
            .astype(ml_dtypes.bfloat16)
        )
        # [C, HC] -> [ki, j, ko, n] with c = ko*128+ki, qcol = j*128+n
        wq = (
            w_attn[:, s : s + HC]
            .reshape(CK, P, NP, P)
            .transpose(1, 2, 0, 3)
            .astype(ml_dtypes.bfloat16)
        )
        wk = (
            w_attn[:, C + s : C + s + HC]
            .reshape(CK, P, NP, P)
            .transpose(1, 2, 0, 3)
            .astype(ml_dtypes.bfloat16)
        )
        # [C, HC] -> [ki, ko, vcol]
        wv = (
            w_attn[:, 2 * C + s : 2 * C + s + HC]
            .reshape(CK, P, HC)
            .transpose(1, 0, 2)
            .astype(ml_dtypes.bfloat16)
        )
        # [HC, C] -> [ki, ko, co], bf16
        wp = (
            w_proj[s : s + HC, :]
            .reshape(HC // P, P, C)
            .transpose(1, 0, 2)
            .astype(ml_dtypes.bfloat16)
        )
        bq = b_attn[s : s + HC].reshape(NP, P).T
        bk = b_attn[C + s : C + s + HC].reshape(NP, P).T
        bv = np.broadcast_to(
            b_attn[2 * C + s : 2 * C + s + HC], (P, HC)
        ).astype(ml_dtypes.bfloat16)
        bp = (
            np.broadcast_to(b_proj, (P, C)).astype(ml_dtypes.bfloat16)
            if g == 0
            else np.zeros((P, C), ml_dtypes.bfloat16)
        )
        maps.append(
            {
                "xT": np.ascontiguousarray(xT),
                "wq": np.ascontiguousarray(wq),
                "wk": np.ascontiguousarray(wk),
                "wv": np.ascontiguousarray(wv),
                "wp": np.ascontiguousarray(wp),
                "bq": np.ascontiguousarray(bq),
                "bk": np.ascontiguousarray(bk),
                "bv": np.ascontiguousarray(bv),
                "bp": np.ascontiguousarray(bp),
            }
        )
    return maps


def _run(inputs, trace=False, trace_cores=None):
    if "nc" not in _CACHE:
        _CACHE["nc"] = _build_nc()
    nc = _CACHE["nc"]
    res = run_bass_kernel_spmd(
        nc,
        _in_maps(inputs),
        list(range(N_CORES)),
        trace=trace,
        trace_cores=trace_cores,
    )
    # chunked RS ownership: even core holds rows [512c, 512c+256),
    # odd core holds rows [512c+256, 512c+512), for c = 0..3
    out = np.empty((B, T, C), np.float32)
    rows = T // RS_CHUNKS
    half = rows // 2
    for b in range(B):
        ev = res.results[2 * b]["out"].astype(np.float32)
        od = res.results[2 * b + 1]["out"].astype(np.float32)
        for rc in range(RS_CHUNKS):
            out[b, rc * rows : rc * rows + half] = ev[rc * half : (rc + 1) * half]
            out[b, rc * rows + half : (rc + 1) * rows] = od[
                rc * half : (rc + 1) * half
            ]
    return out, res


def kernel(**inputs):
    out, _ = _run(inputs)
    return out


# revision 39
# speedup vs baseline: 1.1827x; 1.1827x over previous
"""Causal self-attention (B=4, T=2048, C=1024, H=16) on 8 Trainium2 cores.

Sharding: core c -> batch b = c//2, head-group g = c%2 (8 heads each,
tensor-parallel). QKV + attention + c_proj computed per core on its head
slice; partial c_proj outputs of a (b) pair are summed with chunked
on-device ReduceScatters over the T dimension; host reassembles.

v2: x is pre-transposed/cast to bf16 on the host (layout only, like the
weight reshapes), AV uses v-stationary matmuls streaming 512 queries so
the PE array stays busy (HAM warm), softmax denominators ride as a ones
column of v, and the causal mask is applied in-place on PSUM with one
affine_select per diagonal block.

Self-contained: only imports concourse (installed library) + numpy.
"""

import ml_dtypes
import numpy as np

import concourse.mybir as mybir
import concourse.tile as tile
from concourse import bacc
from concourse.bass_utils import run_bass_kernel_spmd
from concourse.masks import make_identity

B, T, C = 4, 2048, 1024
H_TOTAL, D = 16, 64
N_CORES = 8
HL = H_TOTAL // 2  # local heads per core (8)
HC = HL * D  # local head cols (512)
NP = HL // 2  # head pairs (4)
P = 128
TT = T // P  # 16 t-chunks of 128
CK = C // P  # 8 contraction chunks for qkv
RS_CHUNKS = 4
F32 = mybir.dt.float32
BF16 = mybir.dt.bfloat16
MASK_VAL = -480.0  # -60 after the 1/8 attention scale; exp(-60) ~ 0
SCALE = 1.0 / 8.0  # 1/sqrt(D)

_CACHE = {}
_DEBUG_DUMP = False


def _build_nc():
    nc = bacc.Bacc("TRN2", target_bir_lowering=False, debug=False, num_devices=N_CORES)

    # x pre-transposed and cast on host: [ki, ck, t]
    xT_d = nc.dram_tensor("xT", [4, P, CK, 512], BF16, kind="ExternalInput")
    wq_d = nc.dram_tensor("wq", [P, NP, CK, P], BF16, kind="ExternalInput")
    wk_d = nc.dram_tensor("wk", [P, NP, CK, P], BF16, kind="ExternalInput")
    wv_d = nc.dram_tensor("wv", [P, CK, HC], BF16, kind="ExternalInput")
    bq_d = nc.dram_tensor("bq", [P, NP], F32, kind="ExternalInput")
    bk_d = nc.dram_tensor("bk", [P, NP], F32, kind="ExternalInput")
    bv_d = nc.dram_tensor("bv", [P, HC], BF16, kind="ExternalInput")
    wp_d = nc.dram_tensor("wp", [P, HC // P, C], BF16, kind="ExternalInput")
    bp_d = nc.dram_tensor("bp", [P, C], mybir.dt.float8e4, kind="ExternalInput")
    out_d = nc.dram_tensor("out", [T // 2, C], BF16, kind="ExternalOutput")

    with tile.TileContext(nc) as tc:
        with (
            tc.tile_pool(name="const", bufs=1) as constp,
            tc.tile_pool(name="big", bufs=1) as bigp,
            tc.tile_pool(name="rnorm", bufs=1) as rnp,
            tc.tile_pool(name="zout", bufs=1) as zoutp,
            tc.tile_pool(name="score_ps", bufs=1, space="PSUM") as score_ps,
            tc.tile_pool(name="avmm_ps", bufs=2, space="PSUM") as avmm_ps,
            tc.tile_pool(name="dram", bufs=1, space="DRAM") as dramp,
        ):
            # ---- constants ----
            # dmaskT @ ident seeds the diagonal PSUM block with the causal
            # mask on the PE itself (keeps DVE out of the score->exp chain):
            # dmaskT[p, c] = MASK_VAL where p < c, so (dmaskT^T I)[s, c] =
            # dmaskT[c, s] = MASK_VAL where c < s.
            dmaskT = constp.tile([P, P], BF16)
            nc.vector.memset(dmaskT[:], 0.0)
            nc.gpsimd.affine_select(
                out=dmaskT[:],
                in_=dmaskT[:],
                compare_op=mybir.AluOpType.is_ge,
                fill=MASK_VAL,
                base=0,
                pattern=[[-1, P]],
                channel_multiplier=1,
            )
            ident_bf = constp.tile([P, P], BF16)
            make_identity(nc, ident_bf)
            bq_sb = constp.tile([P, NP], F32)
            nc.sync.dma_start(bq_sb[:], bq_d[:])
            bk_sb = constp.tile([P, NP], F32)
            nc.sync.dma_start(bk_sb[:], bk_d[:])
            # reserve wp space early (needs 8KB contiguous; DMA'd later)
            wp_sb, wp_free = tc.tile([P, HC // P, C], BF16, name="wp_sb")

            # ---- persistent activations ----
            qT = bigp.tile([P, NP, T], BF16)  # q^T [qcol, t]
            kT = bigp.tile([P, NP, T], BF16)  # k^T [kcol, t]
            v_ext = bigp.tile([P, TT, HL, D + 1], BF16)  # v with ones col
            nc.vector.memset(v_ext[:, :, :, D : D + 1], 1.0)
            yT = bigp.tile([P, NP, T], BF16)  # y^T [ci, t]
            # p/xT arena: 64K bf16 elems per partition (128 KB).
            #   u0 p-pair slots (8x2048 = 16K elems): j%3 -> [0,16K),[16K,32K),[32K,48K)
            #   xT (8x2048): [48K, 64K)
            #   u1 p-pair slots (16x2048 = 32K elems): j%2 -> [0,32K),[32K,64K)
            arena = bigp.tile([P, 64 * 1024], BF16)

            def p_view(j, u):
                if u == 0:
                    off = 16384 * (j % 3)
                    return arena[:, off : off + 16384].rearrange(
                        "p (i q) -> p i q", q=2048
                    )
                off = 32768 * (j % 2)
                return arena[:, off : off + 32768].rearrange(
                    "p (i q) -> p i q", q=2048
                )

            xT = arena[:, 49152:65536].rearrange("p (c t) -> p c t", t=T)

            # xT DMA in t-quarters (each contiguous in DRAM) so qkproj(0)
            # starts after ~3 us
            for q4 in range(4):
                nc.sync.dma_start(
                    xT[:, :, q4 * 512 : (q4 + 1) * 512],
                    xT_d[q4],
                )

            # ---- QKV projections ----
            # transient wq/wk tiles live in wp_sb's space (wp is DMA'd only
            # after the last qkproj read; 4 rotating 2KB slots)
            wp_flat = wp_sb[:].rearrange("p a b -> p (a b)")

            def qkproj(j):
                for si, (w_d, b_sb, dstT) in enumerate(
                    ((wq_d, bq_sb, qT), (wk_d, bk_sb, kT))
                ):
                    slot = (2 * j + si) % 4
                    wj = wp_flat[:, slot * 1024 : (slot + 1) * 1024].rearrange(
                        "p (c k) -> p c k", k=P
                    )
                    nc.sync.dma_start(wj[:], w_d[:, j])
                    for u4 in range(T // 512):
                        ps = avmm_ps.tile([P, 1024], F32, tag="avmm", name="qk_ps")
                        for ck in range(CK):
                            nc.tensor.matmul(
                                ps[:, 0:512],
                                wj[:, ck, :],
                                xT[:, ck, u4 * 512 : (u4 + 1) * 512],
                                start=(ck == 0),
                                stop=(ck == CK - 1),
                            )
                        nc.vector.tensor_add(
                            out=dstT[:, j, u4 * 512 : (u4 + 1) * 512],
                            in0=ps[:, 0:512],
                            in1=b_sb[:, j : j + 1].to_broadcast((P, 512)),
                        )

            # wv and bv live in u0 p-slot 2 of the arena: all reads (vproj)
            # are scheduled before score_exp(2, 0) overwrites that slot.
            wv_sb = arena[:, 32768:36864].rearrange("p (c v) -> p c v", v=HC)
            nc.gpsimd.dma_start(wv_sb[:], wv_d[:])
            bv_sb = arena[:, 36864:37376]
            nc.sync.dma_start(bv_sb[:], bv_d[:])

            def vproj(tt_lo, tt_hi):
                for tt in range(tt_lo, tt_hi):
                    ps = avmm_ps.tile([P, 1024], F32, tag="avmm", name="v_ps")
                    for ck in range(CK):
                        nc.tensor.matmul(
                            ps[:, 0:512],
                            xT[:, ck, tt * P : (tt + 1) * P],
                            wv_sb[:, ck, :],
                            start=(ck == 0),
                            stop=(ck == CK - 1),
                        )
                    nc.vector.tensor_add(
                        out=v_ext[:, tt, :, 0:D],
                        in0=ps[:, 0:512].rearrange("p (h d) -> p h d", d=D),
                        in1=bv_sb[:].rearrange("p (h d) -> p h d", d=D),
                    )

            # ---- scores + exp ----
            def score_exp(j, u):
                n_i = 8 * (u + 1)
                pt = p_view(j, u)
                # one PSUM tile per head: double-buffered across units so the
                # next unit's matmuls run while this unit's exp drains. The
                # two heads' matmuls still pair up via row groups 0/64.
                ps2 = [
                    score_ps.tile([P, 1024], F32, tag="score", name=f"sc{hh}")
                    for hh in range(2)
                ]
                for i in range(n_i):
                    d0 = i * P - 1024 * u  # diag block col (within unit)
                    c0 = max(0, d0)
                    jj_diag = 2 * u + d0 // 512 if i >= 8 * u else -1
                    for hh in range(2):
                        hb = hh * D
                        for jj in range(2 * u, 2 * u + 2):
                            if jj < i // 4:
                                continue
                            c0j = (jj - 2 * u) * 512
                            if jj != jj_diag:
                                nc.tensor.matmul(
                                    ps2[hh][:, c0j : c0j + 512],
                                    kT[hb : hb + D, j, i * P : (i + 1) * P],
                                    qT[hb : hb + D, j, jj * 512 : (jj + 1) * 512],
                                    start=True,
                                    stop=True,
                                )
                                continue
                            # diag block: seed [d0, d0+128) with the causal
                            # mask, accumulate scores on top; cols left of the
                            # diag are above-diagonal (never exp'd/streamed),
                            # cols right of it get their own fresh matmul.
                            nc.tensor.matmul(
                                ps2[hh][:, d0 : d0 + P],
                                dmaskT[:],
                                ident_bf[:],
                                start=True,
                                stop=False,
                            )
                            nc.tensor.matmul(
                                ps2[hh][:, d0 : d0 + P],
                                kT[hb : hb + D, j, i * P : (i + 1) * P],
                                qT[hb : hb + D, j, d0 + 1024 * u : d0 + 1024 * u + P],
                                start=False,
                                stop=True,
                            )
                            hi = c0j + 512
                            if d0 + P < hi:
                                nc.tensor.matmul(
                                    ps2[hh][:, d0 + P : hi],
                                    kT[hb : hb + D, j, i * P : (i + 1) * P],
                                    qT[
                                        hb : hb + D,
                                        j,
                                        d0 + P + 1024 * u : hi + 1024 * u,
                                    ],
                                    start=True,
                                    stop=True,
                                )
                    for hh in range(2):
                        nc.scalar.activation(
                            out=pt[:, i, hh * 1024 + c0 : (hh + 1) * 1024],
                            in_=ps2[hh][:, c0:1024],
                            func=mybir.ActivationFunctionType.Exp,
                            scale=SCALE,
                        )

            # ---- attention-value product, v-stationary ----
            # out[0:64] = y^T (unnormalized), out[64] = softmax denominator
            # via the ones column of v_ext; p streams 512 queries per matmul.
            def av_unit(j, u, chunks=None):
                pt = p_view(j, u)
                for c in chunks if chunks is not None else (2 * u, 2 * u + 1):
                    i_max = 4 * (c + 1)
                    qo = (c - 2 * u) * 512
                    av = avmm_ps.tile([P, 1024], F32, tag="avmm", name=f"av{j}{c}")
                    for hh in range(2):
                        h = 2 * j + hh
                        for i in range(i_max):
                            # stream only cols at/after the causal boundary:
                            # p[:, i, :lo] above the diagonal is never written
                            lo = max(qo, i * P - 1024 * u)
                            nc.tensor.matmul(
                                av[0 : D + 1, hh * 512 + lo - qo : hh * 512 + 512],
                                v_ext[:, i, h, :],
                                pt[:, i, hh * 1024 + lo : hh * 1024 + qo + 512],
                                start=(i == 0),
                                stop=(i == i_max - 1),
                            )
                    # normalize: y * (1/denom); recips for both heads land on
                    # lanes 0-63 (rb cols 0-511 even head, 512-1023 odd head)
                    # Copy y+denominators out of PSUM right away (frees the
                    # bank for the next chain; PE never waits on the norm).
                    stage = rnp.tile(
                        [D + 1, 1024], BF16, tag="avst", name="avstage", bufs=2
                    )
                    nc.vector.tensor_copy(out=stage[:], in_=av[0 : D + 1, :])
                    # Lane-parallel reciprocal: DVE recip costs ~9 cyc per
                    # FREE element regardless of lanes, so recip the denoms
                    # as a [128, 8] column (0.15us) instead of a [1, 1024]
                    # row (6.5us). Rows<->columns via transposing DRAM APs;
                    # latency is off the critical path now.
                    d1 = dramp.tile([1, 1024], BF16, tag="d1", name="d1", bufs=2)
                    nc.sync.dma_start(d1[:], stage[D : D + 1, :])
                    dcol = rnp.tile([P, 8], BF16, tag="dcol", name="dcol")
                    nc.sync.dma_start(
                        dcol[:], d1[0:1, :].rearrange("o (c p) -> (o p) c", p=P)
                    )
                    with nc.allow_low_precision(
                        reason="bf16 1/denom: ~0.4% rel on y, within tolerance"
                    ):
                        nc.vector.reciprocal(dcol[:], dcol[:])
                    d2 = dramp.tile([1, 1024], BF16, tag="d2", name="d2", bufs=2)
                    nc.sync.dma_start(
                        d2[0:1, :].rearrange("o (c p) -> (o p) c", p=P), dcol[:]
                    )
                    rb = rnp.tile([P, 512], BF16, tag="rb", name="rb")
                    nc.sync.dma_start(
                        rb[0:D, :], d2[0:1, 0:512].to_broadcast((D, 512))
                    )
                    nc.sync.dma_start(
                        rb[D:P, :], d2[0:1, 512:1024].to_broadcast((D, 512))
                    )
                    nc.vector.tensor_mul(
                        out=yT[0:D, j, c * 512 : (c + 1) * 512],
                        in0=stage[0:D, 0:512],
                        in1=rb[0:D, :],
                    )
                    # partition shift the odd head's unnormalized y to
                    # rows 64-127, then normalize in place there
                    nc.sync.dma_start(
                        yT[D:P, j, c * 512 : (c + 1) * 512],
                        stage[0:D, 512:1024],
                    )
                    nc.vector.tensor_mul(
                        out=yT[D:P, j, c * 512 : (c + 1) * 512],
                        in0=yT[D:P, j, c * 512 : (c + 1) * 512],
                        in1=rb[D:P, :],
                    )
                    if _DEBUG_DUMP and (j, c) == (0, 0):
                        rn_o = nc.dram_tensor(
                            "rn_o", [P, 1024], BF16, kind="ExternalOutput"
                        )
                        nc.sync.dma_start(rn_o[:], rn[:])

            # ---- c_proj + ReduceScatter ----
            z_dram = dramp.tile([T, C], BF16)
            rs_out = dramp.tile([T // 2, C], BF16)
            rows = T // RS_CHUNKS  # 512
            half = rows // 2  # 256

            def proj_mm(tt, wp_sb, bp_sb):
                for n in range(C // 512):
                    ps = avmm_ps.tile([P, 1024], F32, tag="avmm", name="pj_ps")
                    for cc in range(HC // P):
                        nc.tensor.matmul(
                            ps[:, 0:512],
                            yT[:, cc, tt * P : (tt + 1) * P],
                            wp_sb[:, cc, n * 512 : (n + 1) * 512],
                            start=(cc == 0),
                            stop=(cc == HC // P - 1),
                        )
                    z_sb = zoutp.tile([P, 512], BF16, tag="z", name="z_sb")
                    nc.vector.tensor_add(
                        out=z_sb[:],
                        in0=ps[:, 0:512],
                        in1=bp_sb[:, n * 512 : (n + 1) * 512],
                    )
                    nc.sync.dma_start(
                        z_dram[tt * P : (tt + 1) * P, n * 512 : (n + 1) * 512],
                        z_sb[:],
                    )

            def rs_tail(rc):
                nc.gpsimd.collective_compute(
                    "ReduceScatter",
                    mybir.AluOpType.add,
                    replica_groups=[[0, 1], [2, 3], [4, 5], [6, 7]],
                    ins=[z_dram[rc * rows : (rc + 1) * rows, :].opt()],
                    outs=[rs_out[rc * half : (rc + 1) * half, :].opt()],
                )
                nc.sync.dma_start(
                    out_d[rc * half : (rc + 1) * half, :],
                    rs_out[rc * half : (rc + 1) * half, :],
                )

            def proj_rs(rc, wp_sb, bp_sb):
                tt_per_chunk = TT // RS_CHUNKS
                for tt in range(rc * tt_per_chunk, (rc + 1) * tt_per_chunk):
                    proj_mm(tt, wp_sb, bp_sb)
                rs_tail(rc)

            # ---- schedule ----
            qkproj(0)
            score_exp(0, 0)
            qkproj(1)
            vproj(0, 8)
            score_exp(1, 0)
            av_unit(0, 0)
            qkproj(2)
            vproj(8, 16)
            score_exp(2, 0)
            av_unit(1, 0)
            qkproj(3)
            score_exp(3, 0)
            av_unit(2, 0)
            av_unit(3, 0)

            nc.gpsimd.dma_start(wp_sb[:], wp_d[:])
            bp_sb, bp_free = tc.tile([P, C], mybir.dt.float8e4, name="bp_sb")
            nc.gpsimd.dma_start(bp_sb[:], bp_d[:])

            # u1 phase is ACT(exp)-bound: spread proj(0)/proj(1) tt-slices
            # across the unit boundaries as PE filler.
            score_exp(0, 1)
            proj_mm(0, wp_sb, bp_sb)
            proj_mm(1, wp_sb, bp_sb)
            score_exp(1, 1)
            proj_mm(2, wp_sb, bp_sb)
            proj_mm(3, wp_sb, bp_sb)
            rs_tail(0)
            av_unit(0, 1)
            proj_mm(4, wp_sb, bp_sb)
            score_exp(2, 1)
            proj_mm(5, wp_sb, bp_sb)
            proj_mm(6, wp_sb, bp_sb)
            av_unit(1, 1)
            proj_mm(7, wp_sb, bp_sb)
            rs_tail(1)
            score_exp(3, 1)
            # chunk-major tail: finish every head pair's chunk 2 first so
            # proj/RS of rows 1024-1535 overlap the chunk-3 AV work
            av_unit(2, 1, chunks=(2,))
            av_unit(3, 1, chunks=(2,))
            for tt in range(8, 12):
                proj_mm(tt, wp_sb, bp_sb)
            rs_tail(2)
            av_unit(2, 1, chunks=(3,))
            av_unit(3, 1, chunks=(3,))
            for tt in range(12, 16):
                proj_mm(tt, wp_sb, bp_sb)
            rs_tail(3)
            bp_free()
            wp_free()

            if _DEBUG_DUMP:
                qT_o = nc.dram_tensor("qT_o", [P, NP, T], BF16, kind="ExternalOutput")
                kT_o = nc.dram_tensor("kT_o", [P, NP, T], BF16, kind="ExternalOutput")
                v_o = nc.dram_tensor(
                    "v_o", [P, TT, HL, D + 1], BF16, kind="ExternalOutput"
                )
                yT_o = nc.dram_tensor("yT_o", [P, NP, T], BF16, kind="ExternalOutput")
                ar_o = nc.dram_tensor(
                    "ar_o", [P, 64 * 1024], BF16, kind="ExternalOutput"
                )
                nc.sync.dma_start(qT_o[:], qT[:])
                nc.sync.dma_start(kT_o[:], kT[:])
                nc.sync.dma_start(v_o[:], v_ext[:])
                nc.sync.dma_start(yT_o[:], yT[:])
                nc.sync.dma_start(ar_o[:], arena[:])

    nc.compile()
    return nc


def _in_maps(inputs):
    x = np.asarray(inputs["x"], dtype=np.float32)
    w_attn = np.asarray(inputs["w_attn"], dtype=np.float32)
    b_attn = np.asarray(inputs["b_attn"], dtype=np.float32)
    w_proj = np.asarray(inputs["w_proj"], dtype=np.float32)
    b_proj = np.asarray(inputs["b_proj"], dtype=np.float32)

    maps = []
    for core in range(N_CORES):
        b, g = core // 2, core % 2
        s = g * HC
        # x[b] [T, C] -> xT [q4, ki, ck, t%512] with c = ck*128 + ki,
        # each t-quarter contiguous for clean DMA
        xT = (
            x[b]
            .reshape(4, 512, CK, P)
            .transpose(0, 3, 2, 1)
            .astype(ml_dtypes.bfloat16)
        )
        # [C, HC] -> [ki, j, ko, n] with c = ko*128+ki, qcol = j*128+n
        wq = (
            w_attn[:, s : s + HC]
            .reshape(CK, P, NP, P)
            .transpose(1, 2, 0, 3)
            .astype(ml_dtypes.bfloat16)
        )
        wk = (
            w_attn[:, C + s : C + s + HC]
            .reshape(CK, P, NP, P)
            .transpose(1, 2, 0, 3)
            .astype(ml_dtypes.bfloat16)
        )
        # [C, HC] -> [ki, ko, vcol]
        wv = (
            w_attn[:, 2 * C + s : 2 * C + s + HC]
            .reshape(CK, P, HC)
            .transpose(1, 0, 2)
            .astype(ml_dtypes.bfloat16)
        )
        # [HC, C] -> [ki, ko, co], bf16
        wp = (
            w_proj[s : s + HC, :]
            .reshape(HC // P, P, C)
            .transpose(1, 0, 2)
            .astype(ml_dtypes.bfloat16)
        )
        bq = b_attn[s : s + HC].reshape(NP, P).T
        bk = b_attn[C + s : C + s + HC].reshape(NP, P).T
        bv = np.broadcast_to(
            b_attn[2 * C + s : 2 * C + s + HC], (P, HC)
        ).astype(ml_dtypes.bfloat16)
        bp = (
            np.broadcast_to(b_proj, (P, C)).astype(ml_dtypes.float8_e4m3)
            if g == 0
            else np.zeros((P, C), ml_dtypes.float8_e4m3)
        )
        maps.append(
            {
                "xT": np.ascontiguousarray(xT),
                "wq": np.ascontiguousarray(wq),
                "wk": np.ascontiguousarray(wk),
                "wv": np.ascontiguousarray(wv),
                "wp": np.ascontiguousarray(wp),
                "bq": np.ascontiguousarray(bq),
                "bk": np.ascontiguousarray(bk),
                "bv": np.ascontiguousarray(bv),
                "bp": np.ascontiguousarray(bp),
            }
        )
    return maps


def _run(inputs, trace=False, trace_cores=None):
    if "nc" not in _CACHE:
        _CACHE["nc"] = _build_nc()
    nc = _CACHE["nc"]
    res = run_bass_kernel_spmd(
        nc,
        _in_maps(inputs),
        list(range(N_CORES)),
        trace=trace,
        trace_cores=trace_cores,
    )
    # chunked RS ownership: even core holds rows [512c, 512c+256),
    # odd core holds rows [512c+256, 512c+512), for c = 0..3
    out = np.empty((B, T, C), np.float32)
    rows = T // RS_CHUNKS
    half = rows // 2
    for b in range(B):
        ev = res.results[2 * b]["out"].astype(np.float32)
        od = res.results[2 * b + 1]["out"].astype(np.float32)
        for rc in range(RS_CHUNKS):
            out[b, rc * rows : rc * rows + half] = ev[rc * half : (rc + 1) * half]
            out[b, rc * rows + half : (rc + 1) * rows] = od[
                rc * half : (rc + 1) * half
            ]
    return out, res


def kernel(**inputs):
    out, _ = _run(inputs)
    return out


# revision 40
# speedup vs baseline: 1.3231x; 1.1187x over previous
"""Causal self-attention (B=4, T=2048, C=1024, H=16) on 8 Trainium2 cores.

Sharding: core c -> batch b = c//2, head-group g = c%2 (8 heads each,
tensor-parallel). QKV + attention + c_proj computed per core on its head
slice; partial c_proj outputs of a (b) pair are summed with chunked
on-device ReduceScatters over the T dimension; host reassembles.

v2: x is pre-transposed/cast to bf16 on the host (layout only, like the
weight reshapes), AV uses v-stationary matmuls streaming 512 queries so
the PE array stays busy (HAM warm), softmax denominators ride as a ones
column of v, and the causal mask is applied in-place on PSUM with one
affine_select per diagonal block.

Self-contained: only imports concourse (installed library) + numpy.
"""

import ml_dtypes
import numpy as np

import concourse.mybir as mybir
import concourse.tile as tile
from concourse import bacc
from concourse.bass_utils import run_bass_kernel_spmd
from concourse.masks import make_identity

B, T, C = 4, 2048, 1024
H_TOTAL, D = 16, 64
N_CORES = 8
HL = H_TOTAL // 2  # local heads per core (8)
HC = HL * D  # local head cols (512)
NP = HL // 2  # head pairs (4)
P = 128
TT = T // P  # 16 t-chunks of 128
CK = C // P  # 8 contraction chunks for qkv
RS_CHUNKS = 4
F32 = mybir.dt.float32
BF16 = mybir.dt.bfloat16
MASK_VAL = -480.0  # -60 after the 1/8 attention scale; exp(-60) ~ 0
SCALE = 1.0 / 8.0  # 1/sqrt(D)

_CACHE = {}
_DEBUG_DUMP = False


def _build_nc():
    nc = bacc.Bacc("TRN2", target_bir_lowering=False, debug=False, num_devices=N_CORES)

    # x pre-transposed and cast on host: [ki, ck, t]
    xT_d = nc.dram_tensor("xT", [4, P, CK, 512], BF16, kind="ExternalInput")
    wq_d = nc.dram_tensor("wq", [P, NP, CK, P], BF16, kind="ExternalInput")
    wk_d = nc.dram_tensor("wk", [P, NP, CK, P], BF16, kind="ExternalInput")
    wv_d = nc.dram_tensor("wv", [P, CK, HC], BF16, kind="ExternalInput")
    bq_d = nc.dram_tensor("bq", [P, NP], F32, kind="ExternalInput")
    bk_d = nc.dram_tensor("bk", [P, NP], F32, kind="ExternalInput")
    bv_d = nc.dram_tensor("bv", [P, HC], BF16, kind="ExternalInput")
    wp_d = nc.dram_tensor("wp", [P, HC // P, C], BF16, kind="ExternalInput")
    bp_d = nc.dram_tensor("bp", [P, C], mybir.dt.float8e4, kind="ExternalInput")
    out_d = nc.dram_tensor("out", [T // 2, C], BF16, kind="ExternalOutput")

    with tile.TileContext(nc) as tc:
        with (
            tc.tile_pool(name="const", bufs=1) as constp,
            tc.tile_pool(name="big", bufs=1) as bigp,
            tc.tile_pool(name="rnorm", bufs=1) as rnp,
            tc.tile_pool(name="zout", bufs=1) as zoutp,
            tc.tile_pool(name="score_ps", bufs=1, space="PSUM") as score_ps,
            tc.tile_pool(name="avmm_ps", bufs=2, space="PSUM") as avmm_ps,
            tc.tile_pool(name="dram", bufs=1, space="DRAM") as dramp,
        ):
            # ---- constants ----
            # dmaskT @ ident seeds the diagonal PSUM block with the causal
            # mask on the PE itself (keeps DVE out of the score->exp chain):
            # dmaskT[p, c] = MASK_VAL where p < c, so (dmaskT^T I)[s, c] =
            # dmaskT[c, s] = MASK_VAL where c < s.
            dmaskT = constp.tile([P, P], BF16)
            nc.vector.memset(dmaskT[:], 0.0)
            nc.gpsimd.affine_select(
                out=dmaskT[:],
                in_=dmaskT[:],
                compare_op=mybir.AluOpType.is_ge,
                fill=MASK_VAL,
                base=0,
                pattern=[[-1, P]],
                channel_multiplier=1,
            )
            ident_bf = constp.tile([P, P], BF16)
            make_identity(nc, ident_bf)
            bq_sb = constp.tile([P, NP], F32)
            nc.sync.dma_start(bq_sb[:], bq_d[:])
            bk_sb = constp.tile([P, NP], F32)
            nc.sync.dma_start(bk_sb[:], bk_d[:])
            # reserve wp space early (needs 8KB contiguous; DMA'd later)
            wp_sb, wp_free = tc.tile([P, HC // P, C], BF16, name="wp_sb")

            # ---- persistent activations ----
            qT = bigp.tile([P, NP, T], BF16)  # q^T [qcol, t]
            kT = bigp.tile([P, NP, T], BF16)  # k^T [kcol, t]
            v_ext = bigp.tile([P, TT, HL, D + 1], BF16)  # v with ones col
            nc.vector.memset(v_ext[:, :, :, D : D + 1], 1.0)
            yT = bigp.tile([P, NP, T], BF16)  # y^T [ci, t]
            # p/xT arena: 64K bf16 elems per partition (128 KB).
            #   u0 p-pair slots (8x2048 = 16K elems): j%3 -> [0,16K),[16K,32K),[32K,48K)
            #   xT (8x2048): [48K, 64K)
            #   u1 p-pair slots (16x2048 = 32K elems): j%2 -> [0,32K),[32K,64K)
            arena = bigp.tile([P, 64 * 1024], BF16)

            def p_view(j, u):
                if u == 0:
                    off = 16384 * (j % 3)
                    return arena[:, off : off + 16384].rearrange(
                        "p (i q) -> p i q", q=2048
                    )
                off = 32768 * (j % 2)
                return arena[:, off : off + 32768].rearrange(
                    "p (i q) -> p i q", q=2048
                )

            xT = arena[:, 49152:65536].rearrange("p (c t) -> p c t", t=T)

            # xT DMA in t-quarters (each contiguous in DRAM) so qkproj(0)
            # starts after ~3 us
            for q4 in range(4):
                nc.sync.dma_start(
                    xT[:, :, q4 * 512 : (q4 + 1) * 512],
                    xT_d[q4],
                )

            # ---- QKV projections ----
            # transient wq/wk tiles live in wp_sb's space (wp is DMA'd only
            # after the last qkproj read; 4 rotating 2KB slots)
            wp_flat = wp_sb[:].rearrange("p a b -> p (a b)")

            def qkproj(j):
                for si, (w_d, b_sb, dstT) in enumerate(
                    ((wq_d, bq_sb, qT), (wk_d, bk_sb, kT))
                ):
                    slot = (2 * j + si) % 4
                    wj = wp_flat[:, slot * 1024 : (slot + 1) * 1024].rearrange(
                        "p (c k) -> p c k", k=P
                    )
                    nc.sync.dma_start(wj[:], w_d[:, j])
                    for u4 in range(T // 512):
                        ps = avmm_ps.tile([P, 1024], F32, tag="avmm", name="qk_ps")
                        for ck in range(CK):
                            nc.tensor.matmul(
                                ps[:, 0:512],
                                wj[:, ck, :],
                                xT[:, ck, u4 * 512 : (u4 + 1) * 512],
                                start=(ck == 0),
                                stop=(ck == CK - 1),
                            )
                        nc.vector.tensor_add(
                            out=dstT[:, j, u4 * 512 : (u4 + 1) * 512],
                            in0=ps[:, 0:512],
                            in1=b_sb[:, j : j + 1].to_broadcast((P, 512)),
                        )

            # wv and bv live in u0 p-slot 2 of the arena: all reads (vproj)
            # are scheduled before score_exp(2, 0) overwrites that slot.
            wv_sb = arena[:, 32768:36864].rearrange("p (c v) -> p c v", v=HC)
            nc.gpsimd.dma_start(wv_sb[:], wv_d[:])
            bv_sb = arena[:, 36864:37376]
            nc.sync.dma_start(bv_sb[:], bv_d[:])

            def vproj(tt_lo, tt_hi):
                for tt in range(tt_lo, tt_hi):
                    ps = avmm_ps.tile([P, 1024], F32, tag="avmm", name="v_ps")
                    for ck in range(CK):
                        nc.tensor.matmul(
                            ps[:, 0:512],
                            xT[:, ck, tt * P : (tt + 1) * P],
                            wv_sb[:, ck, :],
                            start=(ck == 0),
                            stop=(ck == CK - 1),
                        )
                    nc.vector.tensor_add(
                        out=v_ext[:, tt, :, 0:D],
                        in0=ps[:, 0:512].rearrange("p (h d) -> p h d", d=D),
                        in1=bv_sb[:].rearrange("p (h d) -> p h d", d=D),
                    )

            # ---- scores + exp ----
            def score_exp(j, u):
                n_i = 8 * (u + 1)
                pt = p_view(j, u)
                # one PSUM tile per head: double-buffered across units so the
                # next unit's matmuls run while this unit's exp drains. The
                # two heads' matmuls still pair up via row groups 0/64.
                ps2 = [
                    score_ps.tile([P, 1024], F32, tag="score", name=f"sc{hh}")
                    for hh in range(2)
                ]
                for i in range(n_i):
                    d0 = i * P - 1024 * u  # diag block col (within unit)
                    c0 = max(0, d0)
                    jj_diag = 2 * u + d0 // 512 if i >= 8 * u else -1
                    for hh in range(2):
                        hb = hh * D
                        for jj in range(2 * u, 2 * u + 2):
                            if jj < i // 4:
                                continue
                            c0j = (jj - 2 * u) * 512
                            if jj != jj_diag:
                                nc.tensor.matmul(
                                    ps2[hh][:, c0j : c0j + 512],
                                    kT[hb : hb + D, j, i * P : (i + 1) * P],
                                    qT[hb : hb + D, j, jj * 512 : (jj + 1) * 512],
                                    start=True,
                                    stop=True,
                                )
                                continue
                            # diag block: seed [d0, d0+128) with the causal
                            # mask, accumulate scores on top; cols left of the
                            # diag are above-diagonal (never exp'd/streamed),
                            # cols right of it get their own fresh matmul.
                            nc.tensor.matmul(
                                ps2[hh][:, d0 : d0 + P],
                                dmaskT[:],
                                ident_bf[:],
                                start=True,
                                stop=False,
                            )
                            nc.tensor.matmul(
                                ps2[hh][:, d0 : d0 + P],
                                kT[hb : hb + D, j, i * P : (i + 1) * P],
                                qT[hb : hb + D, j, d0 + 1024 * u : d0 + 1024 * u + P],
                                start=False,
                                stop=True,
                            )
                            hi = c0j + 512
                            if d0 + P < hi:
                                nc.tensor.matmul(
                                    ps2[hh][:, d0 + P : hi],
                                    kT[hb : hb + D, j, i * P : (i + 1) * P],
                                    qT[
                                        hb : hb + D,
                                        j,
                                        d0 + P + 1024 * u : hi + 1024 * u,
                                    ],
                                    start=True,
                                    stop=True,
                                )
                    for hh in range(2):
                        nc.scalar.activation(
                            out=pt[:, i, hh * 1024 + c0 : (hh + 1) * 1024],
                            in_=ps2[hh][:, c0:1024],
                            func=mybir.ActivationFunctionType.Exp,
                            scale=SCALE,
                        )

            # ---- attention-value product, v-stationary ----
            # out[0:64] = y^T (unnormalized), out[64] = softmax denominator
            # via the ones column of v_ext; p streams 512 queries per matmul.
            def av_unit(j, u, chunks=None):
                pt = p_view(j, u)
                for c in chunks if chunks is not None else (2 * u, 2 * u + 1):
                    i_max = 4 * (c + 1)
                    qo = (c - 2 * u) * 512
                    av = avmm_ps.tile([P, 1024], F32, tag="avmm", name=f"av{j}{c}")
                    for hh in range(2):
                        h = 2 * j + hh
                        for i in range(i_max):
                            # stream only cols at/after the causal boundary:
                            # p[:, i, :lo] above the diagonal is never written
                            lo = max(qo, i * P - 1024 * u)
                            nc.tensor.matmul(
                                av[0 : D + 1, hh * 512 + lo - qo : hh * 512 + 512],
                                v_ext[:, i, h, :],
                                pt[:, i, hh * 1024 + lo : hh * 1024 + qo + 512],
                                start=(i == 0),
                                stop=(i == i_max - 1),
                            )
                    # normalize: y * (1/denom); recips for both heads land on
                    # lanes 0-63 (rb cols 0-511 even head, 512-1023 odd head)
                    # Copy y+denominators out of PSUM right away (frees the
                    # bank for the next chain; PE never waits on the norm).
                    stage = rnp.tile(
                        [D + 1, 1024], BF16, tag="avst", name="avstage", bufs=2
                    )
                    nc.vector.tensor_copy(out=stage[:], in_=av[0 : D + 1, :])
                    # Lane-parallel reciprocal: DVE recip costs ~9 cyc per
                    # FREE element regardless of lanes, so recip the denoms
                    # as a [128, 8] column (0.15us) instead of a [1, 1024]
                    # row (6.5us). Rows<->columns via transposing DRAM APs;
                    # latency is off the critical path now.
                    d1 = dramp.tile([1, 1024], BF16, tag="d1", name="d1", bufs=2)
                    nc.gpsimd.dma_start(d1[:], stage[D : D + 1, :])
                    dcol = rnp.tile([P, 8], BF16, tag="dcol", name="dcol")
                    nc.gpsimd.dma_start(
                        dcol[:], d1[0:1, :].rearrange("o (c p) -> (o p) c", p=P)
                    )
                    with nc.allow_low_precision(
                        reason="bf16 1/denom: ~0.4% rel on y, within tolerance"
                    ):
                        nc.vector.reciprocal(dcol[:], dcol[:])
                    d2 = dramp.tile([1, 1024], BF16, tag="d2", name="d2", bufs=2)
                    nc.gpsimd.dma_start(
                        d2[0:1, :].rearrange("o (c p) -> (o p) c", p=P), dcol[:]
                    )
                    rb = rnp.tile([P, 512], BF16, tag="rb", name="rb")
                    nc.gpsimd.dma_start(
                        rb[0:D, :], d2[0:1, 0:512].to_broadcast((D, 512))
                    )
                    nc.gpsimd.dma_start(
                        rb[D:P, :], d2[0:1, 512:1024].to_broadcast((D, 512))
                    )
                    nc.vector.tensor_mul(
                        out=yT[0:D, j, c * 512 : (c + 1) * 512],
                        in0=stage[0:D, 0:512],
                        in1=rb[0:D, :],
                    )
                    # partition shift the odd head's unnormalized y to
                    # rows 64-127, then normalize in place there
                    nc.gpsimd.dma_start(
                        yT[D:P, j, c * 512 : (c + 1) * 512],
                        stage[0:D, 512:1024],
                    )
                    nc.vector.tensor_mul(
                        out=yT[D:P, j, c * 512 : (c + 1) * 512],
                        in0=yT[D:P, j, c * 512 : (c + 1) * 512],
                        in1=rb[D:P, :],
                    )
                    if _DEBUG_DUMP and (j, c) == (0, 0):
                        rn_o = nc.dram_tensor(
                            "rn_o", [P, 1024], BF16, kind="ExternalOutput"
                        )
                        nc.sync.dma_start(rn_o[:], rn[:])

            # ---- c_proj + ReduceScatter ----
            z_dram = dramp.tile([T, C], BF16)
            rs_out = dramp.tile([T // 2, C], BF16)
            rows = T // RS_CHUNKS  # 512
            half = rows // 2  # 256

            def proj_mm(tt, wp_sb, bp_sb):
                for n in range(C // 512):
                    ps = avmm_ps.tile([P, 1024], F32, tag="avmm", name="pj_ps")
                    for cc in range(HC // P):
                        nc.tensor.matmul(
                            ps[:, 0:512],
                            yT[:, cc, tt * P : (tt + 1) * P],
                            wp_sb[:, cc, n * 512 : (n + 1) * 512],
                            start=(cc == 0),
                            stop=(cc == HC // P - 1),
                        )
                    z_sb = zoutp.tile([P, 512], BF16, tag="z", name="z_sb")
                    nc.vector.tensor_add(
                        out=z_sb[:],
                        in0=ps[:, 0:512],
                        in1=bp_sb[:, n * 512 : (n + 1) * 512],
                    )
                    nc.sync.dma_start(
                        z_dram[tt * P : (tt + 1) * P, n * 512 : (n + 1) * 512],
                        z_sb[:],
                    )

            def rs_tail(rc):
                nc.gpsimd.collective_compute(
                    "ReduceScatter",
                    mybir.AluOpType.add,
                    replica_groups=[[0, 1], [2, 3], [4, 5], [6, 7]],
                    ins=[z_dram[rc * rows : (rc + 1) * rows, :].opt()],
                    outs=[rs_out[rc * half : (rc + 1) * half, :].opt()],
                )
                nc.sync.dma_start(
                    out_d[rc * half : (rc + 1) * half, :],
                    rs_out[rc * half : (rc + 1) * half, :],
                )

            def proj_rs(rc, wp_sb, bp_sb):
                tt_per_chunk = TT // RS_CHUNKS
                for tt in range(rc * tt_per_chunk, (rc + 1) * tt_per_chunk):
                    proj_mm(tt, wp_sb, bp_sb)
                rs_tail(rc)

            # ---- schedule ----
            qkproj(0)
            score_exp(0, 0)
            qkproj(1)
            vproj(0, 8)
            score_exp(1, 0)
            av_unit(0, 0)
            qkproj(2)
            vproj(8, 16)
            score_exp(2, 0)
            av_unit(1, 0)
            qkproj(3)
            score_exp(3, 0)
            av_unit(2, 0)
            av_unit(3, 0)

            nc.gpsimd.dma_start(wp_sb[:], wp_d[:])
            bp_sb, bp_free = tc.tile([P, C], mybir.dt.float8e4, name="bp_sb")
            nc.gpsimd.dma_start(bp_sb[:], bp_d[:])

            # u1 phase is ACT(exp)-bound: spread proj(0)/proj(1) tt-slices
            # across the unit boundaries as PE filler.
            score_exp(0, 1)
            proj_mm(0, wp_sb, bp_sb)
            proj_mm(1, wp_sb, bp_sb)
            score_exp(1, 1)
            proj_mm(2, wp_sb, bp_sb)
            proj_mm(3, wp_sb, bp_sb)
            rs_tail(0)
            av_unit(0, 1)
            proj_mm(4, wp_sb, bp_sb)
            score_exp(2, 1)
            proj_mm(5, wp_sb, bp_sb)
            proj_mm(6, wp_sb, bp_sb)
            av_unit(1, 1)
            proj_mm(7, wp_sb, bp_sb)
            rs_tail(1)
            score_exp(3, 1)
            # chunk-major tail: finish every head pair's chunk 2 first so
            # proj/RS of rows 1024-1535 overlap the chunk-3 AV work
            av_unit(2, 1, chunks=(2,))
            av_unit(3, 1, chunks=(2,))
            for tt in range(8, 12):
                proj_mm(tt, wp_sb, bp_sb)
            rs_tail(2)
            av_unit(2, 1, chunks=(3,))
            av_unit(3, 1, chunks=(3,))
            for tt in range(12, 16):
                proj_mm(tt, wp_sb, bp_sb)
            rs_tail(3)
            bp_free()
            wp_free()

            if _DEBUG_DUMP:
                qT_o = nc.dram_tensor("qT_o", [P, NP, T], BF16, kind="ExternalOutput")
                kT_o = nc.dram_tensor("kT_o", [P, NP, T], BF16, kind="ExternalOutput")
                v_o = nc.dram_tensor(
                    "v_o", [P, TT, HL, D + 1], BF16, kind="ExternalOutput"
                )
                yT_o = nc.dram_tensor("yT_o", [P, NP, T], BF16, kind="ExternalOutput")
                ar_o = nc.dram_tensor(
                    "ar_o", [P, 64 * 1024], BF16, kind="ExternalOutput"
                )
                nc.sync.dma_start(qT_o[:], qT[:])
                nc.sync.dma_start(kT_o[:], kT[:])
                nc.sync.dma_start(v_o[:], v_ext[:])
                nc.sync.dma_start(yT_o[:], yT[:])
                nc.sync.dma_start(ar_o[:], arena[:])

    nc.compile()
    return nc


def _in_maps(inputs):
    x = np.asarray(inputs["x"], dtype=np.float32)
    w_attn = np.asarray(inputs["w_attn"], dtype=np.float32)
    b_attn = np.asarray(inputs["b_attn"], dtype=np.float32)
    w_proj = np.asarray(inputs["w_proj"], dtype=np.float32)
    b_proj = np.asarray(inputs["b_proj"], dtype=np.float32)

    maps = []
    for core in range(N_CORES):
        b, g = core // 2, core % 2
        s = g * HC
        # x[b] [T, C] -> xT [q4, ki, ck, t%512] with c = ck*128 + ki,
        # each t-quarter contiguous for clean DMA
        xT = (
            x[b]
            .reshape(4, 512, CK, P)
            .transpose(0, 3, 2, 1)
            .astype(ml_dtypes.bfloat16)
        )
        # [C, HC] -> [ki, j, ko, n] with c = ko*128+ki, qcol = j*128+n
        wq = (
            w_attn[:, s : s + HC]
            .reshape(CK, P, NP, P)
            .transpose(1, 2, 0, 3)
            .astype(ml_dtypes.bfloat16)
        )
        wk = (
            w_attn[:, C + s : C + s + HC]
            .reshape(CK, P, NP, P)
            .transpose(1, 2, 0, 3)
            .astype(ml_dtypes.bfloat16)
        )
        # [C, HC] -> [ki, ko, vcol]
        wv = (
            w_attn[:, 2 * C + s : 2 * C + s + HC]
            .reshape(CK, P, HC)
            .transpose(1, 0, 2)
            .astype(ml_dtypes.bfloat16)
        )
        # [HC, C] -> [ki, ko, co], bf16
        wp = (
            w_proj[s : s + HC, :]
            .reshape(HC // P, P, C)
            .transpose(1, 0, 2)
            .astype(ml_dtypes.bfloat16)
        )
        bq = b_attn[s : s + HC].reshape(NP, P).T
        bk = b_attn[C + s : C + s + HC].reshape(NP, P).T
        bv = np.broadcast_to(
            b_attn[2 * C + s : 2 * C + s + HC], (P, HC)
        ).astype(ml_dtypes.bfloat16)
        bp = (
            np.broadcast_to(b_proj, (P, C)).astype(ml_dtypes.float8_e4m3)
            if g == 0
            else np.zeros((P, C), ml_dtypes.float8_e4m3)
        )
        maps.append(
            {
                "xT": np.ascontiguousarray(xT),
                "wq": np.ascontiguousarray(wq),
                "wk": np.ascontiguousarray(wk),
                "wv": np.ascontiguousarray(wv),
                "wp": np.ascontiguousarray(wp),
                "bq": np.ascontiguousarray(bq),
                "bk": np.ascontiguousarray(bk),
                "bv": np.ascontiguousarray(bv),
                "bp": np.ascontiguousarray(bp),
            }
        )
    return maps


def _run(inputs, trace=False, trace_cores=None):
    if "nc" not in _CACHE:
        _CACHE["nc"] = _build_nc()
    nc = _CACHE["nc"]
    res = run_bass_kernel_spmd(
        nc,
        _in_maps(inputs),
        list(range(N_CORES)),
        trace=trace,
        trace_cores=trace_cores,
    )
    # chunked RS ownership: even core holds rows [512c, 512c+256),
    # odd core holds rows [512c+256, 512c+512), for c = 0..3
    out = np.empty((B, T, C), np.float32)
    rows = T // RS_CHUNKS
    half = rows // 2
    for b in range(B):
        ev = res.results[2 * b]["out"].astype(np.float32)
        od = res.results[2 * b + 1]["out"].astype(np.float32)
        for rc in range(RS_CHUNKS):
            out[b, rc * rows : rc * rows + half] = ev[rc * half : (rc + 1) * half]
            out[b, rc * rows + half : (rc + 1) * rows] = od[
                rc * half : (rc + 1) * half
            ]
    return out, res


def kernel(**inputs):
    out, _ = _run(inputs)
    return out


# revision 42
# speedup vs baseline: 1.4609x; 1.1042x over previous
"""Causal self-attention (B=4, T=2048, C=1024, H=16) on 8 Trainium2 cores.

Sharding: core c -> batch b = c//2, head-group g = c%2 (8 heads each,
tensor-parallel). QKV + attention + c_proj computed per core on its head
slice; partial c_proj outputs of a (b) pair are summed with chunked
on-device ReduceScatters over the T dimension; host reassembles.

v2: x is pre-transposed/cast to bf16 on the host (layout only, like the
weight reshapes), AV uses v-stationary matmuls streaming 512 queries so
the PE array stays busy (HAM warm), softmax denominators ride as a ones
column of v, and the causal mask is applied in-place on PSUM with one
affine_select per diagonal block.

Self-contained: only imports concourse (installed library) + numpy.
"""

import ml_dtypes
import numpy as np

import concourse.mybir as mybir
import concourse.tile as tile
from concourse import bacc
from concourse.bass_utils import run_bass_kernel_spmd
from concourse.masks import make_identity

B, T, C = 4, 2048, 1024
H_TOTAL, D = 16, 64
N_CORES = 8
HL = H_TOTAL // 2  # local heads per core (8)
HC = HL * D  # local head cols (512)
NP = HL // 2  # head pairs (4)
P = 128
TT = T // P  # 16 t-chunks of 128
CK = C // P  # 8 contraction chunks for qkv
RS_CHUNKS = 4
F32 = mybir.dt.float32
BF16 = mybir.dt.bfloat16
MASK_VAL = -480.0  # -60 after the 1/8 attention scale; exp(-60) ~ 0
SCALE = 1.0 / 8.0  # 1/sqrt(D)

_CACHE = {}
_DEBUG_DUMP = False


def _build_nc():
    nc = bacc.Bacc("TRN2", target_bir_lowering=False, debug=False, num_devices=N_CORES)

    # x pre-transposed and cast on host: [ki, ck, t]
    xT_d = nc.dram_tensor("xT", [4, P, CK, 512], BF16, kind="ExternalInput")
    wq_d = nc.dram_tensor("wq", [P, NP, CK, P], BF16, kind="ExternalInput")
    wk_d = nc.dram_tensor("wk", [P, NP, CK, P], BF16, kind="ExternalInput")
    wv_d = nc.dram_tensor("wv", [P, CK, HC], BF16, kind="ExternalInput")
    bq_d = nc.dram_tensor("bq", [P, NP], F32, kind="ExternalInput")
    bk_d = nc.dram_tensor("bk", [P, NP], F32, kind="ExternalInput")
    bv_d = nc.dram_tensor("bv", [P, HC], BF16, kind="ExternalInput")
    wp_d = nc.dram_tensor("wp", [P, HC // P, C], BF16, kind="ExternalInput")
    bp_d = nc.dram_tensor("bp", [P, C], mybir.dt.float8e4, kind="ExternalInput")
    out_d = nc.dram_tensor("out", [T // 2, C], BF16, kind="ExternalOutput")

    with tile.TileContext(nc) as tc:
        with (
            tc.tile_pool(name="const", bufs=1) as constp,
            tc.tile_pool(name="big", bufs=1) as bigp,
            tc.tile_pool(name="rnorm", bufs=1) as rnp,
            tc.tile_pool(name="zout", bufs=1) as zoutp,
            tc.tile_pool(name="score_ps", bufs=1, space="PSUM") as score_ps,
            tc.tile_pool(name="avmm_ps", bufs=2, space="PSUM") as avmm_ps,
            tc.tile_pool(name="dram", bufs=1, space="DRAM") as dramp,
        ):
            # ---- constants ----
            # dmaskT @ ident seeds the diagonal PSUM block with the causal
            # mask on the PE itself (keeps DVE out of the score->exp chain):
            # dmaskT[p, c] = MASK_VAL where p < c, so (dmaskT^T I)[s, c] =
            # dmaskT[c, s] = MASK_VAL where c < s.
            dmaskT = constp.tile([P, P], BF16)
            nc.vector.memset(dmaskT[:], 0.0)
            nc.gpsimd.affine_select(
                out=dmaskT[:],
                in_=dmaskT[:],
                compare_op=mybir.AluOpType.is_ge,
                fill=MASK_VAL,
                base=0,
                pattern=[[-1, P]],
                channel_multiplier=1,
            )
            ident_bf = constp.tile([P, P], BF16)
            make_identity(nc, ident_bf)
            bq_sb = constp.tile([P, NP], F32)
            nc.sync.dma_start(bq_sb[:], bq_d[:])
            bk_sb = constp.tile([P, NP], F32)
            nc.sync.dma_start(bk_sb[:], bk_d[:])
            # reserve wp space early (needs 8KB contiguous; DMA'd later)
            wp_sb, wp_free = tc.tile([P, HC // P, C], BF16, name="wp_sb")

            # ---- persistent activations ----
            qT = bigp.tile([P, NP, T], BF16)  # q^T [qcol, t]
            kT = bigp.tile([P, NP, T], BF16)  # k^T [kcol, t]
            v_ext = bigp.tile([P, TT, HL, D + 1], BF16)  # v with ones col
            nc.vector.memset(v_ext[:, :, :, D : D + 1], 1.0)
            yT = bigp.tile([P, NP, T], BF16)  # y^T [ci, t]
            # p/xT arena: 64K bf16 elems per partition (128 KB).
            #   u0 p-pair slots (8x2048 = 16K elems): j%3 -> [0,16K),[16K,32K),[32K,48K)
            #   xT (8x2048): [48K, 64K)
            #   u1 p-pair slots (16x2048 = 32K elems): j%2 -> [0,32K),[32K,64K)
            arena = bigp.tile([P, 64 * 1024], BF16)

            def p_view(j, u):
                if u == 0:
                    off = 16384 * (j % 3)
                    return arena[:, off : off + 16384].rearrange(
                        "p (i q) -> p i q", q=2048
                    )
                off = 32768 * (j % 2)
                return arena[:, off : off + 32768].rearrange(
                    "p (i q) -> p i q", q=2048
                )

            xT = arena[:, 49152:65536].rearrange("p (c t) -> p c t", t=T)
            d_all = dramp.tile([16, 1024], BF16, name="d_all")
            r_all = dramp.tile([16, 1024], BF16, name="r_all")

            # xT DMA in t-quarters (each contiguous in DRAM) so qkproj(0)
            # starts after ~3 us
            for q4 in range(4):
                nc.sync.dma_start(
                    xT[:, :, q4 * 512 : (q4 + 1) * 512],
                    xT_d[q4],
                )

            # ---- QKV projections ----
            # transient wq/wk tiles live in wp_sb's space (wp is DMA'd only
            # after the last qkproj read; 4 rotating 2KB slots)
            wp_flat = wp_sb[:].rearrange("p a b -> p (a b)")

            def qkproj(j):
                for si, (w_d, b_sb, dstT) in enumerate(
                    ((wq_d, bq_sb, qT), (wk_d, bk_sb, kT))
                ):
                    slot = (2 * j + si) % 4
                    wj = wp_flat[:, slot * 1024 : (slot + 1) * 1024].rearrange(
                        "p (c k) -> p c k", k=P
                    )
                    nc.sync.dma_start(wj[:], w_d[:, j])
                    for u4 in range(T // 512):
                        ps = avmm_ps.tile([P, 1024], F32, tag="avmm", name="qk_ps")
                        for ck in range(CK):
                            nc.tensor.matmul(
                                ps[:, 0:512],
                                wj[:, ck, :],
                                xT[:, ck, u4 * 512 : (u4 + 1) * 512],
                                start=(ck == 0),
                                stop=(ck == CK - 1),
                            )
                        nc.vector.tensor_add(
                            out=dstT[:, j, u4 * 512 : (u4 + 1) * 512],
                            in0=ps[:, 0:512],
                            in1=b_sb[:, j : j + 1].to_broadcast((P, 512)),
                        )

            # wv and bv live in u0 p-slot 2 of the arena: all reads (vproj)
            # are scheduled before score_exp(2, 0) overwrites that slot.
            wv_sb = arena[:, 32768:36864].rearrange("p (c v) -> p c v", v=HC)
            nc.gpsimd.dma_start(wv_sb[:], wv_d[:])
            bv_sb = arena[:, 36864:37376]
            nc.sync.dma_start(bv_sb[:], bv_d[:])

            def vproj(tt_lo, tt_hi):
                for tt in range(tt_lo, tt_hi):
                    ps = avmm_ps.tile([P, 1024], F32, tag="avmm", name="v_ps")
                    for ck in range(CK):
                        nc.tensor.matmul(
                            ps[:, 0:512],
                            xT[:, ck, tt * P : (tt + 1) * P],
                            wv_sb[:, ck, :],
                            start=(ck == 0),
                            stop=(ck == CK - 1),
                        )
                    nc.vector.tensor_add(
                        out=v_ext[:, tt, :, 0:D],
                        in0=ps[:, 0:512].rearrange("p (h d) -> p h d", d=D),
                        in1=bv_sb[:].rearrange("p (h d) -> p h d", d=D),
                    )

            # ---- scores + exp ----
            def score_exp(j, u):
                n_i = 8 * (u + 1)
                pt = p_view(j, u)
                # one PSUM tile per head: double-buffered across units so the
                # next unit's matmuls run while this unit's exp drains. The
                # two heads' matmuls still pair up via row groups 0/64.
                ps2 = [
                    score_ps.tile([P, 1024], F32, tag="score", name=f"sc{hh}")
                    for hh in range(2)
                ]
                for i in range(n_i):
                    d0 = i * P - 1024 * u  # diag block col (within unit)
                    c0 = max(0, d0)
                    jj_diag = 2 * u + d0 // 512 if i >= 8 * u else -1
                    for hh in range(2):
                        hb = hh * D
                        for jj in range(2 * u, 2 * u + 2):
                            if jj < i // 4:
                                continue
                            c0j = (jj - 2 * u) * 512
                            if jj != jj_diag:
                                nc.tensor.matmul(
                                    ps2[hh][:, c0j : c0j + 512],
                                    kT[hb : hb + D, j, i * P : (i + 1) * P],
                                    qT[hb : hb + D, j, jj * 512 : (jj + 1) * 512],
                                    start=True,
                                    stop=True,
                                )
                                continue
                            # diag block: seed [d0, d0+128) with the causal
                            # mask, accumulate scores on top; cols left of the
                            # diag are above-diagonal (never exp'd/streamed),
                            # cols right of it get their own fresh matmul.
                            nc.tensor.matmul(
                                ps2[hh][:, d0 : d0 + P],
                                dmaskT[:],
                                ident_bf[:],
                                start=True,
                                stop=False,
                            )
                            nc.tensor.matmul(
                                ps2[hh][:, d0 : d0 + P],
                                kT[hb : hb + D, j, i * P : (i + 1) * P],
                                qT[hb : hb + D, j, d0 + 1024 * u : d0 + 1024 * u + P],
                                start=False,
                                stop=True,
                            )
                            hi = c0j + 512
                            if d0 + P < hi:
                                nc.tensor.matmul(
                                    ps2[hh][:, d0 + P : hi],
                                    kT[hb : hb + D, j, i * P : (i + 1) * P],
                                    qT[
                                        hb : hb + D,
                                        j,
                                        d0 + P + 1024 * u : hi + 1024 * u,
                                    ],
                                    start=True,
                                    stop=True,
                                )
                    for hh in range(2):
                        nc.scalar.activation(
                            out=pt[:, i, hh * 1024 + c0 : (hh + 1) * 1024],
                            in_=ps2[hh][:, c0:1024],
                            func=mybir.ActivationFunctionType.Exp,
                            scale=SCALE,
                        )

            # ---- attention-value product, v-stationary ----
            # out[0:64] = y^T (unnormalized), out[64] = softmax denominator
            # via the ones column of v_ext; p streams 512 queries per matmul.
            def av_unit(j, u, chunks=None):
                pt = p_view(j, u)
                for c in chunks if chunks is not None else (2 * u, 2 * u + 1):
                    i_max = 4 * (c + 1)
                    qo = (c - 2 * u) * 512
                    av = avmm_ps.tile([P, 1024], F32, tag="avmm", name=f"av{j}{c}")
                    for hh in range(2):
                        h = 2 * j + hh
                        for i in range(i_max):
                            # stream only cols at/after the causal boundary:
                            # p[:, i, :lo] above the diagonal is never written
                            lo = max(qo, i * P - 1024 * u)
                            nc.tensor.matmul(
                                av[0 : D + 1, hh * 512 + lo - qo : hh * 512 + 512],
                                v_ext[:, i, h, :],
                                pt[:, i, hh * 1024 + lo : hh * 1024 + qo + 512],
                                start=(i == 0),
                                stop=(i == i_max - 1),
                            )
                    # Evacuate PSUM immediately (PE never waits on norm):
                    # even head's unnormalized y -> yT rows 0-63 directly,
                    # odd head's + denominator row -> avout staging, then a
                    # DMA partition-shift puts odd y at yT rows 64-127 and
                    # the denominators accumulate in DRAM for batch_norm.
                    cid = 4 * c + j
                    avout = rnp.tile(
                        [P, 1024], BF16, tag="avout", name="avout", bufs=2
                    )
                    nc.vector.tensor_copy(
                        out=yT[0:D, j, c * 512 : (c + 1) * 512],
                        in_=av[0:D, 0:512],
                    )
                    nc.vector.tensor_copy(
                        out=avout[0:D, 512:1024], in_=av[0:D, 512:1024]
                    )
                    nc.vector.tensor_copy(
                        out=avout[D : D + 1, :], in_=av[D : D + 1, :]
                    )
                    nc.gpsimd.dma_start(
                        yT[D:P, j, c * 512 : (c + 1) * 512],
                        avout[0:D, 512:1024],
                    )
                    nc.gpsimd.dma_start(d_all[cid], avout[D : D + 1, :])

            # Batched normalization: one transposing DRAM round-trip turns
            # the [n, 1024] denominator rows into a [128, 8n] column, one
            # lane-parallel reciprocal (DVE recip costs ~9 cyc per FREE
            # element), write back transposed, then broadcast-read each
            # chunk's recips and scale yT in place (rows 0-63 even head,
            # 64-127 odd head).
            def batch_norm(c_lo, c_hi):
                lo, hi = 4 * c_lo, 4 * c_hi
                n = hi - lo
                rb0 = rnp.tile([P, 512], BF16, tag="rb", name="rb")
                dcb = rb0[:, 0:64]
                nc.gpsimd.dma_start(
                    dcb[:, 0 : 8 * n],
                    d_all[lo:hi, :].rearrange("n (c p) -> p (n c)", p=P),
                )
                with nc.allow_low_precision(
                    reason="bf16 1/denom: ~0.4% rel on y, within tolerance"
                ):
                    nc.vector.reciprocal(dcb[:, 0 : 8 * n], dcb[:, 0 : 8 * n])
                nc.gpsimd.dma_start(
                    r_all[lo:hi, :].rearrange("n (c p) -> p (n c)", p=P),
                    dcb[:, 0 : 8 * n],
                )
                for c in range(c_lo, c_hi):
                    for j in range(NP):
                        cid = 4 * c + j
                        rb = rnp.tile([P, 512], BF16, tag="rb", name="rb")
                        nc.gpsimd.dma_start(
                            rb[0:D, :],
                            r_all[cid : cid + 1, 0:512].to_broadcast((D, 512)),
                        )
                        nc.gpsimd.dma_start(
                            rb[D:P, :],
                            r_all[cid : cid + 1, 512:1024].to_broadcast(
                                (D, 512)
                            ),
                        )
                        nc.vector.tensor_mul(
                            out=yT[:, j, c * 512 : (c + 1) * 512],
                            in0=yT[:, j, c * 512 : (c + 1) * 512],
                            in1=rb[:],
                        )

            # ---- c_proj + ReduceScatter ----
            z_dram = dramp.tile([T, C], BF16)
            rs_out = dramp.tile([T // 2, C], BF16)
            rows = T // RS_CHUNKS  # 512
            half = rows // 2  # 256

            def proj_mm(tt, wp_sb, bp_sb):
                for n in range(C // 512):
                    ps = avmm_ps.tile([P, 1024], F32, tag="avmm", name="pj_ps")
                    for cc in range(HC // P):
                        nc.tensor.matmul(
                            ps[:, 0:512],
                            yT[:, cc, tt * P : (tt + 1) * P],
                            wp_sb[:, cc, n * 512 : (n + 1) * 512],
                            start=(cc == 0),
                            stop=(cc == HC // P - 1),
                        )
                    z_sb = zoutp.tile([P, 512], BF16, tag="z", name="z_sb")
                    nc.vector.tensor_add(
                        out=z_sb[:],
                        in0=ps[:, 0:512],
                        in1=bp_sb[:, n * 512 : (n + 1) * 512],
                    )
                    nc.sync.dma_start(
                        z_dram[tt * P : (tt + 1) * P, n * 512 : (n + 1) * 512],
                        z_sb[:],
                    )

            def rs_tail(rc):
                nc.gpsimd.collective_compute(
                    "ReduceScatter",
                    mybir.AluOpType.add,
                    replica_groups=[[0, 1], [2, 3], [4, 5], [6, 7]],
                    ins=[z_dram[rc * rows : (rc + 1) * rows, :].opt()],
                    outs=[rs_out[rc * half : (rc + 1) * half, :].opt()],
                )
                nc.sync.dma_start(
                    out_d[rc * half : (rc + 1) * half, :],
                    rs_out[rc * half : (rc + 1) * half, :],
                )

            def proj_rs(rc, wp_sb, bp_sb):
                tt_per_chunk = TT // RS_CHUNKS
                for tt in range(rc * tt_per_chunk, (rc + 1) * tt_per_chunk):
                    proj_mm(tt, wp_sb, bp_sb)
                rs_tail(rc)

            # ---- schedule ----
            qkproj(0)
            score_exp(0, 0)
            qkproj(1)
            vproj(0, 8)
            score_exp(1, 0)
            av_unit(0, 0)
            qkproj(2)
            vproj(8, 16)
            score_exp(2, 0)
            av_unit(1, 0)
            qkproj(3)
            score_exp(3, 0)
            av_unit(2, 0)
            av_unit(3, 0)

            nc.gpsimd.dma_start(wp_sb[:], wp_d[:])
            bp_sb, bp_free = tc.tile([P, C], mybir.dt.float8e4, name="bp_sb")
            nc.gpsimd.dma_start(bp_sb[:], bp_d[:])

            # u1 phase is ACT(exp)-bound: spread proj(0)/proj(1) tt-slices
            # across the unit boundaries as PE filler.
            score_exp(0, 1)
            batch_norm(0, 2)
            proj_mm(0, wp_sb, bp_sb)
            proj_mm(1, wp_sb, bp_sb)
            score_exp(1, 1)
            proj_mm(2, wp_sb, bp_sb)
            proj_mm(3, wp_sb, bp_sb)
            rs_tail(0)
            av_unit(0, 1)
            proj_mm(4, wp_sb, bp_sb)
            score_exp(2, 1)
            proj_mm(5, wp_sb, bp_sb)
            proj_mm(6, wp_sb, bp_sb)
            av_unit(1, 1)
            proj_mm(7, wp_sb, bp_sb)
            rs_tail(1)
            score_exp(3, 1)
            # chunk-major tail: finish every head pair's chunk 2 first so
            # proj/RS of rows 1024-1535 overlap the chunk-3 AV work
            av_unit(2, 1, chunks=(2,))
            av_unit(3, 1, chunks=(2,))
            batch_norm(2, 3)
            for tt in range(8, 12):
                proj_mm(tt, wp_sb, bp_sb)
            rs_tail(2)
            av_unit(2, 1, chunks=(3,))
            av_unit(3, 1, chunks=(3,))
            batch_norm(3, 4)
            for tt in range(12, 16):
                proj_mm(tt, wp_sb, bp_sb)
            rs_tail(3)
            bp_free()
            wp_free()

            if _DEBUG_DUMP:
                qT_o = nc.dram_tensor("qT_o", [P, NP, T], BF16, kind="ExternalOutput")
                kT_o = nc.dram_tensor("kT_o", [P, NP, T], BF16, kind="ExternalOutput")
                v_o = nc.dram_tensor(
                    "v_o", [P, TT, HL, D + 1], BF16, kind="ExternalOutput"
                )
                yT_o = nc.dram_tensor("yT_o", [P, NP, T], BF16, kind="ExternalOutput")
                ar_o = nc.dram_tensor(
                    "ar_o", [P, 64 * 1024], BF16, kind="ExternalOutput"
                )
                nc.sync.dma_start(qT_o[:], qT[:])
                nc.sync.dma_start(kT_o[:], kT[:])
                nc.sync.dma_start(v_o[:], v_ext[:])
                nc.sync.dma_start(yT_o[:], yT[:])
                nc.sync.dma_start(ar_o[:], arena[:])

    nc.compile()
    return nc


def _in_maps(inputs):
    x = np.asarray(inputs["x"], dtype=np.float32)
    w_attn = np.asarray(inputs["w_attn"], dtype=np.float32)
    b_attn = np.asarray(inputs["b_attn"], dtype=np.float32)
    w_proj = np.asarray(inputs["w_proj"], dtype=np.float32)
    b_proj = np.asarray(inputs["b_proj"], dtype=np.float32)

    maps = []
    for core in range(N_CORES):
        b, g = core // 2, core % 2
        s = g * HC
        # x[b] [T, C] -> xT [q4, ki, ck, t%512] with c = ck*128 + ki,
        # each t-quarter contiguous for clean DMA
        xT = (
            x[b]
            .reshape(4, 512, CK, P)
            .transpose(0, 3, 2, 1)
            .astype(ml_dtypes.bfloat16)
        )
        # [C, HC] -> [ki, j, ko, n] with c = ko*128+ki, qcol = j*128+n
        wq = (
            w_attn[:, s : s + HC]
            .reshape(CK, P, NP, P)
            .transpose(1, 2, 0, 3)
            .astype(ml_dtypes.bfloat16)
        )
        wk = (
            w_attn[:, C + s : C + s + HC]
            .reshape(CK, P, NP, P)
            .transpose(1, 2, 0, 3)
            .astype(ml_dtypes.bfloat16)
        )
        # [C, HC] -> [ki, ko, vcol]
        wv = (
            w_attn[:, 2 * C + s : 2 * C + s + HC]
            .reshape(CK, P, HC)
            .transpose(1, 0, 2)
            .astype(ml_dtypes.bfloat16)
        )
        # [HC, C] -> [ki, ko, co], bf16
        wp = (
            w_proj[s : s + HC, :]
            .reshape(HC // P, P, C)
            .transpose(1, 0, 2)
            .astype(ml_dtypes.bfloat16)
        )
        bq = b_attn[s : s + HC].reshape(NP, P).T
        bk = b_attn[C + s : C + s + HC].reshape(NP, P).T
        bv = np.broadcast_to(
            b_attn[2 * C + s : 2 * C + s + HC], (P, HC)
        ).astype(ml_dtypes.bfloat16)
        bp = (
            np.broadcast_to(b_proj, (P, C)).astype(ml_dtypes.float8_e4m3)
            if g == 0
            else np.zeros((P, C), ml_dtypes.float8_e4m3)
        )
        maps.append(
            {
                "xT": np.ascontiguousarray(xT),
                "wq": np.ascontiguousarray(wq),
                "wk": np.ascontiguousarray(wk),
                "wv": np.ascontiguousarray(wv),
                "wp": np.ascontiguousarray(wp),
                "bq": np.ascontiguousarray(bq),
                "bk": np.ascontiguousarray(bk),
                "bv": np.ascontiguousarray(bv),
                "bp": np.ascontiguousarray(bp),
            }
        )
    return maps


def _run(inputs, trace=False, trace_cores=None):
    if "nc" not in _CACHE:
        _CACHE["nc"] = _build_nc()
    nc = _CACHE["nc"]
    res = run_bass_kernel_spmd(
        nc,
        _in_maps(inputs),
        list(range(N_CORES)),
        trace=trace,
        trace_cores=trace_cores,
    )
    # chunked RS ownership: even core holds rows [512c, 512c+256),
    # odd core holds rows [512c+256, 512c+512), for c = 0..3
    out = np.empty((B, T, C), np.float32)
    rows = T // RS_CHUNKS
    half = rows // 2
    for b in range(B):
        ev = res.results[2 * b]["out"].astype(np.float32)
        od = res.results[2 * b + 1]["out"].astype(np.float32)
        for rc in range(RS_CHUNKS):
            out[b, rc * rows : rc * rows + half] = ev[rc * half : (rc + 1) * half]
            out[b, rc * rows + half : (rc + 1) * rows] = od[
                rc * half : (rc + 1) * half
            ]
    return out, res


def kernel(**inputs):
    out, _ = _run(inputs)
    return out


# revision 43
# speedup vs baseline: 1.4795x; 1.0127x over previous
"""Causal self-attention (B=4, T=2048, C=1024, H=16) on 8 Trainium2 cores.

Sharding: core c -> batch b = c//2, head-group g = c%2 (8 heads each,
tensor-parallel). QKV + attention + c_proj computed per core on its head
slice; partial c_proj outputs of a (b) pair are summed with chunked
on-device ReduceScatters over the T dimension; host reassembles.

v2: x is pre-transposed/cast to bf16 on the host (layout only, like the
weight reshapes), AV uses v-stationary matmuls streaming 512 queries so
the PE array stays busy (HAM warm), softmax denominators ride as a ones
column of v, and the causal mask is applied in-place on PSUM with one
affine_select per diagonal block.

Self-contained: only imports concourse (installed library) + numpy.
"""

import ml_dtypes
import numpy as np

import concourse.mybir as mybir
import concourse.tile as tile
from concourse import bacc
from concourse.bass_utils import run_bass_kernel_spmd
from concourse.masks import make_identity

B, T, C = 4, 2048, 1024
H_TOTAL, D = 16, 64
N_CORES = 8
HL = H_TOTAL // 2  # local heads per core (8)
HC = HL * D  # local head cols (512)
NP = HL // 2  # head pairs (4)
P = 128
TT = T // P  # 16 t-chunks of 128
CK = C // P  # 8 contraction chunks for qkv
RS_CHUNKS = 4
F32 = mybir.dt.float32
BF16 = mybir.dt.bfloat16
MASK_VAL = -480.0  # -60 after the 1/8 attention scale; exp(-60) ~ 0
SCALE = 1.0 / 8.0  # 1/sqrt(D)

_CACHE = {}
_DEBUG_DUMP = False


def _build_nc():
    nc = bacc.Bacc("TRN2", target_bir_lowering=False, debug=False, num_devices=N_CORES)

    # x pre-transposed and cast on host: [ki, ck, t]
    xT_d = nc.dram_tensor("xT", [4, P, CK, 512], BF16, kind="ExternalInput")
    wq_d = nc.dram_tensor("wq", [P, NP, CK, P], BF16, kind="ExternalInput")
    wk_d = nc.dram_tensor("wk", [P, NP, CK, P], BF16, kind="ExternalInput")
    wv_d = nc.dram_tensor("wv", [P, CK, HC], BF16, kind="ExternalInput")
    bq_d = nc.dram_tensor("bq", [P, NP], F32, kind="ExternalInput")
    bk_d = nc.dram_tensor("bk", [P, NP], F32, kind="ExternalInput")
    bv_d = nc.dram_tensor("bv", [P, HC], BF16, kind="ExternalInput")
    wp_d = nc.dram_tensor("wp", [P, HC // P, C], BF16, kind="ExternalInput")
    bp_d = nc.dram_tensor("bp", [P, C], mybir.dt.float8e4, kind="ExternalInput")
    out_d = nc.dram_tensor("out", [T // 2, C], BF16, kind="ExternalOutput")

    with tile.TileContext(nc) as tc:
        with (
            tc.tile_pool(name="const", bufs=1) as constp,
            tc.tile_pool(name="big", bufs=1) as bigp,
            tc.tile_pool(name="rnorm", bufs=1) as rnp,
            tc.tile_pool(name="zout", bufs=1) as zoutp,
            tc.tile_pool(name="score_ps", bufs=1, space="PSUM") as score_ps,
            tc.tile_pool(name="avmm_ps", bufs=2, space="PSUM") as avmm_ps,
            tc.tile_pool(name="dram", bufs=1, space="DRAM") as dramp,
        ):
            # ---- constants ----
            # dmaskT @ ident seeds the diagonal PSUM block with the causal
            # mask on the PE itself (keeps DVE out of the score->exp chain):
            # dmaskT[p, c] = MASK_VAL where p < c, so (dmaskT^T I)[s, c] =
            # dmaskT[c, s] = MASK_VAL where c < s.
            dmaskT = constp.tile([P, P], BF16)
            nc.vector.memset(dmaskT[:], 0.0)
            nc.gpsimd.affine_select(
                out=dmaskT[:],
                in_=dmaskT[:],
                compare_op=mybir.AluOpType.is_ge,
                fill=MASK_VAL,
                base=0,
                pattern=[[-1, P]],
                channel_multiplier=1,
            )
            ident_bf = constp.tile([P, P], BF16)
            make_identity(nc, ident_bf)
            bq_sb = constp.tile([P, NP], F32)
            nc.sync.dma_start(bq_sb[:], bq_d[:])
            bk_sb = constp.tile([P, NP], F32)
            nc.sync.dma_start(bk_sb[:], bk_d[:])
            # reserve wp space early (needs 8KB contiguous; DMA'd later)
            wp_sb, wp_free = tc.tile([P, HC // P, C], BF16, name="wp_sb")

            # ---- persistent activations ----
            qT = bigp.tile([P, NP, T], BF16)  # q^T [qcol, t]
            kT = bigp.tile([P, NP, T], BF16)  # k^T [kcol, t]
            v_ext = bigp.tile([P, TT, HL, D + 1], BF16)  # v with ones col
            nc.vector.memset(v_ext[:, :, :, D : D + 1], 1.0)
            yT = bigp.tile([P, NP, T], BF16)  # y^T [ci, t]
            # p/xT arena: 64K bf16 elems per partition (128 KB).
            #   u0 p-pair slots (8x2048 = 16K elems): j%3 -> [0,16K),[16K,32K),[32K,48K)
            #   xT (8x2048): [48K, 64K)
            #   u1 p-pair slots (16x2048 = 32K elems): j%2 -> [0,32K),[32K,64K)
            arena = bigp.tile([P, 64 * 1024], BF16)

            def p_view(j, u):
                if u == 0:
                    off = 16384 * (j % 3)
                    return arena[:, off : off + 16384].rearrange(
                        "p (i q) -> p i q", q=2048
                    )
                off = 32768 * (j % 2)
                return arena[:, off : off + 32768].rearrange(
                    "p (i q) -> p i q", q=2048
                )

            xT = arena[:, 49152:65536].rearrange("p (c t) -> p c t", t=T)
            d_all = dramp.tile([16, 1024], BF16, name="d_all")
            r_all = dramp.tile([16, 1024], BF16, name="r_all")

            # xT DMA in t-quarters (each contiguous in DRAM) so qkproj(0)
            # starts after ~3 us
            for q4 in range(4):
                nc.sync.dma_start(
                    xT[:, :, q4 * 512 : (q4 + 1) * 512],
                    xT_d[q4],
                )

            # ---- QKV projections ----
            # transient wq/wk tiles live in wp_sb's space (wp is DMA'd only
            # after the last qkproj read; 4 rotating 2KB slots)
            wp_flat = wp_sb[:].rearrange("p a b -> p (a b)")

            def qkproj(j):
                for si, (w_d, b_sb, dstT) in enumerate(
                    ((wq_d, bq_sb, qT), (wk_d, bk_sb, kT))
                ):
                    slot = (2 * j + si) % 4
                    wj = wp_flat[:, slot * 1024 : (slot + 1) * 1024].rearrange(
                        "p (c k) -> p c k", k=P
                    )
                    nc.sync.dma_start(wj[:], w_d[:, j])
                    for u4 in range(T // 512):
                        ps = avmm_ps.tile([P, 1024], F32, tag="avmm", name="qk_ps")
                        for ck in range(CK):
                            nc.tensor.matmul(
                                ps[:, 0:512],
                                wj[:, ck, :],
                                xT[:, ck, u4 * 512 : (u4 + 1) * 512],
                                start=(ck == 0),
                                stop=(ck == CK - 1),
                            )
                        nc.vector.tensor_add(
                            out=dstT[:, j, u4 * 512 : (u4 + 1) * 512],
                            in0=ps[:, 0:512],
                            in1=b_sb[:, j : j + 1].to_broadcast((P, 512)),
                        )

            # wv and bv live in u0 p-slot 2 of the arena: all reads (vproj)
            # are scheduled before score_exp(2, 0) overwrites that slot.
            wv_sb = arena[:, 32768:36864].rearrange("p (c v) -> p c v", v=HC)
            nc.sync.dma_start(wv_sb[:], wv_d[:])
            bv_sb = arena[:, 36864:37376]
            nc.sync.dma_start(bv_sb[:], bv_d[:])

            def vproj(tt_lo, tt_hi):
                for tt in range(tt_lo, tt_hi):
                    ps = avmm_ps.tile([P, 1024], F32, tag="avmm", name="v_ps")
                    for ck in range(CK):
                        nc.tensor.matmul(
                            ps[:, 0:512],
                            xT[:, ck, tt * P : (tt + 1) * P],
                            wv_sb[:, ck, :],
                            start=(ck == 0),
                            stop=(ck == CK - 1),
                        )
                    nc.vector.tensor_add(
                        out=v_ext[:, tt, :, 0:D],
                        in0=ps[:, 0:512].rearrange("p (h d) -> p h d", d=D),
                        in1=bv_sb[:].rearrange("p (h d) -> p h d", d=D),
                    )

            # ---- scores + exp ----
            def score_exp(j, u):
                n_i = 8 * (u + 1)
                pt = p_view(j, u)
                # one PSUM tile per head: double-buffered across units so the
                # next unit's matmuls run while this unit's exp drains. The
                # two heads' matmuls still pair up via row groups 0/64.
                ps2 = [
                    score_ps.tile([P, 1024], F32, tag="score", name=f"sc{hh}")
                    for hh in range(2)
                ]
                for i in range(n_i):
                    d0 = i * P - 1024 * u  # diag block col (within unit)
                    c0 = max(0, d0)
                    jj_diag = 2 * u + d0 // 512 if i >= 8 * u else -1
                    for hh in range(2):
                        hb = hh * D
                        for jj in range(2 * u, 2 * u + 2):
                            if jj < i // 4:
                                continue
                            c0j = (jj - 2 * u) * 512
                            if jj != jj_diag:
                                nc.tensor.matmul(
                                    ps2[hh][:, c0j : c0j + 512],
                                    kT[hb : hb + D, j, i * P : (i + 1) * P],
                                    qT[hb : hb + D, j, jj * 512 : (jj + 1) * 512],
                                    start=True,
                                    stop=True,
                                )
                                continue
                            # diag block: seed [d0, d0+128) with the causal
                            # mask, accumulate scores on top; cols left of the
                            # diag are above-diagonal (never exp'd/streamed),
                            # cols right of it get their own fresh matmul.
                            nc.tensor.matmul(
                                ps2[hh][:, d0 : d0 + P],
                                dmaskT[:],
                                ident_bf[:],
                                start=True,
                                stop=False,
                            )
                            nc.tensor.matmul(
                                ps2[hh][:, d0 : d0 + P],
                                kT[hb : hb + D, j, i * P : (i + 1) * P],
                                qT[hb : hb + D, j, d0 + 1024 * u : d0 + 1024 * u + P],
                                start=False,
                                stop=True,
                            )
                            hi = c0j + 512
                            if d0 + P < hi:
                                nc.tensor.matmul(
                                    ps2[hh][:, d0 + P : hi],
                                    kT[hb : hb + D, j, i * P : (i + 1) * P],
                                    qT[
                                        hb : hb + D,
                                        j,
                                        d0 + P + 1024 * u : hi + 1024 * u,
                                    ],
                                    start=True,
                                    stop=True,
                                )
                    for hh in range(2):
                        nc.scalar.activation(
                            out=pt[:, i, hh * 1024 + c0 : (hh + 1) * 1024],
                            in_=ps2[hh][:, c0:1024],
                            func=mybir.ActivationFunctionType.Exp,
                            scale=SCALE,
                        )

            # ---- attention-value product, v-stationary ----
            # out[0:64] = y^T (unnormalized), out[64] = softmax denominator
            # via the ones column of v_ext; p streams 512 queries per matmul.
            def av_unit(j, u, chunks=None):
                pt = p_view(j, u)
                for c in chunks if chunks is not None else (2 * u, 2 * u + 1):
                    i_max = 4 * (c + 1)
                    qo = (c - 2 * u) * 512
                    av = avmm_ps.tile([P, 1024], F32, tag="avmm", name=f"av{j}{c}")
                    for hh in range(2):
                        h = 2 * j + hh
                        for i in range(i_max):
                            # stream only cols at/after the causal boundary:
                            # p[:, i, :lo] above the diagonal is never written
                            lo = max(qo, i * P - 1024 * u)
                            nc.tensor.matmul(
                                av[0 : D + 1, hh * 512 + lo - qo : hh * 512 + 512],
                                v_ext[:, i, h, :],
                                pt[:, i, hh * 1024 + lo : hh * 1024 + qo + 512],
                                start=(i == 0),
                                stop=(i == i_max - 1),
                            )
                    # Evacuate PSUM immediately (PE never waits on norm):
                    # even head's unnormalized y -> yT rows 0-63 directly,
                    # odd head's + denominator row -> avout staging, then a
                    # DMA partition-shift puts odd y at yT rows 64-127 and
                    # the denominators accumulate in DRAM for batch_norm.
                    cid = 4 * c + j
                    avout = rnp.tile(
                        [P, 1024], BF16, tag="avout", name="avout", bufs=2
                    )
                    nc.vector.tensor_copy(
                        out=yT[0:D, j, c * 512 : (c + 1) * 512],
                        in_=av[0:D, 0:512],
                    )
                    nc.vector.tensor_copy(
                        out=avout[0:D, 512:1024], in_=av[0:D, 512:1024]
                    )
                    nc.vector.tensor_copy(
                        out=avout[D : D + 1, :], in_=av[D : D + 1, :]
                    )
                    nc.scalar.dma_start(
                        yT[D:P, j, c * 512 : (c + 1) * 512],
                        avout[0:D, 512:1024],
                    )
                    nc.scalar.dma_start(d_all[cid], avout[D : D + 1, :])

            # Batched normalization: one transposing DRAM round-trip turns
            # the [n, 1024] denominator rows into a [128, 8n] column, one
            # lane-parallel reciprocal (DVE recip costs ~9 cyc per FREE
            # element), write back transposed, then broadcast-read each
            # chunk's recips and scale yT in place (rows 0-63 even head,
            # 64-127 odd head).
            def batch_norm(c_lo, c_hi):
                lo, hi = 4 * c_lo, 4 * c_hi
                n = hi - lo
                rb0 = rnp.tile([P, 512], BF16, tag="rb", name="rb")
                dcb = rb0[:, 0:64]
                nc.scalar.dma_start(
                    dcb[:, 0 : 8 * n],
                    d_all[lo:hi, :].rearrange("n (c p) -> p (n c)", p=P),
                )
                with nc.allow_low_precision(
                    reason="bf16 1/denom: ~0.4% rel on y, within tolerance"
                ):
                    nc.vector.reciprocal(dcb[:, 0 : 8 * n], dcb[:, 0 : 8 * n])
                nc.scalar.dma_start(
                    r_all[lo:hi, :].rearrange("n (c p) -> p (n c)", p=P),
                    dcb[:, 0 : 8 * n],
                )
                for c in range(c_lo, c_hi):
                    for j in range(NP):
                        cid = 4 * c + j
                        rb = rnp.tile([P, 512], BF16, tag="rb", name="rb")
                        nc.scalar.dma_start(
                            rb[0:D, :],
                            r_all[cid : cid + 1, 0:512].to_broadcast((D, 512)),
                        )
                        nc.scalar.dma_start(
                            rb[D:P, :],
                            r_all[cid : cid + 1, 512:1024].to_broadcast(
                                (D, 512)
                            ),
                        )
                        nc.vector.tensor_mul(
                            out=yT[:, j, c * 512 : (c + 1) * 512],
                            in0=yT[:, j, c * 512 : (c + 1) * 512],
                            in1=rb[:],
                        )

            # ---- c_proj + ReduceScatter ----
            z_dram = dramp.tile([T, C], BF16)
            rs_out = dramp.tile([T // 2, C], BF16)
            rows = T // RS_CHUNKS  # 512
            half = rows // 2  # 256

            def proj_mm(tt, wp_sb, bp_sb):
                for n in range(C // 512):
                    ps = avmm_ps.tile([P, 1024], F32, tag="avmm", name="pj_ps")
                    for cc in range(HC // P):
                        nc.tensor.matmul(
                            ps[:, 0:512],
                            yT[:, cc, tt * P : (tt + 1) * P],
                            wp_sb[:, cc, n * 512 : (n + 1) * 512],
                            start=(cc == 0),
                            stop=(cc == HC // P - 1),
                        )
                    z_sb = zoutp.tile([P, 512], BF16, tag="z", name="z_sb")
                    nc.vector.tensor_add(
                        out=z_sb[:],
                        in0=ps[:, 0:512],
                        in1=bp_sb[:, n * 512 : (n + 1) * 512],
                    )
                    nc.sync.dma_start(
                        z_dram[tt * P : (tt + 1) * P, n * 512 : (n + 1) * 512],
                        z_sb[:],
                    )

            def rs_tail(rc):
                nc.gpsimd.collective_compute(
                    "ReduceScatter",
                    mybir.AluOpType.add,
                    replica_groups=[[0, 1], [2, 3], [4, 5], [6, 7]],
                    ins=[z_dram[rc * rows : (rc + 1) * rows, :].opt()],
                    outs=[rs_out[rc * half : (rc + 1) * half, :].opt()],
                )
                nc.sync.dma_start(
                    out_d[rc * half : (rc + 1) * half, :],
                    rs_out[rc * half : (rc + 1) * half, :],
                )

            def proj_rs(rc, wp_sb, bp_sb):
                tt_per_chunk = TT // RS_CHUNKS
                for tt in range(rc * tt_per_chunk, (rc + 1) * tt_per_chunk):
                    proj_mm(tt, wp_sb, bp_sb)
                rs_tail(rc)

            # ---- schedule ----
            qkproj(0)
            score_exp(0, 0)
            qkproj(1)
            vproj(0, 8)
            score_exp(1, 0)
            av_unit(0, 0)
            qkproj(2)
            vproj(8, 16)
            score_exp(2, 0)
            av_unit(1, 0)
            qkproj(3)
            score_exp(3, 0)
            av_unit(2, 0)
            av_unit(3, 0)

            nc.sync.dma_start(wp_sb[:], wp_d[:])
            bp_sb, bp_free = tc.tile([P, C], mybir.dt.float8e4, name="bp_sb")
            nc.sync.dma_start(bp_sb[:], bp_d[:])

            # u1 phase is ACT(exp)-bound: spread proj(0)/proj(1) tt-slices
            # across the unit boundaries as PE filler.
            score_exp(0, 1)
            batch_norm(0, 2)
            proj_mm(0, wp_sb, bp_sb)
            proj_mm(1, wp_sb, bp_sb)
            score_exp(1, 1)
            proj_mm(2, wp_sb, bp_sb)
            proj_mm(3, wp_sb, bp_sb)
            rs_tail(0)
            av_unit(0, 1)
            proj_mm(4, wp_sb, bp_sb)
            score_exp(2, 1)
            proj_mm(5, wp_sb, bp_sb)
            proj_mm(6, wp_sb, bp_sb)
            av_unit(1, 1)
            proj_mm(7, wp_sb, bp_sb)
            rs_tail(1)
            score_exp(3, 1)
            # chunk-major tail: finish every head pair's chunk 2 first so
            # proj/RS of rows 1024-1535 overlap the chunk-3 AV work
            av_unit(2, 1, chunks=(2,))
            av_unit(3, 1, chunks=(2,))
            batch_norm(2, 3)
            for tt in range(8, 12):
                proj_mm(tt, wp_sb, bp_sb)
            rs_tail(2)
            av_unit(2, 1, chunks=(3,))
            av_unit(3, 1, chunks=(3,))
            batch_norm(3, 4)
            for tt in range(12, 16):
                proj_mm(tt, wp_sb, bp_sb)
            rs_tail(3)
            bp_free()
            wp_free()

            if _DEBUG_DUMP:
                qT_o = nc.dram_tensor("qT_o", [P, NP, T], BF16, kind="ExternalOutput")
                kT_o = nc.dram_tensor("kT_o", [P, NP, T], BF16, kind="ExternalOutput")
                v_o = nc.dram_tensor(
                    "v_o", [P, TT, HL, D + 1], BF16, kind="ExternalOutput"
                )
                yT_o = nc.dram_tensor("yT_o", [P, NP, T], BF16, kind="ExternalOutput")
                ar_o = nc.dram_tensor(
                    "ar_o", [P, 64 * 1024], BF16, kind="ExternalOutput"
                )
                nc.sync.dma_start(qT_o[:], qT[:])
                nc.sync.dma_start(kT_o[:], kT[:])
                nc.sync.dma_start(v_o[:], v_ext[:])
                nc.sync.dma_start(yT_o[:], yT[:])
                nc.sync.dma_start(ar_o[:], arena[:])

    nc.compile()
    return nc


def _in_maps(inputs):
    x = np.asarray(inputs["x"], dtype=np.float32)
    w_attn = np.asarray(inputs["w_attn"], dtype=np.float32)
    b_attn = np.asarray(inputs["b_attn"], dtype=np.float32)
    w_proj = np.asarray(inputs["w_proj"], dtype=np.float32)
    b_proj = np.asarray(inputs["b_proj"], dtype=np.float32)

    maps = []
    for core in range(N_CORES):
        b, g = core // 2, core % 2
        s = g * HC
        # x[b] [T, C] -> xT [q4, ki, ck, t%512] with c = ck*128 + ki,
        # each t-quarter contiguous for clean DMA
        xT = (
            x[b]
            .reshape(4, 512, CK, P)
            .transpose(0, 3, 2, 1)
            .astype(ml_dtypes.bfloat16)
        )
        # [C, HC] -> [ki, j, ko, n] with c = ko*128+ki, qcol = j*128+n
        wq = (
            w_attn[:, s : s + HC]
            .reshape(CK, P, NP, P)
            .transpose(1, 2, 0, 3)
            .astype(ml_dtypes.bfloat16)
        )
        wk = (
            w_attn[:, C + s : C + s + HC]
            .reshape(CK, P, NP, P)
            .transpose(1, 2, 0, 3)
            .astype(ml_dtypes.bfloat16)
        )
        # [C, HC] -> [ki, ko, vcol]
        wv = (
            w_attn[:, 2 * C + s : 2 * C + s + HC]
            .reshape(CK, P, HC)
            .transpose(1, 0, 2)
            .astype(ml_dtypes.bfloat16)
        )
        # [HC, C] -> [ki, ko, co], bf16
        wp = (
            w_proj[s : s + HC, :]
            .reshape(HC // P, P, C)
            .transpose(1, 0, 2)
            .astype(ml_dtypes.bfloat16)
        )
        bq = b_attn[s : s + HC].reshape(NP, P).T
        bk = b_attn[C + s : C + s + HC].reshape(NP, P).T
        bv = np.broadcast_to(
            b_attn[2 * C + s : 2 * C + s + HC], (P, HC)
        ).astype(ml_dtypes.bfloat16)
        bp = (
            np.broadcast_to(b_proj, (P, C)).astype(ml_dtypes.float8_e4m3)
            if g == 0
            else np.zeros((P, C), ml_dtypes.float8_e4m3)
        )
        maps.append(
            {
                "xT": np.ascontiguousarray(xT),
                "wq": np.ascontiguousarray(wq),
                "wk": np.ascontiguousarray(wk),
                "wv": np.ascontiguousarray(wv),
                "wp": np.ascontiguousarray(wp),
                "bq": np.ascontiguousarray(bq),
                "bk": np.ascontiguousarray(bk),
                "bv": np.ascontiguousarray(bv),
                "bp": np.ascontiguousarray(bp),
            }
        )
    return maps


def _run(inputs, trace=False, trace_cores=None):
    if "nc" not in _CACHE:
        _CACHE["nc"] = _build_nc()
    nc = _CACHE["nc"]
    res = run_bass_kernel_spmd(
        nc,
        _in_maps(inputs),
        list(range(N_CORES)),
        trace=trace,
        trace_cores=trace_cores,
    )
    # chunked RS ownership: even core holds rows [512c, 512c+256),
    # odd core holds rows [512c+256, 512c+512), for c = 0..3
    out = np.empty((B, T, C), np.float32)
    rows = T // RS_CHUNKS
    half = rows // 2
    for b in range(B):
        ev = res.results[2 * b]["out"].astype(np.float32)
        od = res.results[2 * b + 1]["out"].astype(np.float32)
        for rc in range(RS_CHUNKS):
            out[b, rc * rows : rc * rows + half] = ev[rc * half : (rc + 1) * half]
            out[b, rc * rows + half : (rc + 1) * rows] = od[
                rc * half : (rc + 1) * half
            ]
    return out, res


def kernel(**inputs):
    out, _ = _run(inputs)
    return out


# revision 44
# speedup vs baseline: 1.4847x; 1.0036x over previous
"""Causal self-attention (B=4, T=2048, C=1024, H=16) on 8 Trainium2 cores.

Sharding: core c -> batch b = c//2, head-group g = c%2 (8 heads each,
tensor-parallel). QKV + attention + c_proj computed per core on its head
slice; partial c_proj outputs of a (b) pair are summed with chunked
on-device ReduceScatters over the T dimension; host reassembles.

v2: x is pre-transposed/cast to bf16 on the host (layout only, like the
weight reshapes), AV uses v-stationary matmuls streaming 512 queries so
the PE array stays busy (HAM warm), softmax denominators ride as a ones
column of v, and the causal mask is applied in-place on PSUM with one
affine_select per diagonal block.

Self-contained: only imports concourse (installed library) + numpy.
"""

import ml_dtypes
import numpy as np

import concourse.mybir as mybir
import concourse.tile as tile
from concourse import bacc
from concourse.bass_utils import run_bass_kernel_spmd
from concourse.masks import make_identity

B, T, C = 4, 2048, 1024
H_TOTAL, D = 16, 64
N_CORES = 8
HL = H_TOTAL // 2  # local heads per core (8)
HC = HL * D  # local head cols (512)
NP = HL // 2  # head pairs (4)
P = 128
TT = T // P  # 16 t-chunks of 128
CK = C // P  # 8 contraction chunks for qkv
RS_CHUNKS = 4
F32 = mybir.dt.float32
BF16 = mybir.dt.bfloat16
MASK_VAL = -480.0  # -60 after the 1/8 attention scale; exp(-60) ~ 0
SCALE = 1.0 / 8.0  # 1/sqrt(D)

_CACHE = {}
_DEBUG_DUMP = False


def _build_nc():
    nc = bacc.Bacc("TRN2", target_bir_lowering=False, debug=False, num_devices=N_CORES)

    # x pre-transposed and cast on host: [ki, ck, t]
    xT_d = nc.dram_tensor("xT", [4, P, CK, 512], BF16, kind="ExternalInput")
    wq_d = nc.dram_tensor("wq", [P, NP, CK, P], BF16, kind="ExternalInput")
    wk_d = nc.dram_tensor("wk", [P, NP, CK, P], BF16, kind="ExternalInput")
    wv_d = nc.dram_tensor("wv", [P, CK, HC], BF16, kind="ExternalInput")
    bq_d = nc.dram_tensor("bq", [P, NP], F32, kind="ExternalInput")
    bk_d = nc.dram_tensor("bk", [P, NP], F32, kind="ExternalInput")
    bv_d = nc.dram_tensor("bv", [P, HC], BF16, kind="ExternalInput")
    wp_d = nc.dram_tensor("wp", [P, HC // P, C], BF16, kind="ExternalInput")
    bp_d = nc.dram_tensor("bp", [P, C], mybir.dt.float8e4, kind="ExternalInput")
    out_d = nc.dram_tensor("out", [T // 2, C], BF16, kind="ExternalOutput")

    with tile.TileContext(nc) as tc:
        with (
            tc.tile_pool(name="const", bufs=1) as constp,
            tc.tile_pool(name="big", bufs=1) as bigp,
            tc.tile_pool(name="rnorm", bufs=1) as rnp,
            tc.tile_pool(name="zout", bufs=1) as zoutp,
            tc.tile_pool(name="score_ps", bufs=1, space="PSUM") as score_ps,
            tc.tile_pool(name="avmm_ps", bufs=2, space="PSUM") as avmm_ps,
            tc.tile_pool(name="dram", bufs=1, space="DRAM") as dramp,
        ):
            # ---- constants ----
            # dmaskT @ ident seeds the diagonal PSUM block with the causal
            # mask on the PE itself (keeps DVE out of the score->exp chain):
            # dmaskT[p, c] = MASK_VAL where p < c, so (dmaskT^T I)[s, c] =
            # dmaskT[c, s] = MASK_VAL where c < s.
            dmaskT = constp.tile([P, P], BF16)
            nc.vector.memset(dmaskT[:], 0.0)
            nc.gpsimd.affine_select(
                out=dmaskT[:],
                in_=dmaskT[:],
                compare_op=mybir.AluOpType.is_ge,
                fill=MASK_VAL,
                base=0,
                pattern=[[-1, P]],
                channel_multiplier=1,
            )
            ident_bf = constp.tile([P, P], BF16)
            make_identity(nc, ident_bf)
            bq_sb = constp.tile([P, NP], F32)
            nc.sync.dma_start(bq_sb[:], bq_d[:])
            bk_sb = constp.tile([P, NP], F32)
            nc.sync.dma_start(bk_sb[:], bk_d[:])
            # reserve wp space early (needs 8KB contiguous; DMA'd later)
            wp_sb, wp_free = tc.tile([P, HC // P, C], BF16, name="wp_sb")

            # ---- persistent activations ----
            qT = bigp.tile([P, NP, T], BF16)  # q^T [qcol, t]
            kT = bigp.tile([P, NP, T], BF16)  # k^T [kcol, t]
            v_ext = bigp.tile([P, TT, HL, D + 1], BF16)  # v with ones col
            nc.vector.memset(v_ext[:, :, :, D : D + 1], 1.0)
            yT = bigp.tile([P, NP, T], BF16)  # y^T [ci, t]
            # p/xT arena: 64K bf16 elems per partition (128 KB).
            #   u0 p-pair slots (8x2048 = 16K elems): j%3 -> [0,16K),[16K,32K),[32K,48K)
            #   xT (8x2048): [48K, 64K)
            #   u1 p-pair slots (16x2048 = 32K elems): j%2 -> [0,32K),[32K,64K)
            arena = bigp.tile([P, 64 * 1024], BF16)

            def p_view(j, u):
                if u == 0:
                    off = 16384 * (j % 3)
                    return arena[:, off : off + 16384].rearrange(
                        "p (i q) -> p i q", q=2048
                    )
                off = 32768 * (j % 2)
                return arena[:, off : off + 32768].rearrange(
                    "p (i q) -> p i q", q=2048
                )

            xT = arena[:, 49152:65536].rearrange("p (c t) -> p c t", t=T)
            d_all = dramp.tile([16, 1024], BF16, name="d_all")
            r_all = dramp.tile([16, 1024], BF16, name="r_all")

            # xT DMA in t-quarters (each contiguous in DRAM) so qkproj(0)
            # starts after ~3 us
            for q4 in range(4):
                nc.sync.dma_start(
                    xT[:, :, q4 * 512 : (q4 + 1) * 512],
                    xT_d[q4],
                )

            # ---- QKV projections ----
            # transient wq/wk tiles live in wp_sb's space (wp is DMA'd only
            # after the last qkproj read; 4 rotating 2KB slots)
            wp_flat = wp_sb[:].rearrange("p a b -> p (a b)")

            def qkproj(j):
                for si, (w_d, b_sb, dstT) in enumerate(
                    ((wq_d, bq_sb, qT), (wk_d, bk_sb, kT))
                ):
                    slot = (2 * j + si) % 4
                    wj = wp_flat[:, slot * 1024 : (slot + 1) * 1024].rearrange(
                        "p (c k) -> p c k", k=P
                    )
                    nc.sync.dma_start(wj[:], w_d[:, j])
                    for u4 in range(T // 512):
                        ps = avmm_ps.tile([P, 1024], F32, tag="avmm", name="qk_ps")
                        for ck in range(CK):
                            nc.tensor.matmul(
                                ps[:, 0:512],
                                wj[:, ck, :],
                                xT[:, ck, u4 * 512 : (u4 + 1) * 512],
                                start=(ck == 0),
                                stop=(ck == CK - 1),
                            )
                        nc.vector.tensor_add(
                            out=dstT[:, j, u4 * 512 : (u4 + 1) * 512],
                            in0=ps[:, 0:512],
                            in1=b_sb[:, j : j + 1].to_broadcast((P, 512)),
                        )

            # wv and bv live in u0 p-slot 2 of the arena: all reads (vproj)
            # are scheduled before score_exp(2, 0) overwrites that slot.
            wv_sb = arena[:, 32768:36864].rearrange("p (c v) -> p c v", v=HC)
            nc.sync.dma_start(wv_sb[:], wv_d[:])
            bv_sb = arena[:, 36864:37376]
            nc.sync.dma_start(bv_sb[:], bv_d[:])

            def vproj(tt_lo, tt_hi):
                for tt in range(tt_lo, tt_hi):
                    ps = avmm_ps.tile([P, 1024], F32, tag="avmm", name="v_ps")
                    for ck in range(CK):
                        nc.tensor.matmul(
                            ps[:, 0:512],
                            xT[:, ck, tt * P : (tt + 1) * P],
                            wv_sb[:, ck, :],
                            start=(ck == 0),
                            stop=(ck == CK - 1),
                        )
                    nc.vector.tensor_add(
                        out=v_ext[:, tt, :, 0:D],
                        in0=ps[:, 0:512].rearrange("p (h d) -> p h d", d=D),
                        in1=bv_sb[:].rearrange("p (h d) -> p h d", d=D),
                    )

            # ---- scores + exp ----
            def score_exp(j, u):
                n_i = 8 * (u + 1)
                pt = p_view(j, u)
                # one PSUM tile per head: double-buffered across units so the
                # next unit's matmuls run while this unit's exp drains. The
                # two heads' matmuls still pair up via row groups 0/64.
                ps2 = [
                    score_ps.tile([P, 1024], F32, tag="score", name=f"sc{hh}")
                    for hh in range(2)
                ]
                for i in range(n_i):
                    d0 = i * P - 1024 * u  # diag block col (within unit)
                    c0 = max(0, d0)
                    jj_diag = 2 * u + d0 // 512 if i >= 8 * u else -1
                    for hh in range(2):
                        hb = hh * D
                        for jj in range(2 * u, 2 * u + 2):
                            if jj < i // 4:
                                continue
                            c0j = (jj - 2 * u) * 512
                            if jj != jj_diag:
                                nc.tensor.matmul(
                                    ps2[hh][:, c0j : c0j + 512],
                                    kT[hb : hb + D, j, i * P : (i + 1) * P],
                                    qT[hb : hb + D, j, jj * 512 : (jj + 1) * 512],
                                    start=True,
                                    stop=True,
                                )
                                continue
                            # diag block: seed [d0, d0+128) with the causal
                            # mask, accumulate scores on top; cols left of the
                            # diag are above-diagonal (never exp'd/streamed),
                            # cols right of it get their own fresh matmul.
                            nc.tensor.matmul(
                                ps2[hh][:, d0 : d0 + P],
                                dmaskT[:],
                                ident_bf[:],
                                start=True,
                                stop=False,
                            )
                            nc.tensor.matmul(
                                ps2[hh][:, d0 : d0 + P],
                                kT[hb : hb + D, j, i * P : (i + 1) * P],
                                qT[hb : hb + D, j, d0 + 1024 * u : d0 + 1024 * u + P],
                                start=False,
                                stop=True,
                            )
                            hi = c0j + 512
                            if d0 + P < hi:
                                nc.tensor.matmul(
                                    ps2[hh][:, d0 + P : hi],
                                    kT[hb : hb + D, j, i * P : (i + 1) * P],
                                    qT[
                                        hb : hb + D,
                                        j,
                                        d0 + P + 1024 * u : hi + 1024 * u,
                                    ],
                                    start=True,
                                    stop=True,
                                )
                    for hh in range(2):
                        nc.scalar.activation(
                            out=pt[:, i, hh * 1024 + c0 : (hh + 1) * 1024],
                            in_=ps2[hh][:, c0:1024],
                            func=mybir.ActivationFunctionType.Exp,
                            scale=SCALE,
                        )

            # ---- attention-value product, v-stationary ----
            # out[0:64] = y^T (unnormalized), out[64] = softmax denominator
            # via the ones column of v_ext; p streams 512 queries per matmul.
            def av_unit(j, u, chunks=None):
                pt = p_view(j, u)
                for c in chunks if chunks is not None else (2 * u, 2 * u + 1):
                    i_max = 4 * (c + 1)
                    qo = (c - 2 * u) * 512
                    av = avmm_ps.tile([P, 1024], F32, tag="avmm", name=f"av{j}{c}")
                    for hh in range(2):
                        h = 2 * j + hh
                        for i in range(i_max):
                            # stream only cols at/after the causal boundary:
                            # p[:, i, :lo] above the diagonal is never written
                            lo = max(qo, i * P - 1024 * u)
                            nc.tensor.matmul(
                                av[0 : D + 1, hh * 512 + lo - qo : hh * 512 + 512],
                                v_ext[:, i, h, :],
                                pt[:, i, hh * 1024 + lo : hh * 1024 + qo + 512],
                                start=(i == 0),
                                stop=(i == i_max - 1),
                            )
                    # Evacuate PSUM immediately (PE never waits on norm):
                    # even head's unnormalized y -> yT rows 0-63 directly,
                    # odd head's + denominator row -> avout staging, then a
                    # DMA partition-shift puts odd y at yT rows 64-127 and
                    # the denominators accumulate in DRAM for batch_norm.
                    cid = 4 * c + j
                    avout = rnp.tile(
                        [P, 1024], BF16, tag="avout", name="avout", bufs=2
                    )
                    nc.vector.tensor_copy(
                        out=yT[0:D, j, c * 512 : (c + 1) * 512],
                        in_=av[0:D, 0:512],
                    )
                    nc.vector.tensor_copy(
                        out=avout[0:D, 512:1024], in_=av[0:D, 512:1024]
                    )
                    nc.vector.tensor_copy(
                        out=avout[D : D + 1, :], in_=av[D : D + 1, :]
                    )
                    nc.sync.dma_start(
                        yT[D:P, j, c * 512 : (c + 1) * 512],
                        avout[0:D, 512:1024],
                    )
                    nc.sync.dma_start(d_all[cid], avout[D : D + 1, :])

            # Batched normalization: one transposing DRAM round-trip turns
            # the [n, 1024] denominator rows into a [128, 8n] column, one
            # lane-parallel reciprocal (DVE recip costs ~9 cyc per FREE
            # element), write back transposed, then broadcast-read each
            # chunk's recips and scale yT in place (rows 0-63 even head,
            # 64-127 odd head).
            def batch_norm(c_lo, c_hi):
                lo, hi = 4 * c_lo, 4 * c_hi
                n = hi - lo
                rb0 = rnp.tile([P, 512], BF16, tag="rb", name="rb")
                dcb = rb0[:, 0:64]
                nc.sync.dma_start(
                    dcb[:, 0 : 8 * n],
                    d_all[lo:hi, :].rearrange("n (c p) -> p (n c)", p=P),
                )
                with nc.allow_low_precision(
                    reason="bf16 1/denom: ~0.4% rel on y, within tolerance"
                ):
                    nc.vector.reciprocal(dcb[:, 0 : 8 * n], dcb[:, 0 : 8 * n])
                nc.sync.dma_start(
                    r_all[lo:hi, :].rearrange("n (c p) -> p (n c)", p=P),
                    dcb[:, 0 : 8 * n],
                )
                for c in range(c_lo, c_hi):
                    for j in range(NP):
                        cid = 4 * c + j
                        rb = rnp.tile([P, 512], BF16, tag="rb", name="rb")
                        nc.sync.dma_start(
                            rb[0:D, :],
                            r_all[cid : cid + 1, 0:512].to_broadcast((D, 512)),
                        )
                        nc.sync.dma_start(
                            rb[D:P, :],
                            r_all[cid : cid + 1, 512:1024].to_broadcast(
                                (D, 512)
                            ),
                        )
                        nc.vector.tensor_mul(
                            out=yT[:, j, c * 512 : (c + 1) * 512],
                            in0=yT[:, j, c * 512 : (c + 1) * 512],
                            in1=rb[:],
                        )

            # ---- c_proj + ReduceScatter ----
            z_dram = dramp.tile([T, C], BF16)
            rs_out = dramp.tile([T // 2, C], BF16)
            rows = T // RS_CHUNKS  # 512
            half = rows // 2  # 256

            def proj_mm(tt, wp_sb, bp_sb):
                for n in range(C // 512):
                    ps = avmm_ps.tile([P, 1024], F32, tag="avmm", name="pj_ps")
                    for cc in range(HC // P):
                        nc.tensor.matmul(
                            ps[:, 0:512],
                            yT[:, cc, tt * P : (tt + 1) * P],
                            wp_sb[:, cc, n * 512 : (n + 1) * 512],
                            start=(cc == 0),
                            stop=(cc == HC // P - 1),
                        )
                    z_sb = zoutp.tile([P, 512], BF16, tag="z", name="z_sb")
                    nc.vector.tensor_add(
                        out=z_sb[:],
                        in0=ps[:, 0:512],
                        in1=bp_sb[:, n * 512 : (n + 1) * 512],
                    )
                    nc.sync.dma_start(
                        z_dram[tt * P : (tt + 1) * P, n * 512 : (n + 1) * 512],
                        z_sb[:],
                    )

            def rs_tail(rc):
                nc.gpsimd.collective_compute(
                    "ReduceScatter",
                    mybir.AluOpType.add,
                    replica_groups=[[0, 1], [2, 3], [4, 5], [6, 7]],
                    ins=[z_dram[rc * rows : (rc + 1) * rows, :].opt()],
                    outs=[rs_out[rc * half : (rc + 1) * half, :].opt()],
                )
                nc.sync.dma_start(
                    out_d[rc * half : (rc + 1) * half, :],
                    rs_out[rc * half : (rc + 1) * half, :],
                )

            def proj_rs(rc, wp_sb, bp_sb):
                tt_per_chunk = TT // RS_CHUNKS
                for tt in range(rc * tt_per_chunk, (rc + 1) * tt_per_chunk):
                    proj_mm(tt, wp_sb, bp_sb)
                rs_tail(rc)

            # ---- schedule ----
            qkproj(0)
            score_exp(0, 0)
            qkproj(1)
            vproj(0, 8)
            score_exp(1, 0)
            av_unit(0, 0)
            qkproj(2)
            vproj(8, 16)
            score_exp(2, 0)
            av_unit(1, 0)
            qkproj(3)
            score_exp(3, 0)
            av_unit(2, 0)
            av_unit(3, 0)

            nc.sync.dma_start(wp_sb[:], wp_d[:])
            bp_sb, bp_free = tc.tile([P, C], mybir.dt.float8e4, name="bp_sb")
            nc.sync.dma_start(bp_sb[:], bp_d[:])

            # u1 phase is ACT(exp)-bound: spread proj(0)/proj(1) tt-slices
            # across the unit boundaries as PE filler.
            score_exp(0, 1)
            batch_norm(0, 2)
            proj_mm(0, wp_sb, bp_sb)
            proj_mm(1, wp_sb, bp_sb)
            score_exp(1, 1)
            proj_mm(2, wp_sb, bp_sb)
            proj_mm(3, wp_sb, bp_sb)
            rs_tail(0)
            av_unit(0, 1)
            proj_mm(4, wp_sb, bp_sb)
            score_exp(2, 1)
            proj_mm(5, wp_sb, bp_sb)
            proj_mm(6, wp_sb, bp_sb)
            av_unit(1, 1)
            proj_mm(7, wp_sb, bp_sb)
            rs_tail(1)
            score_exp(3, 1)
            # chunk-major tail: finish every head pair's chunk 2 first so
            # proj/RS of rows 1024-1535 overlap the chunk-3 AV work
            av_unit(2, 1, chunks=(2,))
            av_unit(3, 1, chunks=(2,))
            batch_norm(2, 3)
            for tt in range(8, 12):
                proj_mm(tt, wp_sb, bp_sb)
            rs_tail(2)
            av_unit(2, 1, chunks=(3,))
            av_unit(3, 1, chunks=(3,))
            batch_norm(3, 4)
            for tt in range(12, 16):
                proj_mm(tt, wp_sb, bp_sb)
            rs_tail(3)
            bp_free()
            wp_free()

            if _DEBUG_DUMP:
                qT_o = nc.dram_tensor("qT_o", [P, NP, T], BF16, kind="ExternalOutput")
                kT_o = nc.dram_tensor("kT_o", [P, NP, T], BF16, kind="ExternalOutput")
                v_o = nc.dram_tensor(
                    "v_o", [P, TT, HL, D + 1], BF16, kind="ExternalOutput"
                )
                yT_o = nc.dram_tensor("yT_o", [P, NP, T], BF16, kind="ExternalOutput")
                ar_o = nc.dram_tensor(
                    "ar_o", [P, 64 * 1024], BF16, kind="ExternalOutput"
                )
                nc.sync.dma_start(qT_o[:], qT[:])
                nc.sync.dma_start(kT_o[:], kT[:])
                nc.sync.dma_start(v_o[:], v_ext[:])
                nc.sync.dma_start(yT_o[:], yT[:])
                nc.sync.dma_start(ar_o[:], arena[:])

    nc.compile()
    return nc


def _in_maps(inputs):
    x = np.asarray(inputs["x"], dtype=np.float32)
    w_attn = np.asarray(inputs["w_attn"], dtype=np.float32)
    b_attn = np.asarray(inputs["b_attn"], dtype=np.float32)
    w_proj = np.asarray(inputs["w_proj"], dtype=np.float32)
    b_proj = np.asarray(inputs["b_proj"], dtype=np.float32)

    maps = []
    for core in range(N_CORES):
        b, g = core // 2, core % 2
        s = g * HC
        # x[b] [T, C] -> xT [q4, ki, ck, t%512] with c = ck*128 + ki,
        # each t-quarter contiguous for clean DMA
        xT = (
            x[b]
            .reshape(4, 512, CK, P)
            .transpose(0, 3, 2, 1)
            .astype(ml_dtypes.bfloat16)
        )
        # [C, HC] -> [ki, j, ko, n] with c = ko*128+ki, qcol = j*128+n
        wq = (
            w_attn[:, s : s + HC]
            .reshape(CK, P, NP, P)
            .transpose(1, 2, 0, 3)
            .astype(ml_dtypes.bfloat16)
        )
        wk = (
            w_attn[:, C + s : C + s + HC]
            .reshape(CK, P, NP, P)
            .transpose(1, 2, 0, 3)
            .astype(ml_dtypes.bfloat16)
        )
        # [C, HC] -> [ki, ko, vcol]
        wv = (
            w_attn[:, 2 * C + s : 2 * C + s + HC]
            .reshape(CK, P, HC)
            .transpose(1, 0, 2)
            .astype(ml_dtypes.bfloat16)
        )
        # [HC, C] -> [ki, ko, co], bf16
        wp = (
            w_proj[s : s + HC, :]
            .reshape(HC // P, P, C)
            .transpose(1, 0, 2)
            .astype(ml_dtypes.bfloat16)
        )
        bq = b_attn[s : s + HC].reshape(NP, P).T
        bk = b_attn[C + s : C + s + HC].reshape(NP, P).T
        bv = np.broadcast_to(
            b_attn[2 * C + s : 2 * C + s + HC], (P, HC)
        ).astype(ml_dtypes.bfloat16)
        bp = (
            np.broadcast_to(b_proj, (P, C)).astype(ml_dtypes.float8_e4m3)
            if g == 0
            else np.zeros((P, C), ml_dtypes.float8_e4m3)
        )
        maps.append(
            {
                "xT": np.ascontiguousarray(xT),
                "wq": np.ascontiguousarray(wq),
                "wk": np.ascontiguousarray(wk),
                "wv": np.ascontiguousarray(wv),
                "wp": np.ascontiguousarray(wp),
                "bq": np.ascontiguousarray(bq),
                "bk": np.ascontiguousarray(bk),
                "bv": np.ascontiguousarray(bv),
                "bp": np.ascontiguousarray(bp),
            }
        )
    return maps


def _run(inputs, trace=False, trace_cores=None):
    if "nc" not in _CACHE:
        _CACHE["nc"] = _build_nc()
    nc = _CACHE["nc"]
    res = run_bass_kernel_spmd(
        nc,
        _in_maps(inputs),
        list(range(N_CORES)),
        trace=trace,
        trace_cores=trace_cores,
    )
    # chunked RS ownership: even core holds rows [512c, 512c+256),
    # odd core holds rows [512c+256, 512c+512), for c = 0..3
    out = np.empty((B, T, C), np.float32)
    rows = T // RS_CHUNKS
    half = rows // 2
    for b in range(B):
        ev = res.results[2 * b]["out"].astype(np.float32)
        od = res.results[2 * b + 1]["out"].astype(np.float32)
        for rc in range(RS_CHUNKS):
            out[b, rc * rows : rc * rows + half] = ev[rc * half : (rc + 1) * half]
            out[b, rc * rows + half : (rc + 1) * rows] = od[
                rc * half : (rc + 1) * half
            ]
    return out, res


def kernel(**inputs):
    out, _ = _run(inputs)
    return out


# revision 47
# speedup vs baseline: 1.7181x; 1.1571x over previous
"""Causal self-attention (B=4, T=2048, C=1024, H=16) on 8 Trainium2 cores.

Sharding: core c -> batch b = c//2, head-group g = c%2 (8 heads each,
tensor-parallel). QKV + attention + c_proj computed per core on its head
slice; partial c_proj outputs of a (b) pair are summed with chunked
on-device ReduceScatters over the T dimension; host reassembles.

v2: x is pre-transposed/cast to bf16 on the host (layout only, like the
weight reshapes), AV uses v-stationary matmuls streaming 512 queries so
the PE array stays busy (HAM warm), softmax denominators ride as a ones
column of v, and the causal mask is applied in-place on PSUM with one
affine_select per diagonal block.

Self-contained: only imports concourse (installed library) + numpy.
"""

import ml_dtypes
import numpy as np

import concourse.mybir as mybir
import concourse.tile as tile
from concourse import bacc
from concourse.bass_utils import run_bass_kernel_spmd
from concourse.masks import make_identity

B, T, C = 4, 2048, 1024
H_TOTAL, D = 16, 64
N_CORES = 8
HL = H_TOTAL // 2  # local heads per core (8)
HC = HL * D  # local head cols (512)
NP = HL // 2  # head pairs (4)
P = 128
TT = T // P  # 16 t-chunks of 128
CK = C // P  # 8 contraction chunks for qkv
RS_CHUNKS = 4
F32 = mybir.dt.float32
BF16 = mybir.dt.bfloat16
MASK_VAL = -480.0  # -60 after the 1/8 attention scale; exp(-60) ~ 0
SCALE = 1.0 / 8.0  # 1/sqrt(D)

_CACHE = {}
_DEBUG_DUMP = False


def _build_nc():
    nc = bacc.Bacc("TRN2", target_bir_lowering=False, debug=False, num_devices=N_CORES)

    # x pre-transposed and cast on host: [ki, ck, t]
    xT_d = nc.dram_tensor("xT", [4, P, CK, 512], BF16, kind="ExternalInput")
    wq_d = nc.dram_tensor("wq", [P, NP, CK, P], BF16, kind="ExternalInput")
    wk_d = nc.dram_tensor("wk", [P, NP, CK, P], BF16, kind="ExternalInput")
    wv_d = nc.dram_tensor("wv", [P, CK, HC], BF16, kind="ExternalInput")
    bq_d = nc.dram_tensor("bq", [P, NP], F32, kind="ExternalInput")
    bk_d = nc.dram_tensor("bk", [P, NP], F32, kind="ExternalInput")
    bv_d = nc.dram_tensor("bv", [P, HC], BF16, kind="ExternalInput")
    wp_d = nc.dram_tensor("wp", [P, HC // P, C], BF16, kind="ExternalInput")
    bp_d = nc.dram_tensor("bp", [P, C], mybir.dt.float8e4, kind="ExternalInput")
    out_d = nc.dram_tensor("out", [T // 2, C], BF16, kind="ExternalOutput")

    with tile.TileContext(nc) as tc:
        with (
            tc.tile_pool(name="const", bufs=1) as constp,
            tc.tile_pool(name="big", bufs=1) as bigp,
            tc.tile_pool(name="rnorm", bufs=1) as rnp,
            tc.tile_pool(name="zout", bufs=1) as zoutp,
            tc.tile_pool(name="score_ps", bufs=1, space="PSUM") as score_ps,
            tc.tile_pool(name="avmm_ps", bufs=2, space="PSUM") as avmm_ps,
            tc.tile_pool(name="dram", bufs=1, space="DRAM") as dramp,
        ):
            # ---- constants ----
            # dmaskT @ ident seeds the diagonal PSUM block with the causal
            # mask on the PE itself (keeps DVE out of the score->exp chain):
            # dmaskT[p, c] = MASK_VAL where p < c, so (dmaskT^T I)[s, c] =
            # dmaskT[c, s] = MASK_VAL where c < s.
            dmaskT = constp.tile([P, P], BF16)
            nc.vector.memset(dmaskT[:], 0.0)
            nc.gpsimd.affine_select(
                out=dmaskT[:],
                in_=dmaskT[:],
                compare_op=mybir.AluOpType.is_ge,
                fill=MASK_VAL,
                base=0,
                pattern=[[-1, P]],
                channel_multiplier=1,
            )
            ident_bf = constp.tile([P, P], BF16)
            make_identity(nc, ident_bf)
            bq_sb = constp.tile([P, NP], F32)
            nc.sync.dma_start(bq_sb[:], bq_d[:])
            bk_sb = constp.tile([P, NP], F32)
            nc.sync.dma_start(bk_sb[:], bk_d[:])
            # reserve wp space early (needs 8KB contiguous; DMA'd later)
            wp_sb, wp_free = tc.tile([P, HC // P, C], BF16, name="wp_sb")

            # ---- persistent activations ----
            qT = bigp.tile([P, NP, T], BF16)  # q^T [qcol, t]
            kT = bigp.tile([P, NP, T], BF16)  # k^T [kcol, t]
            v_ext = bigp.tile([P, TT, HL, D + 1], BF16)  # v with ones col
            nc.vector.memset(v_ext[:, :, :, D : D + 1], 1.0)
            yT = bigp.tile([P, NP, T], BF16)  # y^T [ci, t]
            # p/xT arena: 64K bf16 elems per partition (128 KB).
            #   u0 p-pair slots (8x2048 = 16K elems): j%3 -> [0,16K),[16K,32K),[32K,48K)
            #   xT (8x2048): [48K, 64K)
            #   u1 p-pair slots (16x2048 = 32K elems): j%2 -> [0,32K),[32K,64K)
            arena = bigp.tile([P, 64 * 1024], BF16)

            def p_view(j, u):
                if u == 0:
                    off = 16384 * (j % 3)
                    return arena[:, off : off + 16384].rearrange(
                        "p (i q) -> p i q", q=2048
                    )
                off = 32768 * (j % 2)
                return arena[:, off : off + 32768].rearrange(
                    "p (i q) -> p i q", q=2048
                )

            xT = arena[:, 49152:65536].rearrange("p (c t) -> p c t", t=T)
            d_all = dramp.tile([16, 1024], BF16, name="d_all")

            # xT DMA in t-quarters (each contiguous in DRAM) so qkproj(0)
            # starts after ~3 us
            for q4 in range(4):
                nc.sync.dma_start(
                    xT[:, :, q4 * 512 : (q4 + 1) * 512],
                    xT_d[q4],
                )

            # ---- QKV projections ----
            # transient wq/wk tiles live in wp_sb's space (wp is DMA'd only
            # after the last qkproj read; 4 rotating 2KB slots)
            wp_flat = wp_sb[:].rearrange("p a b -> p (a b)")

            def qkproj(j):
                for si, (w_d, b_sb, dstT) in enumerate(
                    ((wq_d, bq_sb, qT), (wk_d, bk_sb, kT))
                ):
                    slot = (2 * j + si) % 4
                    wj = wp_flat[:, slot * 1024 : (slot + 1) * 1024].rearrange(
                        "p (c k) -> p c k", k=P
                    )
                    nc.sync.dma_start(wj[:], w_d[:, j])
                    for u4 in range(T // 512):
                        ps = avmm_ps.tile([P, 1024], F32, tag="avmm", name="qk_ps")
                        for ck in range(CK):
                            nc.tensor.matmul(
                                ps[:, 0:512],
                                wj[:, ck, :],
                                xT[:, ck, u4 * 512 : (u4 + 1) * 512],
                                start=(ck == 0),
                                stop=(ck == CK - 1),
                            )
                        nc.vector.tensor_add(
                            out=dstT[:, j, u4 * 512 : (u4 + 1) * 512],
                            in0=ps[:, 0:512],
                            in1=b_sb[:, j : j + 1].to_broadcast((P, 512)),
                        )

            # wv and bv live in u0 p-slot 2 of the arena: all reads (vproj)
            # are scheduled before score_exp(2, 0) overwrites that slot.
            wv_sb = arena[:, 32768:36864].rearrange("p (c v) -> p c v", v=HC)
            nc.sync.dma_start(wv_sb[:], wv_d[:])
            bv_sb = arena[:, 36864:37376]
            nc.sync.dma_start(bv_sb[:], bv_d[:])

            def vproj(tt_lo, tt_hi):
                for tt in range(tt_lo, tt_hi):
                    ps = avmm_ps.tile([P, 1024], F32, tag="avmm", name="v_ps")
                    for ck in range(CK):
                        nc.tensor.matmul(
                            ps[:, 0:512],
                            xT[:, ck, tt * P : (tt + 1) * P],
                            wv_sb[:, ck, :],
                            start=(ck == 0),
                            stop=(ck == CK - 1),
                        )
                    nc.vector.tensor_add(
                        out=v_ext[:, tt, :, 0:D],
                        in0=ps[:, 0:512].rearrange("p (h d) -> p h d", d=D),
                        in1=bv_sb[:].rearrange("p (h d) -> p h d", d=D),
                    )

            # ---- scores + exp ----
            def score_exp(j, u):
                n_i = 8 * (u + 1)
                pt = p_view(j, u)
                # one PSUM tile per head: double-buffered across units so the
                # next unit's matmuls run while this unit's exp drains. The
                # two heads' matmuls still pair up via row groups 0/64.
                ps2 = [
                    score_ps.tile([P, 1024], F32, tag="score", name=f"sc{hh}")
                    for hh in range(2)
                ]
                for i in range(n_i):
                    d0 = i * P - 1024 * u  # diag block col (within unit)
                    c0 = max(0, d0)
                    jj_diag = 2 * u + d0 // 512 if i >= 8 * u else -1
                    for hh in range(2):
                        hb = hh * D
                        for jj in range(2 * u, 2 * u + 2):
                            if jj < i // 4:
                                continue
                            c0j = (jj - 2 * u) * 512
                            if jj != jj_diag:
                                nc.tensor.matmul(
                                    ps2[hh][:, c0j : c0j + 512],
                                    kT[hb : hb + D, j, i * P : (i + 1) * P],
                                    qT[hb : hb + D, j, jj * 512 : (jj + 1) * 512],
                                    start=True,
                                    stop=True,
                                )
                                continue
                            # diag block: seed [d0, d0+128) with the causal
                            # mask, accumulate scores on top; cols left of the
                            # diag are above-diagonal (never exp'd/streamed),
                            # cols right of it get their own fresh matmul.
                            nc.tensor.matmul(
                                ps2[hh][:, d0 : d0 + P],
                                dmaskT[:],
                                ident_bf[:],
                                start=True,
                                stop=False,
                            )
                            nc.tensor.matmul(
                                ps2[hh][:, d0 : d0 + P],
                                kT[hb : hb + D, j, i * P : (i + 1) * P],
                                qT[hb : hb + D, j, d0 + 1024 * u : d0 + 1024 * u + P],
                                start=False,
                                stop=True,
                            )
                            hi = c0j + 512
                            if d0 + P < hi:
                                nc.tensor.matmul(
                                    ps2[hh][:, d0 + P : hi],
                                    kT[hb : hb + D, j, i * P : (i + 1) * P],
                                    qT[
                                        hb : hb + D,
                                        j,
                                        d0 + P + 1024 * u : hi + 1024 * u,
                                    ],
                                    start=True,
                                    stop=True,
                                )
                    for hh in range(2):
                        nc.scalar.activation(
                            out=pt[:, i, hh * 1024 + c0 : (hh + 1) * 1024],
                            in_=ps2[hh][:, c0:1024],
                            func=mybir.ActivationFunctionType.Exp,
                            scale=SCALE,
                        )

            # ---- attention-value product, v-stationary ----
            # out[0:64] = y^T (unnormalized), out[64] = softmax denominator
            # via the ones column of v_ext; p streams 512 queries per matmul.
            def av_unit(j, u, chunks=None):
                pt = p_view(j, u)
                for c in chunks if chunks is not None else (2 * u, 2 * u + 1):
                    i_max = 4 * (c + 1)
                    qo = (c - 2 * u) * 512
                    av = avmm_ps.tile([P, 1024], F32, tag="avmm", name=f"av{j}{c}")
                    for hh in range(2):
                        h = 2 * j + hh
                        for i in range(i_max):
                            # stream only cols at/after the causal boundary:
                            # p[:, i, :lo] above the diagonal is never written
                            lo = max(qo, i * P - 1024 * u)
                            nc.tensor.matmul(
                                av[0 : D + 1, hh * 512 + lo - qo : hh * 512 + 512],
                                v_ext[:, i, h, :],
                                pt[:, i, hh * 1024 + lo : hh * 1024 + qo + 512],
                                start=(i == 0),
                                stop=(i == i_max - 1),
                            )
                    # Evacuate PSUM (PE never waits on the norm): y goes to
                    # yT unnormalized (even direct, odd via DMA partition
                    # shift). 1/denominator = exp(-ln(d)) on the scalar
                    # engine (reads the PSUM row directly; ~1.1us per call,
                    # ~2 ULP accuracy), then the recip row round-trips via
                    # DRAM to broadcast across lanes and ONE in-place DVE
                    # multiply normalizes both heads.
                    cid = 4 * c + j
                    avout = rnp.tile(
                        [P, 1024], BF16, tag="avout", name="avout", bufs=2
                    )
                    nc.vector.tensor_copy(
                        out=yT[0:D, j, c * 512 : (c + 1) * 512],
                        in_=av[0:D, 0:512],
                    )
                    nc.vector.tensor_copy(
                        out=avout[0:D, 512:1024], in_=av[0:D, 512:1024]
                    )
                    nc.sync.dma_start(
                        yT[D:P, j, c * 512 : (c + 1) * 512],
                        avout[0:D, 512:1024],
                    )
                    nc.scalar.activation(
                        out=avout[D : D + 1, :],
                        in_=av[D : D + 1, :],
                        func=mybir.ActivationFunctionType.Ln,
                    )
                    nc.scalar.activation(
                        out=avout[D : D + 1, :],
                        in_=avout[D : D + 1, :],
                        func=mybir.ActivationFunctionType.Exp,
                        scale=-1.0,
                    )
                    nc.sync.dma_start(d_all[cid], avout[D : D + 1, :])
                    rb = rnp.tile([P, 512], BF16, tag="rb", name="rb")
                    nc.sync.dma_start(
                        rb[0:D, :],
                        d_all[cid : cid + 1, 0:512].to_broadcast((D, 512)),
                    )
                    nc.sync.dma_start(
                        rb[D:P, :],
                        d_all[cid : cid + 1, 512:1024].to_broadcast((D, 512)),
                    )
                    nc.vector.tensor_mul(
                        out=yT[:, j, c * 512 : (c + 1) * 512],
                        in0=yT[:, j, c * 512 : (c + 1) * 512],
                        in1=rb[:],
                    )

            # ---- c_proj + ReduceScatter ----
            z_dram = dramp.tile([T, C], BF16)
            rs_out = dramp.tile([T // 2, C], BF16)
            rows = T // RS_CHUNKS  # 512
            half = rows // 2  # 256

            def proj_mm(tt, wp_sb, bp_sb):
                for n in range(C // 512):
                    ps = avmm_ps.tile([P, 1024], F32, tag="avmm", name="pj_ps")
                    for cc in range(HC // P):
                        nc.tensor.matmul(
                            ps[:, 0:512],
                            yT[:, cc, tt * P : (tt + 1) * P],
                            wp_sb[:, cc, n * 512 : (n + 1) * 512],
                            start=(cc == 0),
                            stop=(cc == HC // P - 1),
                        )
                    z_sb = zoutp.tile([P, 512], BF16, tag="z", name="z_sb")
                    nc.vector.tensor_add(
                        out=z_sb[:],
                        in0=ps[:, 0:512],
                        in1=bp_sb[:, n * 512 : (n + 1) * 512],
                    )
                    nc.sync.dma_start(
                        z_dram[tt * P : (tt + 1) * P, n * 512 : (n + 1) * 512],
                        z_sb[:],
                    )

            def rs_tail(rc):
                nc.gpsimd.collective_compute(
                    "ReduceScatter",
                    mybir.AluOpType.add,
                    replica_groups=[[0, 1], [2, 3], [4, 5], [6, 7]],
                    ins=[z_dram[rc * rows : (rc + 1) * rows, :].opt()],
                    outs=[rs_out[rc * half : (rc + 1) * half, :].opt()],
                )
                nc.sync.dma_start(
                    out_d[rc * half : (rc + 1) * half, :],
                    rs_out[rc * half : (rc + 1) * half, :],
                )

            def proj_rs(rc, wp_sb, bp_sb):
                tt_per_chunk = TT // RS_CHUNKS
                for tt in range(rc * tt_per_chunk, (rc + 1) * tt_per_chunk):
                    proj_mm(tt, wp_sb, bp_sb)
                rs_tail(rc)

            # ---- schedule ----
            qkproj(0)
            score_exp(0, 0)
            qkproj(1)
            vproj(0, 8)
            score_exp(1, 0)
            av_unit(0, 0)
            qkproj(2)
            vproj(8, 16)
            score_exp(2, 0)
            av_unit(1, 0)
            qkproj(3)
            score_exp(3, 0)
            av_unit(2, 0)
            av_unit(3, 0)

            nc.sync.dma_start(wp_sb[:], wp_d[:])
            bp_sb, bp_free = tc.tile([P, C], mybir.dt.float8e4, name="bp_sb")
            nc.sync.dma_start(bp_sb[:], bp_d[:])

            # u1 phase is ACT(exp)-bound: spread proj(0)/proj(1) tt-slices
            # across the unit boundaries as PE filler.
            score_exp(0, 1)
            proj_mm(0, wp_sb, bp_sb)
            proj_mm(1, wp_sb, bp_sb)
            score_exp(1, 1)
            proj_mm(2, wp_sb, bp_sb)
            proj_mm(3, wp_sb, bp_sb)
            rs_tail(0)
            av_unit(0, 1)
            proj_mm(4, wp_sb, bp_sb)
            score_exp(2, 1)
            proj_mm(5, wp_sb, bp_sb)
            proj_mm(6, wp_sb, bp_sb)
            av_unit(1, 1)
            proj_mm(7, wp_sb, bp_sb)
            rs_tail(1)
            score_exp(3, 1)
            # chunk-major tail: finish every head pair's chunk 2 first so
            # proj/RS of rows 1024-1535 overlap the chunk-3 AV work
            av_unit(2, 1, chunks=(2,))
            av_unit(3, 1, chunks=(2,))
            for tt in range(8, 12):
                proj_mm(tt, wp_sb, bp_sb)
            rs_tail(2)
            av_unit(2, 1, chunks=(3,))
            av_unit(3, 1, chunks=(3,))
            for tt in range(12, 16):
                proj_mm(tt, wp_sb, bp_sb)
            rs_tail(3)
            bp_free()
            wp_free()

            if _DEBUG_DUMP:
                qT_o = nc.dram_tensor("qT_o", [P, NP, T], BF16, kind="ExternalOutput")
                kT_o = nc.dram_tensor("kT_o", [P, NP, T], BF16, kind="ExternalOutput")
                v_o = nc.dram_tensor(
                    "v_o", [P, TT, HL, D + 1], BF16, kind="ExternalOutput"
                )
                yT_o = nc.dram_tensor("yT_o", [P, NP, T], BF16, kind="ExternalOutput")
                ar_o = nc.dram_tensor(
                    "ar_o", [P, 64 * 1024], BF16, kind="ExternalOutput"
                )
                nc.sync.dma_start(qT_o[:], qT[:])
                nc.sync.dma_start(kT_o[:], kT[:])
                nc.sync.dma_start(v_o[:], v_ext[:])
                nc.sync.dma_start(yT_o[:], yT[:])
                nc.sync.dma_start(ar_o[:], arena[:])

    nc.compile()
    return nc


def _in_maps(inputs):
    x = np.asarray(inputs["x"], dtype=np.float32)
    w_attn = np.asarray(inputs["w_attn"], dtype=np.float32)
    b_attn = np.asarray(inputs["b_attn"], dtype=np.float32)
    w_proj = np.asarray(inputs["w_proj"], dtype=np.float32)
    b_proj = np.asarray(inputs["b_proj"], dtype=np.float32)

    maps = []
    for core in range(N_CORES):
        b, g = core // 2, core % 2
        s = g * HC
        # x[b] [T, C] -> xT [q4, ki, ck, t%512] with c = ck*128 + ki,
        # each t-quarter contiguous for clean DMA
        xT = (
            x[b]
            .reshape(4, 512, CK, P)
            .transpose(0, 3, 2, 1)
            .astype(ml_dtypes.bfloat16)
        )
        # [C, HC] -> [ki, j, ko, n] with c = ko*128+ki, qcol = j*128+n
        wq = (
            w_attn[:, s : s + HC]
            .reshape(CK, P, NP, P)
            .transpose(1, 2, 0, 3)
            .astype(ml_dtypes.bfloat16)
        )
        wk = (
            w_attn[:, C + s : C + s + HC]
            .reshape(CK, P, NP, P)
            .transpose(1, 2, 0, 3)
            .astype(ml_dtypes.bfloat16)
        )
        # [C, HC] -> [ki, ko, vcol]
        wv = (
            w_attn[:, 2 * C + s : 2 * C + s + HC]
            .reshape(CK, P, HC)
            .transpose(1, 0, 2)
            .astype(ml_dtypes.bfloat16)
        )
        # [HC, C] -> [ki, ko, co], bf16
        wp = (
            w_proj[s : s + HC, :]
            .reshape(HC // P, P, C)
            .transpose(1, 0, 2)
            .astype(ml_dtypes.bfloat16)
        )
        bq = b_attn[s : s + HC].reshape(NP, P).T
        bk = b_attn[C + s : C + s + HC].reshape(NP, P).T
        bv = np.broadcast_to(
            b_attn[2 * C + s : 2 * C + s + HC], (P, HC)
        ).astype(ml_dtypes.bfloat16)
        bp = (
            np.broadcast_to(b_proj, (P, C)).astype(ml_dtypes.float8_e4m3)
            if g == 0
            else np.zeros((P, C), ml_dtypes.float8_e4m3)
        )
        maps.append(
            {
                "xT": np.ascontiguousarray(xT),
                "wq": np.ascontiguousarray(wq),
                "wk": np.ascontiguousarray(wk),
                "wv": np.ascontiguousarray(wv),
                "wp": np.ascontiguousarray(wp),
                "bq": np.ascontiguousarray(bq),
                "bk": np.ascontiguousarray(bk),
                "bv": np.ascontiguousarray(bv),
                "bp": np.ascontiguousarray(bp),
            }
        )
    return maps


def _run(inputs, trace=False, trace_cores=None):
    if "nc" not in _CACHE:
        _CACHE["nc"] = _build_nc()
    nc = _CACHE["nc"]
    res = run_bass_kernel_spmd(
        nc,
        _in_maps(inputs),
        list(range(N_CORES)),
        trace=trace,
        trace_cores=trace_cores,
    )
    # chunked RS ownership: even core holds rows [512c, 512c+256),
    # odd core holds rows [512c+256, 512c+512), for c = 0..3
    out = np.empty((B, T, C), np.float32)
    rows = T // RS_CHUNKS
    half = rows // 2
    for b in range(B):
        ev = res.results[2 * b]["out"].astype(np.float32)
        od = res.results[2 * b + 1]["out"].astype(np.float32)
        for rc in range(RS_CHUNKS):
            out[b, rc * rows : rc * rows + half] = ev[rc * half : (rc + 1) * half]
            out[b, rc * rows + half : (rc + 1) * rows] = od[
                rc * half : (rc + 1) * half
            ]
    return out, res


def kernel(**inputs):
    out, _ = _run(inputs)
    return out


# revision 48
# speedup vs baseline: 1.8110x; 1.0541x over previous
"""Causal self-attention (B=4, T=2048, C=1024, H=16) on 8 Trainium2 cores.

Sharding: core c -> batch b = c//2, head-group g = c%2 (8 heads each,
tensor-parallel). QKV + attention + c_proj computed per core on its head
slice; partial c_proj outputs of a (b) pair are summed with chunked
on-device ReduceScatters over the T dimension; host reassembles.

v2: x is pre-transposed/cast to bf16 on the host (layout only, like the
weight reshapes), AV uses v-stationary matmuls streaming 512 queries so
the PE array stays busy (HAM warm), softmax denominators ride as a ones
column of v, and the causal mask is applied in-place on PSUM with one
affine_select per diagonal block.

Self-contained: only imports concourse (installed library) + numpy.
"""

import ml_dtypes
import numpy as np

import concourse.mybir as mybir
import concourse.tile as tile
from concourse import bacc
from concourse.bass_utils import run_bass_kernel_spmd
from concourse.masks import make_identity

B, T, C = 4, 2048, 1024
H_TOTAL, D = 16, 64
N_CORES = 8
HL = H_TOTAL // 2  # local heads per core (8)
HC = HL * D  # local head cols (512)
NP = HL // 2  # head pairs (4)
P = 128
TT = T // P  # 16 t-chunks of 128
CK = C // P  # 8 contraction chunks for qkv
RS_CHUNKS = 4
F32 = mybir.dt.float32
BF16 = mybir.dt.bfloat16
MASK_VAL = -480.0  # -60 after the 1/8 attention scale; exp(-60) ~ 0
SCALE = 1.0 / 8.0  # 1/sqrt(D)

_CACHE = {}
_DEBUG_DUMP = False

# Force the ACT table selector onto natural_log_exp_and_others (has BOTH
# ln and exp) instead of thrashing ~1.3us table reloads between the
# exp-only and ln-only sets on every softmax-denominator reciprocal.
import concourse.bacc as _bacc_mod  # noqa: E402

_orig_gat = _bacc_mod.get_activation_tables


def _gat_pinned(arch):
    t = _orig_gat(arch)
    exp_t = mybir.ActivationFunctionType.Exp
    ln_t = mybir.ActivationFunctionType.Ln
    return {
        name: (
            set()
            if name != "natural_log_exp_and_others"
            and (exp_t in fns or ln_t in fns)
            else fns
        )
        for name, fns in t.items()
    }


_bacc_mod.get_activation_tables = _gat_pinned


def _build_nc():
    nc = bacc.Bacc("TRN2", target_bir_lowering=False, debug=False, num_devices=N_CORES)

    # x pre-transposed and cast on host: [ki, ck, t]
    xT_d = nc.dram_tensor("xT", [4, P, CK, 512], BF16, kind="ExternalInput")
    wq_d = nc.dram_tensor("wq", [P, NP, CK, P], BF16, kind="ExternalInput")
    wk_d = nc.dram_tensor("wk", [P, NP, CK, P], BF16, kind="ExternalInput")
    wv_d = nc.dram_tensor("wv", [P, CK, HC], BF16, kind="ExternalInput")
    bq_d = nc.dram_tensor("bq", [P, NP], F32, kind="ExternalInput")
    bk_d = nc.dram_tensor("bk", [P, NP], F32, kind="ExternalInput")
    bv_d = nc.dram_tensor("bv", [P, HC], BF16, kind="ExternalInput")
    wp_d = nc.dram_tensor("wp", [P, HC // P, C], BF16, kind="ExternalInput")
    bp_d = nc.dram_tensor("bp", [P, C], mybir.dt.float8e4, kind="ExternalInput")
    out_d = nc.dram_tensor("out", [T // 2, C], BF16, kind="ExternalOutput")

    with tile.TileContext(nc) as tc:
        with (
            tc.tile_pool(name="const", bufs=1) as constp,
            tc.tile_pool(name="big", bufs=1) as bigp,
            tc.tile_pool(name="rnorm", bufs=1) as rnp,
            tc.tile_pool(name="zout", bufs=1) as zoutp,
            tc.tile_pool(name="score_ps", bufs=1, space="PSUM") as score_ps,
            tc.tile_pool(name="avmm_ps", bufs=2, space="PSUM") as avmm_ps,
            tc.tile_pool(name="dram", bufs=1, space="DRAM") as dramp,
        ):
            # ---- constants ----
            # dmaskT @ ident seeds the diagonal PSUM block with the causal
            # mask on the PE itself (keeps DVE out of the score->exp chain):
            # dmaskT[p, c] = MASK_VAL where p < c, so (dmaskT^T I)[s, c] =
            # dmaskT[c, s] = MASK_VAL where c < s.
            dmaskT = constp.tile([P, P], BF16)
            nc.vector.memset(dmaskT[:], 0.0)
            nc.gpsimd.affine_select(
                out=dmaskT[:],
                in_=dmaskT[:],
                compare_op=mybir.AluOpType.is_ge,
                fill=MASK_VAL,
                base=0,
                pattern=[[-1, P]],
                channel_multiplier=1,
            )
            ident_bf = constp.tile([P, P], BF16)
            make_identity(nc, ident_bf)
            bq_sb = constp.tile([P, NP], F32)
            nc.sync.dma_start(bq_sb[:], bq_d[:])
            bk_sb = constp.tile([P, NP], F32)
            nc.sync.dma_start(bk_sb[:], bk_d[:])
            # reserve wp space early (needs 8KB contiguous; DMA'd later)
            wp_sb, wp_free = tc.tile([P, HC // P, C], BF16, name="wp_sb")

            # ---- persistent activations ----
            qT = bigp.tile([P, NP, T], BF16)  # q^T [qcol, t]
            kT = bigp.tile([P, NP, T], BF16)  # k^T [kcol, t]
            v_ext = bigp.tile([P, TT, HL, D + 1], BF16)  # v with ones col
            nc.vector.memset(v_ext[:, :, :, D : D + 1], 1.0)
            yT = bigp.tile([P, NP, T], BF16)  # y^T [ci, t]
            # p/xT arena: 64K bf16 elems per partition (128 KB).
            #   u0 p-pair slots (8x2048 = 16K elems): j%3 -> [0,16K),[16K,32K),[32K,48K)
            #   xT (8x2048): [48K, 64K)
            #   u1 p-pair slots (16x2048 = 32K elems): j%2 -> [0,32K),[32K,64K)
            arena = bigp.tile([P, 64 * 1024], BF16)

            def p_view(j, u):
                if u == 0:
                    off = 16384 * (j % 3)
                    return arena[:, off : off + 16384].rearrange(
                        "p (i q) -> p i q", q=2048
                    )
                off = 32768 * (j % 2)
                return arena[:, off : off + 32768].rearrange(
                    "p (i q) -> p i q", q=2048
                )

            xT = arena[:, 49152:65536].rearrange("p (c t) -> p c t", t=T)
            d_all = dramp.tile([16, 1024], BF16, name="d_all")

            # xT DMA in t-quarters (each contiguous in DRAM) so qkproj(0)
            # starts after ~3 us
            for q4 in range(4):
                nc.sync.dma_start(
                    xT[:, :, q4 * 512 : (q4 + 1) * 512],
                    xT_d[q4],
                )

            # ---- QKV projections ----
            # transient wq/wk tiles live in wp_sb's space (wp is DMA'd only
            # after the last qkproj read; 4 rotating 2KB slots)
            wp_flat = wp_sb[:].rearrange("p a b -> p (a b)")

            def qkproj(j):
                for si, (w_d, b_sb, dstT) in enumerate(
                    ((wq_d, bq_sb, qT), (wk_d, bk_sb, kT))
                ):
                    slot = (2 * j + si) % 4
                    wj = wp_flat[:, slot * 1024 : (slot + 1) * 1024].rearrange(
                        "p (c k) -> p c k", k=P
                    )
                    nc.sync.dma_start(wj[:], w_d[:, j])
                    for u4 in range(T // 512):
                        ps = avmm_ps.tile([P, 1024], F32, tag="avmm", name="qk_ps")
                        for ck in range(CK):
                            nc.tensor.matmul(
                                ps[:, 0:512],
                                wj[:, ck, :],
                                xT[:, ck, u4 * 512 : (u4 + 1) * 512],
                                start=(ck == 0),
                                stop=(ck == CK - 1),
                            )
                        nc.vector.tensor_add(
                            out=dstT[:, j, u4 * 512 : (u4 + 1) * 512],
                            in0=ps[:, 0:512],
                            in1=b_sb[:, j : j + 1].to_broadcast((P, 512)),
                        )

            # wv and bv live in u0 p-slot 2 of the arena: all reads (vproj)
            # are scheduled before score_exp(2, 0) overwrites that slot.
            wv_sb = arena[:, 32768:36864].rearrange("p (c v) -> p c v", v=HC)
            nc.sync.dma_start(wv_sb[:], wv_d[:])
            bv_sb = arena[:, 36864:37376]
            nc.sync.dma_start(bv_sb[:], bv_d[:])

            def vproj(tt_lo, tt_hi):
                for tt in range(tt_lo, tt_hi):
                    ps = avmm_ps.tile([P, 1024], F32, tag="avmm", name="v_ps")
                    for ck in range(CK):
                        nc.tensor.matmul(
                            ps[:, 0:512],
                            xT[:, ck, tt * P : (tt + 1) * P],
                            wv_sb[:, ck, :],
                            start=(ck == 0),
                            stop=(ck == CK - 1),
                        )
                    nc.vector.tensor_add(
                        out=v_ext[:, tt, :, 0:D],
                        in0=ps[:, 0:512].rearrange("p (h d) -> p h d", d=D),
                        in1=bv_sb[:].rearrange("p (h d) -> p h d", d=D),
                    )

            # ---- scores + exp ----
            def score_exp(j, u):
                n_i = 8 * (u + 1)
                pt = p_view(j, u)
                # one PSUM tile per head: double-buffered across units so the
                # next unit's matmuls run while this unit's exp drains. The
                # two heads' matmuls still pair up via row groups 0/64.
                ps2 = [
                    score_ps.tile([P, 1024], F32, tag="score", name=f"sc{hh}")
                    for hh in range(2)
                ]
                for i in range(n_i):
                    d0 = i * P - 1024 * u  # diag block col (within unit)
                    c0 = max(0, d0)
                    jj_diag = 2 * u + d0 // 512 if i >= 8 * u else -1
                    for hh in range(2):
                        hb = hh * D
                        for jj in range(2 * u, 2 * u + 2):
                            if jj < i // 4:
                                continue
                            c0j = (jj - 2 * u) * 512
                            if jj != jj_diag:
                                nc.tensor.matmul(
                                    ps2[hh][:, c0j : c0j + 512],
                                    kT[hb : hb + D, j, i * P : (i + 1) * P],
                                    qT[hb : hb + D, j, jj * 512 : (jj + 1) * 512],
                                    start=True,
                                    stop=True,
                                )
                                continue
                            # diag block: seed [d0, d0+128) with the causal
                            # mask, accumulate scores on top; cols left of the
                            # diag are above-diagonal (never exp'd/streamed),
                            # cols right of it get their own fresh matmul.
                            nc.tensor.matmul(
                                ps2[hh][:, d0 : d0 + P],
                                dmaskT[:],
                                ident_bf[:],
                                start=True,
                                stop=False,
                            )
                            nc.tensor.matmul(
                                ps2[hh][:, d0 : d0 + P],
                                kT[hb : hb + D, j, i * P : (i + 1) * P],
                                qT[hb : hb + D, j, d0 + 1024 * u : d0 + 1024 * u + P],
                                start=False,
                                stop=True,
                            )
                            hi = c0j + 512
                            if d0 + P < hi:
                                nc.tensor.matmul(
                                    ps2[hh][:, d0 + P : hi],
                                    kT[hb : hb + D, j, i * P : (i + 1) * P],
                                    qT[
                                        hb : hb + D,
                                        j,
                                        d0 + P + 1024 * u : hi + 1024 * u,
                                    ],
                                    start=True,
                                    stop=True,
                                )
                    for hh in range(2):
                        nc.scalar.activation(
                            out=pt[:, i, hh * 1024 + c0 : (hh + 1) * 1024],
                            in_=ps2[hh][:, c0:1024],
                            func=mybir.ActivationFunctionType.Exp,
                            scale=SCALE,
                        )

            # ---- attention-value product, v-stationary ----
            # out[0:64] = y^T (unnormalized), out[64] = softmax denominator
            # via the ones column of v_ext; p streams 512 queries per matmul.
            def av_unit(j, u, chunks=None):
                pt = p_view(j, u)
                for c in chunks if chunks is not None else (2 * u, 2 * u + 1):
                    i_max = 4 * (c + 1)
                    qo = (c - 2 * u) * 512
                    av = avmm_ps.tile([P, 1024], F32, tag="avmm", name=f"av{j}{c}")
                    for hh in range(2):
                        h = 2 * j + hh
                        for i in range(i_max):
                            # stream only cols at/after the causal boundary:
                            # p[:, i, :lo] above the diagonal is never written
                            lo = max(qo, i * P - 1024 * u)
                            nc.tensor.matmul(
                                av[0 : D + 1, hh * 512 + lo - qo : hh * 512 + 512],
                                v_ext[:, i, h, :],
                                pt[:, i, hh * 1024 + lo : hh * 1024 + qo + 512],
                                start=(i == 0),
                                stop=(i == i_max - 1),
                            )
                    # Evacuate PSUM (PE never waits on the norm): y goes to
                    # yT unnormalized (even direct, odd via DMA partition
                    # shift). 1/denominator = exp(-ln(d)) on the scalar
                    # engine (reads the PSUM row directly; ~1.1us per call,
                    # ~2 ULP accuracy), then the recip row round-trips via
                    # DRAM to broadcast across lanes and ONE in-place DVE
                    # multiply normalizes both heads.
                    cid = 4 * c + j
                    avout = rnp.tile(
                        [P, 1024], BF16, tag="avout", name="avout", bufs=2
                    )
                    nc.vector.tensor_copy(
                        out=yT[0:D, j, c * 512 : (c + 1) * 512],
                        in_=av[0:D, 0:512],
                    )
                    nc.vector.tensor_copy(
                        out=avout[0:D, 512:1024], in_=av[0:D, 512:1024]
                    )
                    nc.sync.dma_start(
                        yT[D:P, j, c * 512 : (c + 1) * 512],
                        avout[0:D, 512:1024],
                    )
                    nc.scalar.activation(
                        out=avout[D : D + 1, :],
                        in_=av[D : D + 1, :],
                        func=mybir.ActivationFunctionType.Ln,
                    )
                    nc.scalar.activation(
                        out=avout[D : D + 1, :],
                        in_=avout[D : D + 1, :],
                        func=mybir.ActivationFunctionType.Exp,
                        scale=-1.0,
                    )
                    nc.sync.dma_start(d_all[cid], avout[D : D + 1, :])
                    rb = rnp.tile([P, 512], BF16, tag="rb", name="rb")
                    nc.sync.dma_start(
                        rb[0:D, :],
                        d_all[cid : cid + 1, 0:512].to_broadcast((D, 512)),
                    )
                    nc.sync.dma_start(
                        rb[D:P, :],
                        d_all[cid : cid + 1, 512:1024].to_broadcast((D, 512)),
                    )
                    nc.vector.tensor_mul(
                        out=yT[:, j, c * 512 : (c + 1) * 512],
                        in0=yT[:, j, c * 512 : (c + 1) * 512],
                        in1=rb[:],
                    )

            # ---- c_proj + ReduceScatter ----
            z_dram = dramp.tile([T, C], BF16)
            rs_out = dramp.tile([T // 2, C], BF16)
            rows = T // RS_CHUNKS  # 512
            half = rows // 2  # 256

            def proj_mm(tt, wp_sb, bp_sb):
                for n in range(C // 512):
                    ps = avmm_ps.tile([P, 1024], F32, tag="avmm", name="pj_ps")
                    for cc in range(HC // P):
                        nc.tensor.matmul(
                            ps[:, 0:512],
                            yT[:, cc, tt * P : (tt + 1) * P],
                            wp_sb[:, cc, n * 512 : (n + 1) * 512],
                            start=(cc == 0),
                            stop=(cc == HC // P - 1),
                        )
                    z_sb = zoutp.tile([P, 512], BF16, tag="z", name="z_sb")
                    nc.vector.tensor_add(
                        out=z_sb[:],
                        in0=ps[:, 0:512],
                        in1=bp_sb[:, n * 512 : (n + 1) * 512],
                    )
                    nc.sync.dma_start(
                        z_dram[tt * P : (tt + 1) * P, n * 512 : (n + 1) * 512],
                        z_sb[:],
                    )

            def rs_tail(rc):
                nc.gpsimd.collective_compute(
                    "ReduceScatter",
                    mybir.AluOpType.add,
                    replica_groups=[[0, 1], [2, 3], [4, 5], [6, 7]],
                    ins=[z_dram[rc * rows : (rc + 1) * rows, :].opt()],
                    outs=[rs_out[rc * half : (rc + 1) * half, :].opt()],
                )
                nc.sync.dma_start(
                    out_d[rc * half : (rc + 1) * half, :],
                    rs_out[rc * half : (rc + 1) * half, :],
                )

            def proj_rs(rc, wp_sb, bp_sb):
                tt_per_chunk = TT // RS_CHUNKS
                for tt in range(rc * tt_per_chunk, (rc + 1) * tt_per_chunk):
                    proj_mm(tt, wp_sb, bp_sb)
                rs_tail(rc)

            # ---- schedule ----
            qkproj(0)
            score_exp(0, 0)
            qkproj(1)
            vproj(0, 8)
            score_exp(1, 0)
            av_unit(0, 0)
            qkproj(2)
            vproj(8, 16)
            score_exp(2, 0)
            av_unit(1, 0)
            qkproj(3)
            score_exp(3, 0)
            av_unit(2, 0)
            av_unit(3, 0)

            nc.sync.dma_start(wp_sb[:], wp_d[:])
            bp_sb, bp_free = tc.tile([P, C], mybir.dt.float8e4, name="bp_sb")
            nc.sync.dma_start(bp_sb[:], bp_d[:])

            # u1 phase is ACT(exp)-bound: spread proj(0)/proj(1) tt-slices
            # across the unit boundaries as PE filler.
            score_exp(0, 1)
            proj_mm(0, wp_sb, bp_sb)
            proj_mm(1, wp_sb, bp_sb)
            score_exp(1, 1)
            proj_mm(2, wp_sb, bp_sb)
            proj_mm(3, wp_sb, bp_sb)
            rs_tail(0)
            av_unit(0, 1)
            proj_mm(4, wp_sb, bp_sb)
            score_exp(2, 1)
            proj_mm(5, wp_sb, bp_sb)
            proj_mm(6, wp_sb, bp_sb)
            av_unit(1, 1)
            proj_mm(7, wp_sb, bp_sb)
            rs_tail(1)
            score_exp(3, 1)
            # chunk-major tail: finish every head pair's chunk 2 first so
            # proj/RS of rows 1024-1535 overlap the chunk-3 AV work
            av_unit(2, 1, chunks=(2,))
            av_unit(3, 1, chunks=(2,))
            for tt in range(8, 12):
                proj_mm(tt, wp_sb, bp_sb)
            rs_tail(2)
            av_unit(2, 1, chunks=(3,))
            av_unit(3, 1, chunks=(3,))
            for tt in range(12, 16):
                proj_mm(tt, wp_sb, bp_sb)
            rs_tail(3)
            bp_free()
            wp_free()

            if _DEBUG_DUMP:
                qT_o = nc.dram_tensor("qT_o", [P, NP, T], BF16, kind="ExternalOutput")
                kT_o = nc.dram_tensor("kT_o", [P, NP, T], BF16, kind="ExternalOutput")
                v_o = nc.dram_tensor(
                    "v_o", [P, TT, HL, D + 1], BF16, kind="ExternalOutput"
                )
                yT_o = nc.dram_tensor("yT_o", [P, NP, T], BF16, kind="ExternalOutput")
                ar_o = nc.dram_tensor(
                    "ar_o", [P, 64 * 1024], BF16, kind="ExternalOutput"
                )
                nc.sync.dma_start(qT_o[:], qT[:])
                nc.sync.dma_start(kT_o[:], kT[:])
                nc.sync.dma_start(v_o[:], v_ext[:])
                nc.sync.dma_start(yT_o[:], yT[:])
                nc.sync.dma_start(ar_o[:], arena[:])

    nc.compile()
    return nc


def _in_maps(inputs):
    x = np.asarray(inputs["x"], dtype=np.float32)
    w_attn = np.asarray(inputs["w_attn"], dtype=np.float32)
    b_attn = np.asarray(inputs["b_attn"], dtype=np.float32)
    w_proj = np.asarray(inputs["w_proj"], dtype=np.float32)
    b_proj = np.asarray(inputs["b_proj"], dtype=np.float32)

    maps = []
    for core in range(N_CORES):
        b, g = core // 2, core % 2
        s = g * HC
        # x[b] [T, C] -> xT [q4, ki, ck, t%512] with c = ck*128 + ki,
        # each t-quarter contiguous for clean DMA
        xT = (
            x[b]
            .reshape(4, 512, CK, P)
            .transpose(0, 3, 2, 1)
            .astype(ml_dtypes.bfloat16)
        )
        # [C, HC] -> [ki, j, ko, n] with c = ko*128+ki, qcol = j*128+n
        wq = (
            w_attn[:, s : s + HC]
            .reshape(CK, P, NP, P)
            .transpose(1, 2, 0, 3)
            .astype(ml_dtypes.bfloat16)
        )
        wk = (
            w_attn[:, C + s : C + s + HC]
            .reshape(CK, P, NP, P)
            .transpose(1, 2, 0, 3)
            .astype(ml_dtypes.bfloat16)
        )
        # [C, HC] -> [ki, ko, vcol]
        wv = (
            w_attn[:, 2 * C + s : 2 * C + s + HC]
            .reshape(CK, P, HC)
            .transpose(1, 0, 2)
            .astype(ml_dtypes.bfloat16)
        )
        # [HC, C] -> [ki, ko, co], bf16
        wp = (
            w_proj[s : s + HC, :]
            .reshape(HC // P, P, C)
            .transpose(1, 0, 2)
            .astype(ml_dtypes.bfloat16)
        )
        bq = b_attn[s : s + HC].reshape(NP, P).T
        bk = b_attn[C + s : C + s + HC].reshape(NP, P).T
        bv = np.broadcast_to(
            b_attn[2 * C + s : 2 * C + s + HC], (P, HC)
        ).astype(ml_dtypes.bfloat16)
        bp = (
            np.broadcast_to(b_proj, (P, C)).astype(ml_dtypes.float8_e4m3)
            if g == 0
            else np.zeros((P, C), ml_dtypes.float8_e4m3)
        )
        maps.append(
            {
                "xT": np.ascontiguousarray(xT),
                "wq": np.ascontiguousarray(wq),
                "wk": np.ascontiguousarray(wk),
                "wv": np.ascontiguousarray(wv),
                "wp": np.ascontiguousarray(wp),
                "bq": np.ascontiguousarray(bq),
                "bk": np.ascontiguousarray(bk),
                "bv": np.ascontiguousarray(bv),
                "bp": np.ascontiguousarray(bp),
            }
        )
    return maps


def _run(inputs, trace=False, trace_cores=None):
    if "nc" not in _CACHE:
        _CACHE["nc"] = _build_nc()
    nc = _CACHE["nc"]
    res = run_bass_kernel_spmd(
        nc,
        _in_maps(inputs),
        list(range(N_CORES)),
        trace=trace,
        trace_cores=trace_cores,
    )
    # chunked RS ownership: even core holds rows [512c, 512c+256),
    # odd core holds rows [512c+256, 512c+512), for c = 0..3
    out = np.empty((B, T, C), np.float32)
    rows = T // RS_CHUNKS
    half = rows // 2
    for b in range(B):
        ev = res.results[2 * b]["out"].astype(np.float32)
        od = res.results[2 * b + 1]["out"].astype(np.float32)
        for rc in range(RS_CHUNKS):
            out[b, rc * rows : rc * rows + half] = ev[rc * half : (rc + 1) * half]
            out[b, rc * rows + half : (rc + 1) * rows] = od[
                rc * half : (rc + 1) * half
            ]
    return out, res


def kernel(**inputs):
    out, _ = _run(inputs)
    return out


# revision 51
# speedup vs baseline: 1.8452x; 1.0189x over previous
"""Causal self-attention (B=4, T=2048, C=1024, H=16) on 8 Trainium2 cores.

Sharding: core c -> batch b = c//2, head-group g = c%2 (8 heads each,
tensor-parallel). QKV + attention + c_proj computed per core on its head
slice; partial c_proj outputs of a (b) pair are summed with chunked
on-device ReduceScatters over the T dimension; host reassembles.

v2: x is pre-transposed/cast to bf16 on the host (layout only, like the
weight reshapes), AV uses v-stationary matmuls streaming 512 queries so
the PE array stays busy (HAM warm), softmax denominators ride as a ones
column of v, and the causal mask is applied in-place on PSUM with one
affine_select per diagonal block.

Self-contained: only imports concourse (installed library) + numpy.
"""

import ml_dtypes
import numpy as np

import concourse.mybir as mybir
import concourse.tile as tile
from concourse import bacc
from concourse.bass_utils import run_bass_kernel_spmd
from concourse.masks import make_identity

B, T, C = 4, 2048, 1024
H_TOTAL, D = 16, 64
N_CORES = 8
HL = H_TOTAL // 2  # local heads per core (8)
HC = HL * D  # local head cols (512)
NP = HL // 2  # head pairs (4)
P = 128
TT = T // P  # 16 t-chunks of 128
CK = C // P  # 8 contraction chunks for qkv
RS_CHUNKS = 4
F32 = mybir.dt.float32
BF16 = mybir.dt.bfloat16
MASK_VAL = -480.0  # -60 after the 1/8 attention scale; exp(-60) ~ 0
SCALE = 1.0 / 8.0  # 1/sqrt(D)

_CACHE = {}
_DEBUG_DUMP = False

# Force the ACT table selector onto natural_log_exp_and_others (has BOTH
# ln and exp) instead of thrashing ~1.3us table reloads between the
# exp-only and ln-only sets on every softmax-denominator reciprocal.
import concourse.bacc as _bacc_mod  # noqa: E402

_orig_gat = _bacc_mod.get_activation_tables


def _gat_pinned(arch):
    t = _orig_gat(arch)
    exp_t = mybir.ActivationFunctionType.Exp
    ln_t = mybir.ActivationFunctionType.Ln
    return {
        name: (
            set()
            if name != "natural_log_exp_and_others"
            and (exp_t in fns or ln_t in fns)
            else fns
        )
        for name, fns in t.items()
    }


_bacc_mod.get_activation_tables = _gat_pinned


def _build_nc():
    nc = bacc.Bacc("TRN2", target_bir_lowering=False, debug=False, num_devices=N_CORES)

    # x pre-transposed and cast on host: [ki, ck, t]
    xT_d = nc.dram_tensor("xT", [4, P, CK, 512], BF16, kind="ExternalInput")
    wq_d = nc.dram_tensor("wq", [P, NP, CK, P], BF16, kind="ExternalInput")
    wk_d = nc.dram_tensor("wk", [P, NP, CK, P], BF16, kind="ExternalInput")
    wv_d = nc.dram_tensor("wv", [P, CK, HC], BF16, kind="ExternalInput")
    bq_d = nc.dram_tensor("bq", [P, NP], F32, kind="ExternalInput")
    bk_d = nc.dram_tensor("bk", [P, NP], F32, kind="ExternalInput")
    bv_d = nc.dram_tensor("bv", [P, HC], BF16, kind="ExternalInput")
    wp_d = nc.dram_tensor("wp", [P, HC // P, C], BF16, kind="ExternalInput")
    bp_d = nc.dram_tensor("bp", [P, C], mybir.dt.float8e4, kind="ExternalInput")
    out_d = nc.dram_tensor("out", [T // 2, C], BF16, kind="ExternalOutput")

    with tile.TileContext(nc) as tc:
        with (
            tc.tile_pool(name="const", bufs=1) as constp,
            tc.tile_pool(name="big", bufs=1) as bigp,
            tc.tile_pool(name="rnorm", bufs=1) as rnp,
            tc.tile_pool(name="zout", bufs=1) as zoutp,
            tc.tile_pool(name="score_ps", bufs=1, space="PSUM") as score_ps,
            tc.tile_pool(name="avmm_ps", bufs=2, space="PSUM") as avmm_ps,
            tc.tile_pool(name="dram", bufs=1, space="DRAM") as dramp,
        ):
            # ---- constants ----
            # dmaskT @ ident seeds the diagonal PSUM block with the causal
            # mask on the PE itself (keeps DVE out of the score->exp chain):
            # dmaskT[p, c] = MASK_VAL where p < c, so (dmaskT^T I)[s, c] =
            # dmaskT[c, s] = MASK_VAL where c < s.
            dmaskT = constp.tile([P, P], BF16)
            nc.vector.memset(dmaskT[:], 0.0)
            nc.gpsimd.affine_select(
                out=dmaskT[:],
                in_=dmaskT[:],
                compare_op=mybir.AluOpType.is_ge,
                fill=MASK_VAL,
                base=0,
                pattern=[[-1, P]],
                channel_multiplier=1,
            )
            ident_bf = constp.tile([P, P], BF16)
            make_identity(nc, ident_bf)
            bq_sb = constp.tile([P, NP], F32)
            nc.sync.dma_start(bq_sb[:], bq_d[:])
            bk_sb = constp.tile([P, NP], F32)
            nc.sync.dma_start(bk_sb[:], bk_d[:])
            # reserve wp space early (needs 8KB contiguous; DMA'd later)
            wp_sb, wp_free = tc.tile([P, HC // P, C], BF16, name="wp_sb")

            # ---- persistent activations ----
            qT = bigp.tile([P, NP, T], BF16)  # q^T [qcol, t]
            kT = bigp.tile([P, NP, T], BF16)  # k^T [kcol, t]
            v_ext = bigp.tile([P, TT, HL, D + 1], BF16)  # v with ones col
            nc.vector.memset(v_ext[:, :, :, D : D + 1], 1.0)
            yT = bigp.tile([P, NP, T], BF16)  # y^T [ci, t]
            # p/xT arena: 64K bf16 elems per partition (128 KB).
            #   u0 p-pair slots (8x2048 = 16K elems): j%3 -> [0,16K),[16K,32K),[32K,48K)
            #   xT (8x2048): [48K, 64K)
            #   u1 p-pair slots (16x2048 = 32K elems): j%2 -> [0,32K),[32K,64K)
            arena = bigp.tile([P, 64 * 1024], BF16)

            def p_view(j, u):
                if u == 0:
                    off = 16384 * (j % 3)
                    return arena[:, off : off + 16384].rearrange(
                        "p (i q) -> p i q", q=2048
                    )
                off = 32768 * (j % 2)
                return arena[:, off : off + 32768].rearrange(
                    "p (i q) -> p i q", q=2048
                )

            xT = arena[:, 49152:65536].rearrange("p (c t) -> p c t", t=T)
            d_all = dramp.tile([16, 1024], BF16, name="d_all")

            # xT DMA in t-quarters (each contiguous in DRAM). Only quarter
            # 0 goes ahead of qkproj(0)'s weight loads in the DMA FIFO so
            # the first matmuls start after ~4 us; the rest stream behind.
            nc.sync.dma_start(xT[:, :, 0:512], xT_d[0])

            # ---- QKV projections ----
            # transient wq/wk tiles live in wp_sb's space (wp is DMA'd only
            # after the last qkproj read; 4 rotating 2KB slots)
            wp_flat = wp_sb[:].rearrange("p a b -> p (a b)")

            def qkproj(j, emit_xq=False):
                for si, (w_d, b_sb, dstT) in enumerate(
                    ((wq_d, bq_sb, qT), (wk_d, bk_sb, kT))
                ):
                    slot = (2 * j + si) % 4
                    wj = wp_flat[:, slot * 1024 : (slot + 1) * 1024].rearrange(
                        "p (c k) -> p c k", k=P
                    )
                    nc.sync.dma_start(wj[:], w_d[:, j])
                    for u4 in range(T // 512):
                        if emit_xq and si == 0 and u4 > 0:
                            nc.sync.dma_start(
                                xT[:, :, u4 * 512 : (u4 + 1) * 512], xT_d[u4]
                            )
                        ps = avmm_ps.tile([P, 1024], F32, tag="avmm", name="qk_ps")
                        for ck in range(CK):
                            nc.tensor.matmul(
                                ps[:, 0:512],
                                wj[:, ck, :],
                                xT[:, ck, u4 * 512 : (u4 + 1) * 512],
                                start=(ck == 0),
                                stop=(ck == CK - 1),
                            )
                        nc.vector.tensor_add(
                            out=dstT[:, j, u4 * 512 : (u4 + 1) * 512],
                            in0=ps[:, 0:512],
                            in1=b_sb[:, j : j + 1].to_broadcast((P, 512)),
                        )

            # wv and bv live in u0 p-slot 2 of the arena: all reads (vproj)
            # are scheduled before score_exp(2, 0) overwrites that slot.
            wv_sb = arena[:, 32768:36864].rearrange("p (c v) -> p c v", v=HC)
            bv_sb = arena[:, 36864:37376]

            def load_wv():
                nc.sync.dma_start(wv_sb[:], wv_d[:])
                nc.sync.dma_start(bv_sb[:], bv_d[:])

            def vproj(tt_lo, tt_hi):
                for tt in range(tt_lo, tt_hi):
                    ps = avmm_ps.tile([P, 1024], F32, tag="avmm", name="v_ps")
                    for ck in range(CK):
                        nc.tensor.matmul(
                            ps[:, 0:512],
                            xT[:, ck, tt * P : (tt + 1) * P],
                            wv_sb[:, ck, :],
                            start=(ck == 0),
                            stop=(ck == CK - 1),
                        )
                    nc.vector.tensor_add(
                        out=v_ext[:, tt, :, 0:D],
                        in0=ps[:, 0:512].rearrange("p (h d) -> p h d", d=D),
                        in1=bv_sb[:].rearrange("p (h d) -> p h d", d=D),
                    )

            # ---- scores + exp ----
            def score_exp(j, u):
                n_i = 8 * (u + 1)
                pt = p_view(j, u)
                # one PSUM tile per head: double-buffered across units so the
                # next unit's matmuls run while this unit's exp drains. The
                # two heads' matmuls still pair up via row groups 0/64.
                ps2 = [
                    score_ps.tile([P, 1024], F32, tag="score", name=f"sc{hh}")
                    for hh in range(2)
                ]
                for i in range(n_i):
                    d0 = i * P - 1024 * u  # diag block col (within unit)
                    c0 = max(0, d0)
                    jj_diag = 2 * u + d0 // 512 if i >= 8 * u else -1
                    for hh in range(2):
                        hb = hh * D
                        for jj in range(2 * u, 2 * u + 2):
                            if jj < i // 4:
                                continue
                            c0j = (jj - 2 * u) * 512
                            if jj != jj_diag:
                                nc.tensor.matmul(
                                    ps2[hh][:, c0j : c0j + 512],
                                    kT[hb : hb + D, j, i * P : (i + 1) * P],
                                    qT[hb : hb + D, j, jj * 512 : (jj + 1) * 512],
                                    start=True,
                                    stop=True,
                                )
                                continue
                            # diag block: seed [d0, d0+128) with the causal
                            # mask, accumulate scores on top; cols left of the
                            # diag are above-diagonal (never exp'd/streamed),
                            # cols right of it get their own fresh matmul.
                            nc.tensor.matmul(
                                ps2[hh][:, d0 : d0 + P],
                                dmaskT[:],
                                ident_bf[:],
                                start=True,
                                stop=False,
                            )
                            nc.tensor.matmul(
                                ps2[hh][:, d0 : d0 + P],
                                kT[hb : hb + D, j, i * P : (i + 1) * P],
                                qT[hb : hb + D, j, d0 + 1024 * u : d0 + 1024 * u + P],
                                start=False,
                                stop=True,
                            )
                            hi = c0j + 512
                            if d0 + P < hi:
                                nc.tensor.matmul(
                                    ps2[hh][:, d0 + P : hi],
                                    kT[hb : hb + D, j, i * P : (i + 1) * P],
                                    qT[
                                        hb : hb + D,
                                        j,
                                        d0 + P + 1024 * u : hi + 1024 * u,
                                    ],
                                    start=True,
                                    stop=True,
                                )
                    for hh in range(2):
                        nc.scalar.activation(
                            out=pt[:, i, hh * 1024 + c0 : (hh + 1) * 1024],
                            in_=ps2[hh][:, c0:1024],
                            func=mybir.ActivationFunctionType.Exp,
                            scale=SCALE,
                        )

            # ---- attention-value product, v-stationary ----
            # out[0:64] = y^T (unnormalized), out[64] = softmax denominator
            # via the ones column of v_ext; p streams 512 queries per matmul.
            def av_unit(j, u, chunks=None):
                pt = p_view(j, u)
                for c in chunks if chunks is not None else (2 * u, 2 * u + 1):
                    i_max = 4 * (c + 1)
                    qo = (c - 2 * u) * 512
                    av = avmm_ps.tile([P, 1024], F32, tag="avmm", name=f"av{j}{c}")
                    for hh in range(2):
                        h = 2 * j + hh
                        for i in range(i_max):
                            # stream only cols at/after the causal boundary:
                            # p[:, i, :lo] above the diagonal is never written
                            lo = max(qo, i * P - 1024 * u)
                            nc.tensor.matmul(
                                av[0 : D + 1, hh * 512 + lo - qo : hh * 512 + 512],
                                v_ext[:, i, h, :],
                                pt[:, i, hh * 1024 + lo : hh * 1024 + qo + 512],
                                start=(i == 0),
                                stop=(i == i_max - 1),
                            )
                    # Evacuate PSUM (PE never waits on the norm): y goes to
                    # yT unnormalized (even direct, odd via DMA partition
                    # shift). 1/denominator = exp(-ln(d)) on the scalar
                    # engine (reads the PSUM row directly; ~1.1us per call,
                    # ~2 ULP accuracy), then the recip row round-trips via
                    # DRAM to broadcast across lanes and ONE in-place DVE
                    # multiply normalizes both heads.
                    cid = 4 * c + j
                    avout = rnp.tile(
                        [P, 1024], BF16, tag="avout", name="avout", bufs=2
                    )
                    nc.vector.tensor_copy(
                        out=yT[0:D, j, c * 512 : (c + 1) * 512],
                        in_=av[0:D, 0:512],
                    )
                    nc.vector.tensor_copy(
                        out=avout[0:D, 512:1024], in_=av[0:D, 512:1024]
                    )
                    nc.sync.dma_start(
                        yT[D:P, j, c * 512 : (c + 1) * 512],
                        avout[0:D, 512:1024],
                    )
                    nc.scalar.activation(
                        out=avout[D : D + 1, :],
                        in_=av[D : D + 1, :],
                        func=mybir.ActivationFunctionType.Ln,
                    )
                    nc.scalar.activation(
                        out=avout[D : D + 1, :],
                        in_=avout[D : D + 1, :],
                        func=mybir.ActivationFunctionType.Exp,
                        scale=-1.0,
                    )
                    nc.sync.dma_start(d_all[cid], avout[D : D + 1, :])
                    rb = rnp.tile([P, 512], BF16, tag="rb", name="rb")
                    nc.sync.dma_start(
                        rb[0:D, :],
                        d_all[cid : cid + 1, 0:512].to_broadcast((D, 512)),
                    )
                    nc.sync.dma_start(
                        rb[D:P, :],
                        d_all[cid : cid + 1, 512:1024].to_broadcast((D, 512)),
                    )
                    nc.vector.tensor_mul(
                        out=yT[:, j, c * 512 : (c + 1) * 512],
                        in0=yT[:, j, c * 512 : (c + 1) * 512],
                        in1=rb[:],
                    )

            # ---- c_proj + ReduceScatter ----
            z_dram = dramp.tile([T, C], BF16)
            rs_out = dramp.tile([T // 2, C], BF16)
            rows = T // RS_CHUNKS  # 512
            half = rows // 2  # 256

            def proj_mm(tt, wp_sb, bp_sb):
                for n in range(C // 512):
                    ps = avmm_ps.tile([P, 1024], F32, tag="avmm", name="pj_ps")
                    for cc in range(HC // P):
                        nc.tensor.matmul(
                            ps[:, 0:512],
                            yT[:, cc, tt * P : (tt + 1) * P],
                            wp_sb[:, cc, n * 512 : (n + 1) * 512],
                            start=(cc == 0),
                            stop=(cc == HC // P - 1),
                        )
                    z_sb = zoutp.tile([P, 512], BF16, tag="z", name="z_sb")
                    nc.vector.tensor_add(
                        out=z_sb[:],
                        in0=ps[:, 0:512],
                        in1=bp_sb[:, n * 512 : (n + 1) * 512],
                    )
                    nc.sync.dma_start(
                        z_dram[tt * P : (tt + 1) * P, n * 512 : (n + 1) * 512],
                        z_sb[:],
                    )

            def rs_range(lo, n):
                nc.gpsimd.collective_compute(
                    "ReduceScatter",
                    mybir.AluOpType.add,
                    replica_groups=[[0, 1], [2, 3], [4, 5], [6, 7]],
                    ins=[z_dram[lo : lo + n, :].opt()],
                    outs=[rs_out[lo // 2 : (lo + n) // 2, :].opt()],
                )
                nc.sync.dma_start(
                    out_d[lo // 2 : (lo + n) // 2, :],
                    rs_out[lo // 2 : (lo + n) // 2, :],
                )

            def rs_tail(rc):
                rs_range(rc * rows, rows)

            def proj_rs(rc, wp_sb, bp_sb):
                tt_per_chunk = TT // RS_CHUNKS
                for tt in range(rc * tt_per_chunk, (rc + 1) * tt_per_chunk):
                    proj_mm(tt, wp_sb, bp_sb)
                rs_tail(rc)

            # ---- schedule ----
            qkproj(0, emit_xq=True)
            load_wv()
            score_exp(0, 0)
            qkproj(1)
            vproj(0, 8)
            score_exp(1, 0)
            av_unit(0, 0)
            qkproj(2)
            vproj(8, 16)
            score_exp(2, 0)
            av_unit(1, 0)
            qkproj(3)
            score_exp(3, 0)
            av_unit(2, 0)
            av_unit(3, 0)

            nc.sync.dma_start(wp_sb[:], wp_d[:])
            bp_sb, bp_free = tc.tile([P, C], mybir.dt.float8e4, name="bp_sb")
            nc.sync.dma_start(bp_sb[:], bp_d[:])

            # u1 phase is ACT(exp)-bound: spread proj(0)/proj(1) tt-slices
            # across the unit boundaries as PE filler.
            score_exp(0, 1)
            proj_mm(0, wp_sb, bp_sb)
            proj_mm(1, wp_sb, bp_sb)
            score_exp(1, 1)
            proj_mm(2, wp_sb, bp_sb)
            proj_mm(3, wp_sb, bp_sb)
            rs_tail(0)
            av_unit(0, 1)
            proj_mm(4, wp_sb, bp_sb)
            score_exp(2, 1)
            proj_mm(5, wp_sb, bp_sb)
            proj_mm(6, wp_sb, bp_sb)
            av_unit(1, 1)
            proj_mm(7, wp_sb, bp_sb)
            rs_tail(1)
            score_exp(3, 1)
            # chunk-major tail: finish every head pair's chunk 2 first so
            # proj/RS of rows 1024-1535 overlap the chunk-3 AV work
            av_unit(2, 1, chunks=(2,))
            av_unit(3, 1, chunks=(2,))
            for tt in range(8, 12):
                proj_mm(tt, wp_sb, bp_sb)
            rs_tail(2)
            av_unit(2, 1, chunks=(3,))
            av_unit(3, 1, chunks=(3,))
            for tt in range(12, 14):
                proj_mm(tt, wp_sb, bp_sb)
            rs_range(1536, 256)
            for tt in range(14, 16):
                proj_mm(tt, wp_sb, bp_sb)
            rs_range(1792, 256)
            bp_free()
            wp_free()

            if _DEBUG_DUMP:
                qT_o = nc.dram_tensor("qT_o", [P, NP, T], BF16, kind="ExternalOutput")
                kT_o = nc.dram_tensor("kT_o", [P, NP, T], BF16, kind="ExternalOutput")
                v_o = nc.dram_tensor(
                    "v_o", [P, TT, HL, D + 1], BF16, kind="ExternalOutput"
                )
                yT_o = nc.dram_tensor("yT_o", [P, NP, T], BF16, kind="ExternalOutput")
                ar_o = nc.dram_tensor(
                    "ar_o", [P, 64 * 1024], BF16, kind="ExternalOutput"
                )
                nc.sync.dma_start(qT_o[:], qT[:])
                nc.sync.dma_start(kT_o[:], kT[:])
                nc.sync.dma_start(v_o[:], v_ext[:])
                nc.sync.dma_start(yT_o[:], yT[:])
                nc.sync.dma_start(ar_o[:], arena[:])

    nc.compile()
    return nc


def _in_maps(inputs):
    x = np.asarray(inputs["x"], dtype=np.float32)
    w_attn = np.asarray(inputs["w_attn"], dtype=np.float32)
    b_attn = np.asarray(inputs["b_attn"], dtype=np.float32)
    w_proj = np.asarray(inputs["w_proj"], dtype=np.float32)
    b_proj = np.asarray(inputs["b_proj"], dtype=np.float32)

    maps = []
    for core in range(N_CORES):
        b, g = core // 2, core % 2
        s = g * HC
        # x[b] [T, C] -> xT [q4, ki, ck, t%512] with c = ck*128 + ki,
        # each t-quarter contiguous for clean DMA
        xT = (
            x[b]
            .reshape(4, 512, CK, P)
            .transpose(0, 3, 2, 1)
            .astype(ml_dtypes.bfloat16)
        )
        # [C, HC] -> [ki, j, ko, n] with c = ko*128+ki, qcol = j*128+n
        wq = (
            w_attn[:, s : s + HC]
            .reshape(CK, P, NP, P)
            .transpose(1, 2, 0, 3)
            .astype(ml_dtypes.bfloat16)
        )
        wk = (
            w_attn[:, C + s : C + s + HC]
            .reshape(CK, P, NP, P)
            .transpose(1, 2, 0, 3)
            .astype(ml_dtypes.bfloat16)
        )
        # [C, HC] -> [ki, ko, vcol]
        wv = (
            w_attn[:, 2 * C + s : 2 * C + s + HC]
            .reshape(CK, P, HC)
            .transpose(1, 0, 2)
            .astype(ml_dtypes.bfloat16)
        )
        # [HC, C] -> [ki, ko, co], bf16
        wp = (
            w_proj[s : s + HC, :]
            .reshape(HC // P, P, C)
            .transpose(1, 0, 2)
            .astype(ml_dtypes.bfloat16)
        )
        bq = b_attn[s : s + HC].reshape(NP, P).T
        bk = b_attn[C + s : C + s + HC].reshape(NP, P).T
        bv = np.broadcast_to(
            b_attn[2 * C + s : 2 * C + s + HC], (P, HC)
        ).astype(ml_dtypes.bfloat16)
        bp = (
            np.broadcast_to(b_proj, (P, C)).astype(ml_dtypes.float8_e4m3)
            if g == 0
            else np.zeros((P, C), ml_dtypes.float8_e4m3)
        )
        maps.append(
            {
                "xT": np.ascontiguousarray(xT),
                "wq": np.ascontiguousarray(wq),
                "wk": np.ascontiguousarray(wk),
                "wv": np.ascontiguousarray(wv),
                "wp": np.ascontiguousarray(wp),
                "bq": np.ascontiguousarray(bq),
                "bk": np.ascontiguousarray(bk),
                "bv": np.ascontiguousarray(bv),
                "bp": np.ascontiguousarray(bp),
            }
        )
    return maps


def _run(inputs, trace=False, trace_cores=None):
    if "nc" not in _CACHE:
        _CACHE["nc"] = _build_nc()
    nc = _CACHE["nc"]
    res = run_bass_kernel_spmd(
        nc,
        _in_maps(inputs),
        list(range(N_CORES)),
        trace=trace,
        trace_cores=trace_cores,
    )
    # chunked RS ownership per range (lo, n): even core holds rows
    # [lo, lo+n/2) at rs_out[lo/2:...], odd core the upper half
    out = np.empty((B, T, C), np.float32)
    ranges = [(0, 512), (512, 512), (1024, 512), (1536, 256), (1792, 256)]
    for b in range(B):
        ev = res.results[2 * b]["out"].astype(np.float32)
        od = res.results[2 * b + 1]["out"].astype(np.float32)
        for lo, n in ranges:
            h = n // 2
            out[b, lo : lo + h] = ev[lo // 2 : lo // 2 + h]
            out[b, lo + h : lo + n] = od[lo // 2 : lo // 2 + h]
    return out, res


def kernel(**inputs):
    out, _ = _run(inputs)
    return out
